# revision 1
# baseline (speedup 1.0000x reference)
"""DualResolutionAttention Trainium2 kernel (8 NeuronCores, Bass/Tile).

Sharding: core c -> (batch b = c//4, group g = c%4).
Each core computes local heads {2g, 2g+1} and global heads {2g, 2g+1} over the
full sequence, plus the output channel slice [128g, 128g+128) of each branch.
Two AllGathers within each 4-core batch group: (1) the compressed stream cgT
(each core computes a 128-row slice), (2) the normalized attention outputs
(bf16).  The gate is computed exactly via u = w_proj @ (w_gate[:,0]-w_gate[:,1])
so no cross-core reduction of full proj outputs is needed.

Compute dtypes: float32r (fp32 rounded to 11-bit mantissa; full-rate PE) for
qkv/compress/scores/PV matmuls, bf16 for gathered attention + proj, fp32
accumulation everywhere.
"""
import os
import sys

sys.path.insert(0, "/opt/trn_rl_repo")
os.environ.setdefault("JAX_PLATFORMS", "axon,cpu")

from contextlib import ExitStack

import numpy as np

import concourse.bass as bass
import concourse.mybir as mybir
import concourse.tile as tile
from concourse import bacc
from concourse.bass_utils import run_bass_kernel_spmd
from concourse.masks import make_identity

FP32 = mybir.dt.float32
FP32R = mybir.dt.float32r
BF16 = mybir.dt.bfloat16
AF = mybir.ActivationFunctionType

# Problem constants
B, T, E = 2, 4096, 1024
LD = 512            # local/global stream dim
D = 64              # head dim
HH = 8              # heads per branch
R = 4               # compression ratio
Tc = T // R         # 1024
NCORES = 8
GROUPS = [[0, 1, 2, 3], [4, 5, 6, 7]]

NEG = -1.0e9


def round_fp32r(x):
    """Round fp32 to fp32r (11-bit mantissa, low 12 bits zero), RNE."""
    u = np.ascontiguousarray(np.asarray(x, np.float32)).view(np.uint32)
    lo = u & np.uint32(0xFFF)
    base = u & ~np.uint32(0xFFF)
    lsb = (u >> np.uint32(12)) & np.uint32(1)
    out = base + np.where((lo > 0x800) | ((lo == 0x800) & (lsb == 1)),
                          np.uint32(0x1000), np.uint32(0))
    return out.view(np.float32)


# ---------------------------------------------------------------------------
# Program builder
# ---------------------------------------------------------------------------

def _attention(nc, tc, ctx, name, nQT2, qT, kT, v_sb, attT, stag, stag_rows_per):
    """Attention body: S^T layout, [v|ones] PV, sums via ones row.

    Head-interleaved + software-pipelined kb loop: both heads' independent
    score/exp/PV chains are interleaved and scores run one kb ahead of PV so
    the PE never waits on the ACT exp.
    """
    ps_s = ctx.enter_context(tc.tile_pool(name=f"{name}_ps_s", bufs=1, space="PSUM"))
    ps_o = ctx.enter_context(tc.tile_pool(name=f"{name}_ps_o", bufs=1, space="PSUM"))
    p_pool = ctx.enter_context(tc.tile_pool(name=f"{name}_p", bufs=4))
    hold_pool = ctx.enter_context(tc.tile_pool(name=f"{name}_hold", bufs=2))

    mask_tri = tc_consts["mask_tri"]
    ident_b = tc_consts["ident_b"]

    def emit_scores(h, q2, kb):
        delta = kb - 8 * q2
        t0 = max(0, 128 * delta)
        psum_s = ps_s.tile([128, 1024], FP32, name=f"{name}_s{h}")
        p_sb = p_pool.tile([128, 1024], FP32R, name=f"{name}_pt{h}")
        for qs in (0, 512):
            if qs + 512 <= (t0 // 512) * 512:
                continue
            nc.tensor.matmul(
                psum_s[:, qs:qs + 512],
                kT[64 * h:64 * h + 64, 128 * kb:128 * kb + 128],
                qT[64 * h:64 * h + 64, 1024 * q2 + qs:1024 * q2 + qs + 512],
                start=True, stop=True,
            )
        if delta >= 0:
            # causal mask via PE accumulation (no cross-engine hop)
            nc.tensor.matmul(
                psum_s[:, t0:t0 + 128], ident_b[:], mask_tri[:],
                start=False, stop=True, skip_group_check=True,
            )
        nc.scalar.activation(p_sb[:, t0:1024], psum_s[:, t0:1024], AF.Exp)
        return p_sb, t0

    def emit_pv(h, kb, nkb, psum_o, p_sb, t0):
        for qs in (0, 512):
            lo = max(qs, t0)
            hi = qs + 512
            if lo >= hi:
                continue
            nc.tensor.matmul(
                psum_o[:, lo:hi],
                v_sb[kb][:, 65 * h:65 * h + 65],
                p_sb[:, lo:hi],
                start=(kb == 0), stop=(kb == nkb - 1),
                skip_group_check=True,
            )

    for q2 in range(nQT2):
        nkb = 8 * q2 + 8
        psum_o = [ps_o.tile([65, 1024], FP32, name=f"{name}_o{h}")
                  for h in range(2)]
        pend = {}
        for kb in range(nkb):
            for h in range(2):
                pend[h, kb] = emit_scores(h, q2, kb)
            for h in range(2):
                if kb >= 1:
                    p_sb, t0 = pend.pop((h, kb - 1))
                    emit_pv(h, kb - 1, nkb, psum_o[h], p_sb, t0)
        for h in range(2):
            p_sb, t0 = pend.pop((h, nkb - 1))
            emit_pv(h, nkb - 1, nkb, psum_o[h], p_sb, t0)
        for h in range(2):
            hold = hold_pool.tile([65, 1024], FP32, name=f"{name}_hd")
            nc.vector.tensor_copy(hold[:], psum_o[h][:])
            nc.sync.dma_start(
                out=attT[64 * h:64 * h + 64, 1024 * q2:1024 * q2 + 1024],
                in_=hold[0:64, :],
            )
            r0 = stag_rows_per * (nQT2 * h + q2)
            nc.sync.dma_start(
                out=stag[r0:r0 + stag_rows_per, :],
                in_=hold[64:65, :],
            )


tc_consts = {}


def build_program():
    nc = bacc.Bacc(None, target_bir_lowering=False)

    def inp(name, shape, dt=FP32R):
        return nc.declare_dram_parameter(name, list(shape), dt, isOutput=False)

    # data
    xlt = inp("xlt", [4, 128, T])            # x[b,:,:512].T chunks
    xct = inp("xct", [32, 128, Tc])          # x[b].reshape(Tc, 4096).T chunks
    # weights (f32r unless noted)
    wc = inp("wc", [32, 128, 128])           # compress slice lhsT chunks
    bc = inp("bc", [128, 1], FP32)
    wqk = inp("wqk", [2, 4, 128, 128])       # local per-head [q/8 | k] lhsT
    bqk = inp("bqk", [2, 128, 1], FP32)
    wv = inp("wv", [4, 128, 128])            # local [va | vb] lhsT
    bv = inp("bv", [128, 1], FP32)
    wgqk = inp("wgqk", [2, 4, 128, 128])
    bgqk = inp("bgqk", [2, 128, 1], FP32)
    wgv = inp("wgv", [4, 128, 128])
    bgv = inp("bgv", [128, 1], FP32)
    wpl = inp("wpl", [4, 128, 128], BF16)    # w_lproj d-chunk x c-slice
    bpl = inp("bpl", [128, 128], FP32)       # b_lproj[c-slice] replicated
    wpg = inp("wpg", [4, 128, 128], BF16)
    bpg = inp("bpg", [128, 128], FP32)
    ul = inp("ul", [4, 128, 1], BF16)
    ug = inp("ug", [4, 128, 1], BF16)
    c0h = inp("c0h", [128, 1], FP32)         # 0.5 * gate const
    sel2 = inp("sel2", [2, 128], BF16)       # head-row selector
    repA = inp("repA", [128, 128], FP32)     # x4 expander (even 32-blocks)
    repB = inp("repB", [128, 128], FP32)     # x4 expander (odd 32-blocks)
    repAb = inp("repAb", [128, 128], BF16)
    repBb = inp("repBb", [128, 128], BF16)
    maskt = inp("maskt", [128, 128], BF16)   # strict lower-tri NEG
    out_loc = nc.declare_dram_parameter("out_loc", [T, 128], FP32, isOutput=True)
    out_glob = nc.declare_dram_parameter("out_glob", [T, 128], FP32, isOutput=True)

    with tile.TileContext(nc) as tc:
      with ExitStack() as top:
        dram = top.enter_context(tc.tile_pool(name="dram", bufs=1, space="DRAM"))
        const = top.enter_context(tc.tile_pool(name="const", bufs=1))
        persist = top.enter_context(tc.tile_pool(name="persist", bufs=1))

        # constants
        ident = const.tile([128, 128], FP32, name="ident")
        make_identity(nc, ident[:])
        mask_tri = const.tile([128, 128], BF16, name="mask_tri")
        nc.sync.dma_start(out=mask_tri[:], in_=maskt[:])
        tc_consts["mask_tri"] = mask_tri
        ident_b = const.tile([128, 128], BF16, name="ident_b")
        make_identity(nc, ident_b[:])
        tc_consts["ident_b"] = ident_b
        sel2_sb = const.tile([2, 128], BF16, name="sel2_sb")
        nc.sync.dma_start(out=sel2_sb[:], in_=sel2[:])
        repA_sb = const.tile([128, 128], FP32, name="repA_sb")
        nc.sync.dma_start(out=repA_sb[:], in_=repA[:])
        repB_sb = const.tile([128, 128], FP32, name="repB_sb")
        nc.sync.dma_start(out=repB_sb[:], in_=repB[:])
        repAb_sb = const.tile([128, 128], BF16, name="repAb_sb")
        nc.sync.dma_start(out=repAb_sb[:], in_=repAb[:])
        repBb_sb = const.tile([128, 128], BF16, name="repBb_sb")
        nc.sync.dma_start(out=repBb_sb[:], in_=repBb[:])
        ones1 = const.tile([1, 1], FP32, name="ones1")
        nc.gpsimd.memset(ones1[:], 1.0)
        biases = {}
        for nm, src, shp in (("bc", bc, [128, 1]), ("bv", bv, [128, 1]),
                             ("bgv", bgv, [128, 1]), ("c0h", c0h, [128, 1]),
                             ("bpl", bpl, [128, 128]), ("bpg", bpg, [128, 128])):
            t = const.tile(shp, FP32, name=f"cb_{nm}")
            nc.sync.dma_start(out=t[:], in_=src[:])
            biases[nm] = t
        bqk_sb = []
        bgqk_sb = []
        for h in range(2):
            t = const.tile([128, 1], FP32, name=f"bqk_sb{h}")
            nc.sync.dma_start(out=t[:], in_=bqk[h])
            bqk_sb.append(t)
            t2 = const.tile([128, 1], FP32, name=f"bgqk_sb{h}")
            nc.sync.dma_start(out=t2[:], in_=bgqk[h])
            bgqk_sb.append(t2)

        # ---------------------------------------------- phase A+B scope
        pab = top.enter_context(ExitStack())
        cg_all = [persist.tile([128, Tc], FP32R, name=f"cg_all{i}")
                  for i in range(4)]

        # persistent attention inputs
        qT_l = persist.tile([128, T], FP32R, name="qT_l")
        kT_l = persist.tile([128, T], FP32R, name="kT_l")
        qT_g = persist.tile([128, Tc], FP32R, name="qT_g")
        kT_g = persist.tile([128, Tc], FP32R, name="kT_g")
        v_sb_l = [persist.tile([128, 130], FP32R, name=f"vsb{i}")
                  for i in range(32)]
        v_sb_g = [persist.tile([128, 130], FP32R, name=f"vgsb{i}")
                  for i in range(8)]

        # ------------------------------------------------------ phase B: qkv
        with ExitStack() as pb:
            xlp = pb.enter_context(tc.tile_pool(name="xlt_pool", bufs=1))
            wqp = pb.enter_context(tc.tile_pool(name="wq_pool", bufs=1))
            psB = pb.enter_context(tc.tile_pool(name="psB", bufs=3, space="PSUM"))
            stg = pb.enter_context(tc.tile_pool(name="stgB", bufs=3))
            vTp = pb.enter_context(tc.tile_pool(name="vT_pool", bufs=1))

            xlt_sb = []
            for ch in range(4):
                xt = xlp.tile([128, T], FP32R, name=f"xlt{ch}")
                nc.sync.dma_start(out=xt[:], in_=xlt[ch])
                xlt_sb.append(xt)
            wqk_sb = [[None] * 4 for _ in range(2)]
            wv_sb = [None] * 4
            wgqk_sb = [[None] * 4 for _ in range(2)]
            wgv_sb = [None] * 4
            for h in range(2):
                for ch in range(4):
                    t = wqp.tile([128, 128], FP32R, name=f"wqk{h}_{ch}")
                    nc.sync.dma_start(out=t[:], in_=wqk[h, ch])
                    wqk_sb[h][ch] = t

            for ch in range(4):
                t = wqp.tile([128, 128], FP32R, name=f"wv{ch}")
                nc.sync.dma_start(out=t[:], in_=wv[ch])
                wv_sb[ch] = t


            def qkv_mm(dst_q, dst_k, weights, bias_ap, rhs_chunks, n512, h):
                # packed [q|k] matmul per 512-tile, evict, split via DMA rebase
                for qt in range(n512):
                    ps = psB.tile([128, 512], FP32, name="psB_t")
                    for ch in range(4):
                        nc.tensor.matmul(
                            ps[:], weights[ch][:],
                            rhs_chunks[ch][:, 512 * qt:512 * qt + 512],
                            start=(ch == 0), stop=(ch == 3))
                    st = stg.tile([128, 512], FP32R, name="stB")
                    nc.scalar.activation(st[:], ps[:], AF.Identity, bias=bias_ap)
                    nc.sync.dma_start(
                        out=dst_q[64 * h:64 * h + 64, 512 * qt:512 * qt + 512],
                        in_=st[0:64, :])
                    nc.sync.dma_start(
                        out=dst_k[64 * h:64 * h + 64, 512 * qt:512 * qt + 512],
                        in_=st[64:128, :])

            for h in range(2):
                qkv_mm(qT_l, kT_l, wqk_sb[h], bqk_sb[h][:], xlt_sb, 8, h)

            # v: packed [va|vb] -> vT_ab, then PE-transpose into v_sb + ones
            vT_l = vTp.tile([128, T], FP32R, name="vT_l")
            for (vT_ab, rhs_chunks, wgt, bias_ap, n512) in (
                    (vT_l, xlt_sb, wv_sb, biases["bv"][:], 8),):
                for qt in range(n512):
                    ps = psB.tile([128, 512], FP32, name="psB_t")
                    for ch in range(4):
                        nc.tensor.matmul(
                            ps[:], wgt[ch][:],
                            rhs_chunks[ch][:, 512 * qt:512 * qt + 512],
                            start=(ch == 0), stop=(ch == 3))
                    nc.scalar.activation(vT_ab[:, 512 * qt:512 * qt + 512],
                                         ps[:], AF.Identity, bias=bias_ap)
            for (vT_ab, v_tiles, nkb) in ((vT_l, v_sb_l, 32),):
                for kb in range(nkb):
                    for h in range(2):
                        ps = psB.tile([128, 512], FP32, name="psB_t")
                        nc.tensor.transpose(
                            ps[0:128, 0:64],
                            vT_ab[64 * h:64 * h + 64,
                                  128 * kb:128 * kb + 128].bitcast(FP32),
                            ident[64 * h:64 * h + 64, 64 * h:64 * h + 64])
                        nc.scalar.activation(
                            v_tiles[kb][:, 65 * h:65 * h + 64],
                            ps[0:128, 0:64], AF.Copy)
                    for h in range(2):
                        nc.scalar.activation(
                            v_tiles[kb][:, 65 * h + 64:65 * h + 65],
                            biases["bv"][:, 0:1],
                            AF.Copy, scale=0.0, bias=1.0)

        # compress emitted AFTER local qkv: the xct stream loads during
        # local qkv, and the cgT AllGather overlaps local attention
        xp = pab.enter_context(tc.tile_pool(name="xct_pool", bufs=3))
        wp = pab.enter_context(tc.tile_pool(name="wc_pool", bufs=8))
        cgp = pab.enter_context(tc.tile_pool(name="cg_pool", bufs=1))
        psA = pab.enter_context(tc.tile_pool(name="psA", bufs=1, space="PSUM"))
        cgT = cgp.tile([128, Tc], FP32R, name="cgT")
        ps0 = psA.tile([128, 512], FP32, name="psA_0")
        ps1 = psA.tile([128, 512], FP32, name="psA_1")
        for ch in range(32):
            wt = wp.tile([128, 128], FP32R, name="wc_t")
            nc.sync.dma_start(out=wt[:], in_=wc[ch])
            xt = xp.tile([128, Tc], FP32R, name="xct_t")
            nc.sync.dma_start(out=xt[:], in_=xct[ch])
            nc.tensor.matmul(ps0[:], wt[:], xt[:, 0:512],
                             start=(ch == 0), stop=(ch == 31))
            nc.tensor.matmul(ps1[:], wt[:], xt[:, 512:1024],
                             start=(ch == 0), stop=(ch == 31))
        nc.scalar.activation(cgT[:, 0:512], ps0[:],
                             AF.Identity, bias=biases["bc"][:])
        nc.scalar.activation(cgT[:, 512:1024], ps1[:],
                             AF.Identity, bias=biases["bc"][:])
        cg_contrib = dram.tile([128, Tc], FP32R, name="cg_contrib")
        cg_gathered = dram.tile([512, Tc], FP32R, name="cg_gathered")
        nc.sync.dma_start(out=cg_contrib[:], in_=cgT[:])
        nc.gpsimd.collective_compute(
            "AllGather", mybir.AluOpType.bypass, replica_groups=GROUPS,
            ins=[cg_contrib.opt()], outs=[cg_gathered.opt()],
        )
        for i in range(4):
            nc.sync.dma_start(out=cg_all[i][:],
                              in_=cg_gathered[128 * i:128 * i + 128, :])


        pab.close()

        # ------------------------------------------------------ phase C: attention
        pcs = top.enter_context(ExitStack())
        cpool = pcs.enter_context(tc.tile_pool(name="c_pool", bufs=1))
        attT_l = cpool.tile([128, T], FP32, name="attT_l")
        attT_g = cpool.tile([128, Tc], FP32, name="attT_g")
        stag_l = cpool.tile([128, 64], FP32, name="stag_l")
        stag_g = cpool.tile([32, 64], FP32, name="stag_g")
        attn_l = cpool.tile([128, T], BF16, name="attn_l")
        attn_g = cpool.tile([128, Tc], BF16, name="attn_g")

        def normalize_and_gather(tag, stag, attT, attn, recip_shape, n512,
                                 contrib, gathered):
            with ExitStack() as pn:
                rp = pn.enter_context(tc.tile_pool(name=f"rp_{tag}", bufs=1))
                psN = pn.enter_context(
                    tc.tile_pool(name=f"psN_{tag}", bufs=2, space="PSUM"))
                rec = rp.tile(list(recip_shape), BF16, name=f"rec_{tag}")
                with nc.allow_low_precision(reason="softmax denom bf16"):
                    nc.vector.reciprocal(rec[:], stag[:])
                recip2 = rp.tile([2, 512 * n512], BF16, name=f"recip2_{tag}")
                nblk = recip_shape[0] // 32
                for h in range(2):
                    for q2 in range(nblk):
                        r0 = 16 * (nblk * h + q2)
                        nc.sync.dma_start(
                            out=recip2[h:h + 1, 1024 * q2:1024 * q2 + 1024],
                            in_=rec[r0:r0 + 16, :])
                for qt in range(n512):
                    ps = psN.tile([128, 512], FP32, name=f"psN_{tag}_t")
                    nc.tensor.matmul(ps[:], sel2_sb[:],
                                     recip2[:, 512 * qt:512 * qt + 512],
                                     start=True, stop=True)
                    with nc.allow_low_precision(reason="attnorm bf16"):
                        nc.vector.tensor_mul(
                            attn[:, 512 * qt:512 * qt + 512],
                            attT[:, 512 * qt:512 * qt + 512], ps[:])
                nc.sync.dma_start(out=contrib[:], in_=attn[:])
                nc.gpsimd.collective_compute(
                    "AllGather", mybir.AluOpType.bypass, replica_groups=GROUPS,
                    ins=[contrib.opt()], outs=[gathered.opt()],
                )

        contrib_l = dram.tile([128, T], BF16, name="attnl_contrib")
        gathered_l = dram.tile([512, T], BF16, name="attnl_gathered")
        contrib_g = dram.tile([128, Tc], BF16, name="attng_contrib")
        gathered_g = dram.tile([512, Tc], BF16, name="attng_gathered")

        with ExitStack() as pc1:
            _attention(nc, tc, pc1, "la", 4, qT_l, kT_l, v_sb_l, attT_l,
                       stag_l, 16)
        normalize_and_gather("l", stag_l, attT_l, attn_l, [128, 64], 8,
                             contrib_l, gathered_l)

        # global qkv emitted after local attention: the cgT AllGather overlaps
        # the whole local-attention phase instead of stalling the PE stream.
        with ExitStack() as pg:
            wgp = pg.enter_context(tc.tile_pool(name="wg_pool", bufs=1))
            psG = pg.enter_context(tc.tile_pool(name="psG", bufs=3, space="PSUM"))
            stgG = pg.enter_context(tc.tile_pool(name="stgG", bufs=3))
            vTgp = pg.enter_context(tc.tile_pool(name="vTg_pool", bufs=1))
            wgqk_sb = [[None] * 4 for _ in range(2)]
            wgv_sb = [None] * 4
            for h in range(2):
                for ch in range(4):
                    t2 = wgp.tile([128, 128], FP32R, name=f"wgqk{h}_{ch}")
                    nc.sync.dma_start(out=t2[:], in_=wgqk[h, ch])
                    wgqk_sb[h][ch] = t2
            for ch in range(4):
                t2 = wgp.tile([128, 128], FP32R, name=f"wgv{ch}")
                nc.sync.dma_start(out=t2[:], in_=wgv[ch])
                wgv_sb[ch] = t2
            for h in range(2):
                for qt in range(2):
                    ps = psG.tile([128, 512], FP32, name="psG_t")
                    for ch in range(4):
                        nc.tensor.matmul(
                            ps[:], wgqk_sb[h][ch][:],
                            cg_all[ch][:, 512 * qt:512 * qt + 512],
                            start=(ch == 0), stop=(ch == 3))
                    st = stgG.tile([128, 512], FP32R, name="stG")
                    nc.scalar.activation(st[:], ps[:], AF.Identity,
                                         bias=bgqk_sb[h][:])
                    nc.sync.dma_start(
                        out=qT_g[64 * h:64 * h + 64, 512 * qt:512 * qt + 512],
                        in_=st[0:64, :])
                    nc.sync.dma_start(
                        out=kT_g[64 * h:64 * h + 64, 512 * qt:512 * qt + 512],
                        in_=st[64:128, :])
            vT_g = vTgp.tile([128, Tc], FP32R, name="vT_g")
            for qt in range(2):
                ps = psG.tile([128, 512], FP32, name="psG_t")
                for ch in range(4):
                    nc.tensor.matmul(
                        ps[:], wgv_sb[ch][:],
                        cg_all[ch][:, 512 * qt:512 * qt + 512],
                        start=(ch == 0), stop=(ch == 3))
                nc.scalar.activation(vT_g[:, 512 * qt:512 * qt + 512],
                                     ps[:], AF.Identity, bias=biases["bgv"][:])
            for kb in range(8):
                for h in range(2):
                    ps = psG.tile([128, 512], FP32, name="psG_t")
                    nc.tensor.transpose(
                        ps[0:128, 0:64],
                        vT_g[64 * h:64 * h + 64,
                             128 * kb:128 * kb + 128].bitcast(FP32),
                        ident[64 * h:64 * h + 64, 64 * h:64 * h + 64])
                    nc.scalar.activation(
                        v_sb_g[kb][:, 65 * h:65 * h + 64],
                        ps[0:128, 0:64], AF.Copy)
                for h in range(2):
                    nc.scalar.activation(
                        v_sb_g[kb][:, 65 * h + 64:65 * h + 65],
                        biases["bv"][:, 0:1], AF.Copy, scale=0.0, bias=1.0)

        with ExitStack() as pc2:
            _attention(nc, tc, pc2, "ga", 1, qT_g, kT_g, v_sb_g, attT_g,
                       stag_g, 16)
        normalize_and_gather("g", stag_g, attT_g, attn_g, [32, 64], 2,
                             contrib_g, gathered_g)

        pcs.close()

        # ------------------------------------------------------ phase D: proj+gate
        with ExitStack() as pd:
            ap = pd.enter_context(tc.tile_pool(name="attall_pool", bufs=1))
            wpp = pd.enter_context(tc.tile_pool(name="wp_pool", bufs=1))
            psP = pd.enter_context(tc.tile_pool(name="psP", bufs=2, space="PSUM"))
            psZ = pd.enter_context(tc.tile_pool(name="psZ", bufs=2, space="PSUM"))
            psE = pd.enter_context(tc.tile_pool(name="psE", bufs=2, space="PSUM"))
            gp = pd.enter_context(tc.tile_pool(name="gproj_pool", bufs=1))
            zp = pd.enter_context(tc.tile_pool(name="z_pool", bufs=1))
            outp = pd.enter_context(tc.tile_pool(name="out_pool", bufs=3))

            att_all = []
            attg_all = []
            for i in range(4):
                t = ap.tile([128, T], BF16, name=f"attall{i}")
                nc.sync.dma_start(out=t[:],
                                  in_=gathered_l[128 * i:128 * i + 128, :])
                att_all.append(t)
                t2 = ap.tile([128, Tc], BF16, name=f"attgall{i}")
                nc.sync.dma_start(out=t2[:],
                                  in_=gathered_g[128 * i:128 * i + 128, :])
                attg_all.append(t2)
            wpl_sb = []
            wpg_sb = []
            ul_sb = []
            ug_sb = []
            for ch in range(4):
                t = wpp.tile([128, 128], BF16, name=f"wpl{ch}")
                nc.sync.dma_start(out=t[:], in_=wpl[ch])
                wpl_sb.append(t)
                t = wpp.tile([128, 128], BF16, name=f"wpg{ch}")
                nc.sync.dma_start(out=t[:], in_=wpg[ch])
                wpg_sb.append(t)
                t = wpp.tile([128, 1], BF16, name=f"ul{ch}")
                nc.sync.dma_start(out=t[:], in_=ul[ch])
                ul_sb.append(t)
                t = wpp.tile([128, 1], BF16, name=f"ug{ch}")
                nc.sync.dma_start(out=t[:], in_=ug[ch])
                ug_sb.append(t)

            # gate logits rows
            z_row = zp.tile([1, T + Tc], FP32, name="z_row")
            for (att, u_sb, off, n512) in ((att_all, ul_sb, 0, 8),
                                           (attg_all, ug_sb, T, 2)):
                for qt in range(n512):
                    ps = psZ.tile([1, 512], FP32, name="psZ_t")
                    for ch in range(4):
                        nc.tensor.matmul(
                            ps[:], u_sb[ch][:],
                            att[ch][:, 512 * qt:512 * qt + 512],
                            start=(ch == 0), stop=(ch == 3))
                    nc.scalar.activation(
                        z_row[0:1, off + 512 * qt:off + 512 * qt + 512],
                        ps[:], AF.Copy)
            # z transposes + x4 expand of global part -> zsum [128, 32]
            ps_zsum = psZ.tile([128, 32], FP32, name="ps_zsum", bufs=1)
            ps_zgt = psZ.tile([128, 8], FP32, name="ps_zgt", bufs=1)
            for j in range(8):
                nc.tensor.matmul(ps_zgt[:, j:j + 1],
                                 z_row[0:1, T + 128 * j:T + 128 * j + 128],
                                 ones1[:], start=True, stop=True,
                                 skip_group_check=True)
            zgt_sb = zp.tile([128, 8], FP32, name="zgt_sb")
            nc.scalar.activation(zgt_sb[:], ps_zgt[:], AF.Copy)
            for tb in range(32):
                nc.tensor.matmul(ps_zsum[:, tb:tb + 1],
                                 z_row[0:1, 128 * tb:128 * tb + 128],
                                 ones1[:], start=True, stop=False,
                                 skip_group_check=True)
                base = 64 * ((tb % 4) // 2)
                rep = repA_sb if tb % 2 == 0 else repB_sb
                nc.tensor.matmul(ps_zsum[:, tb:tb + 1],
                                 rep[base:base + 64, :],
                                 zgt_sb[base:base + 64, tb // 4:tb // 4 + 1],
                                 start=False, stop=True,
                                 skip_group_check=True)
            tanh_sb = zp.tile([128, 32], FP32, name="tanh_sb")
            nc.scalar.activation(tanh_sb[:], ps_zsum[:], AF.Tanh,
                                 scale=0.5, bias=biases["c0h"][:])
            g0 = zp.tile([128, 32], FP32, name="g0")
            g1 = zp.tile([128, 32], FP32, name="g1")
            nc.vector.tensor_scalar(g0[:], tanh_sb[:], 0.5, 0.5,
                                    mybir.AluOpType.mult, mybir.AluOpType.add)
            nc.vector.tensor_scalar(g1[:], tanh_sb[:], -0.5, 0.5,
                                    mybir.AluOpType.mult, mybir.AluOpType.add)

            # local proj + gating + out
            for tb in range(32):
                ps = psP.tile([128, 128], FP32, name="psP_t")
                for ch in range(4):
                    nc.tensor.matmul(ps[:], att_all[ch][:, 128 * tb:128 * tb + 128],
                                     wpl_sb[ch][:], start=(ch == 0), stop=(ch == 3))
                tmp = outp.tile([128, 128], FP32, name="tmpl")
                nc.vector.tensor_add(tmp[:], ps[:], biases["bpl"][:])
                o = outp.tile([128, 128], FP32, name="outl")
                nc.vector.tensor_scalar_mul(o[:], tmp[:], g0[:, tb:tb + 1])
                nc.sync.dma_start(out=out_loc[128 * tb:128 * tb + 128, :], in_=o[:])

            # global proj (Tc rows) -> bf16 + bias, then expand x4 + gating
            gproj_sb = []
            for tbg in range(8):
                ps = psP.tile([128, 128], FP32, name="psP_t")
                for ch in range(4):
                    nc.tensor.matmul(ps[:],
                                     attg_all[ch][:, 128 * tbg:128 * tbg + 128],
                                     wpg_sb[ch][:], start=(ch == 0), stop=(ch == 3))
                gt = gp.tile([128, 128], BF16, name=f"gproj{tbg}")
                with nc.allow_low_precision(reason="gproj bf16 for expand"):
                    nc.vector.tensor_add(gt[:], ps[:], biases["bpg"][:])
                gproj_sb.append(gt)
            for tb in range(32):
                ps = psE.tile([128, 128], FP32, name="psE_t")
                base = 64 * ((tb % 4) // 2)
                rep = repAb_sb if tb % 2 == 0 else repBb_sb
                nc.tensor.matmul(ps[:], rep[base:base + 64, :],
                                 gproj_sb[tb // 4][base:base + 64, :],
                                 start=True, stop=True)
                o = outp.tile([128, 128], FP32, name="outg")
                nc.vector.tensor_scalar_mul(o[:], ps[:], g1[:, tb:tb + 1])
                nc.sync.dma_start(out=out_glob[128 * tb:128 * tb + 128, :], in_=o[:])

    nc.finalize()
    return nc


# ---------------------------------------------------------------------------
# Host side
# ---------------------------------------------------------------------------

_NC_CACHE = []


def _get_program():
    if not _NC_CACHE:
        _NC_CACHE.append(build_program())
    return _NC_CACHE[0]


def _prep_inputs(x, w_lqkv, b_lqkv, w_gqkv, b_gqkv, w_comp, b_comp,
                 w_lproj, b_lproj, w_gproj, b_gproj, w_gate, b_gate):
    f32 = np.float32
    wd = (w_gate[:, 0] - w_gate[:, 1]).astype(f32)
    u_l = (w_lproj @ wd[:LD]).astype(f32)
    u_g = (w_gproj @ wd[LD:]).astype(f32)
    c0 = float(b_lproj @ wd[:LD] + b_gproj @ wd[LD:] + b_gate[0] - b_gate[1])

    mask_tri = np.where(np.arange(128)[None, :] >= np.arange(128)[:, None],
                        0.0, NEG).astype(f32)
    # expander matrices: out row p (within 128-block) <- source row j (within
    # a 64-row block).  E0: p//4 == j (first 32 source rows); E1: p//4 == j-32.
    e0 = np.zeros((64, 128), f32)
    e0[np.arange(128) // 4, np.arange(128)] = 1.0
    e1 = np.zeros((64, 128), f32)
    e1[32 + np.arange(128) // 4, np.arange(128)] = 1.0
    repA_ = np.concatenate([e0, e0], axis=0)
    repB_ = np.concatenate([e1, e1], axis=0)
    sel2 = np.zeros((2, 128), f32)
    sel2[0, 0:64] = 1.0
    sel2[1, 64:128] = 1.0

    def head_qk(wqkv, bqkv, h):
        wq = wqkv[:, D * h:D * h + D] / 8.0
        bq = bqkv[D * h:D * h + D] / 8.0
        wk = wqkv[:, LD + D * h:LD + D * h + D]
        bk = bqkv[LD + D * h:LD + D * h + D]
        w = np.concatenate([wq, wk], axis=1)          # [512, 128]
        b = np.concatenate([bq, bk])                  # [128]
        return (round_fp32r(w.reshape(4, 128, 128)),
                b.astype(f32).reshape(128, 1))

    def head_v(wqkv, bqkv, ha, hb):
        w = np.concatenate([wqkv[:, 2 * LD + D * ha:2 * LD + D * ha + D],
                            wqkv[:, 2 * LD + D * hb:2 * LD + D * hb + D]], axis=1)
        b = np.concatenate([bqkv[2 * LD + D * ha:2 * LD + D * ha + D],
                            bqkv[2 * LD + D * hb:2 * LD + D * hb + D]])
        return round_fp32r(w.reshape(4, 128, 128)), b.astype(f32).reshape(128, 1)

    in_maps = []
    for core in range(NCORES):
        b_idx, g = core // 4, core % 4
        ha, hb = 2 * g, 2 * g + 1
        cs = slice(128 * g, 128 * g + 128)

        xlt = round_fp32r(np.ascontiguousarray(x[b_idx, :, :LD].T)
                          .reshape(4, 128, T))
        xct = round_fp32r(np.ascontiguousarray(
            x[b_idx].reshape(Tc, R * E).T).reshape(32, 128, Tc))
        wc_s = round_fp32r(np.ascontiguousarray(
            w_comp[:, LD + 128 * g:LD + 128 * g + 128]).reshape(32, 128, 128))
        bc_s = b_comp[LD + 128 * g:LD + 128 * g + 128].astype(f32).reshape(128, 1)

        wqk_a, bqk_a = head_qk(w_lqkv, b_lqkv, ha)
        wqk_b, bqk_b = head_qk(w_lqkv, b_lqkv, hb)
        wv_s, bv_s = head_v(w_lqkv, b_lqkv, ha, hb)
        wgqk_a, bgqk_a = head_qk(w_gqkv, b_gqkv, ha)
        wgqk_b, bgqk_b = head_qk(w_gqkv, b_gqkv, hb)
        wgv_s, bgv_s = head_v(w_gqkv, b_gqkv, ha, hb)

        in_maps.append({
            "xlt": xlt, "xct": xct, "wc": wc_s, "bc": bc_s,
            "wqk": np.stack([wqk_a, wqk_b]),
            "bqk": np.stack([bqk_a, bqk_b]),
            "wv": wv_s, "bv": bv_s,
            "wgqk": np.stack([wgqk_a, wgqk_b]),
            "bgqk": np.stack([bgqk_a, bgqk_b]),
            "wgv": wgv_s, "bgv": bgv_s,
            "wpl": w_lproj[:, cs].reshape(4, 128, 128).astype(np.float32),
            "bpl": np.tile(b_lproj[cs].astype(f32), (128, 1)),
            "wpg": w_gproj[:, cs].reshape(4, 128, 128).astype(np.float32),
            "bpg": np.tile(b_gproj[cs].astype(f32), (128, 1)),
            "ul": u_l.reshape(4, 128, 1),
            "ug": u_g.reshape(4, 128, 1),
            "c0h": np.full((128, 1), 0.5 * c0, f32),
            "sel2": sel2, "repA": repA_, "repB": repB_,
            "repAb": repA_, "repBb": repB_,
            "maskt": mask_tri,
        })

    # cast bf16-declared params host-side (run path converts via dtype of param)
    import ml_dtypes
    for m in in_maps:
        for k in ("wpl", "wpg", "ul", "ug", "sel2", "repAb", "repBb", "maskt"):
            m[k] = m[k].astype(ml_dtypes.bfloat16)
    return in_maps


def _run(in_maps, trace=False):
    nc = _get_program()
    return run_bass_kernel_spmd(nc, in_maps, list(range(NCORES)), trace=trace)


def kernel(**inputs):
    in_maps = _prep_inputs(**inputs)
    res = _run(in_maps)
    return assemble(res.results)


def assemble(results):
    out = np.empty((B, T, E), np.float32)
    for core in range(NCORES):
        b_idx, g = core // 4, core % 4
        out[b_idx, :, 128 * g:128 * g + 128] = results[core]["out_loc"]
        out[b_idx, :, LD + 128 * g:LD + 128 * g + 128] = results[core]["out_glob"]
    return out


def kernel_traced(**inputs):
    """test.py helper: returns (output, BassKernelResults with timing)."""
    in_maps = _prep_inputs(**inputs)
    res = _run(in_maps, trace=True)
    return assemble(res.results), res



# revision 20
# speedup vs baseline: 1.2705x; 1.2705x over previous
"""DualResolutionAttention Trainium2 kernel (8 NeuronCores, Bass/Tile).

Sharding: core c -> (batch b = c//4, group g = c%4).
Each core computes local heads {2g, 2g+1} and global heads {2g, 2g+1} over the
full sequence, plus the output channel slice [128g, 128g+128) of each branch.
Three AllGathers within each 4-core batch group: (1) compressed stream cgT,
(2) normalized local attention (fp16), (3) normalized global attention (fp16).

v2 design (vs baseline): fp16 compute everywhere (FWL weight loads, 2x less
DMA/SBUF), V computed directly in [token, vdim] layout (no PE transposes),
q/k evicted straight from PSUM to qT/kT (packed per-head weights), gate logits
folded into the projection matmuls as a 129th output column, masks via 64-row
identity matmuls (no PE tiling-mode switch inside attention).
"""
import os
import sys

sys.path.insert(0, "/opt/trn_rl_repo")
os.environ.setdefault("JAX_PLATFORMS", "axon,cpu")

from contextlib import ExitStack

import numpy as np

import concourse.bass as bass
import concourse.mybir as mybir
import concourse.tile as tile
from concourse import bacc
from concourse.bass_utils import run_bass_kernel_spmd
from concourse.masks import make_identity

FP32 = mybir.dt.float32
FP16 = mybir.dt.float16
AF = mybir.ActivationFunctionType

# Problem constants
B, T, E = 2, 4096, 1024
LD = 512            # local/global stream dim
D = 64              # head dim
HH = 8              # heads per branch
R = 4               # compression ratio
Tc = T // R         # 1024
NCORES = 8
GROUPS = [[0, 1, 2, 3], [4, 5, 6, 7]]

NEG = -30000.0      # fp16-safe mask value


# ---------------------------------------------------------------------------
# Program builder
# ---------------------------------------------------------------------------

def _attention(nc, tc, ctx, name, nQT2, qT, kT, v_sb, comb, rec, consts):
    """Attention body: S^T layout scores, [v|ones] PV with denominator row.

    Head-interleaved, kb loop software-pipelined (scores one kb ahead of PV).
    Scores run as 64x128 row-tiled matmuls (heads packed in partition halves);
    causal mask applied via two 64-row identity matmuls (same tiling mode).
    comb[h] is a [65, n512*512] fp16 tile: rows 0:64 = unnormalized attT,
    row 64 = softmax denominator.  rec[h] [1, n512*512] gets 1/denominator
    (denominator hops partitions 64->0 via a tiny SBUF DMA).
    """
    ps_s = ctx.enter_context(tc.tile_pool(name=f"{name}_ps_s", bufs=1, space="PSUM"))
    ps_o = ctx.enter_context(tc.tile_pool(name=f"{name}_ps_o", bufs=1, space="PSUM"))
    p_pool = ctx.enter_context(tc.tile_pool(name=f"{name}_p", bufs=4))
    dnp = ctx.enter_context(tc.tile_pool(name=f"{name}_dn", bufs=2))

    mask_tri = consts["mask_tri"]
    ident = consts["ident"]

    def emit_scores(h, q2, kb):
        delta = kb - 8 * q2
        t0 = max(0, 128 * delta)
        psum_s = ps_s.tile([128, 1024], FP32, name=f"{name}_s{h}")
        p_sb = p_pool.tile([128, 1024], FP16, name=f"{name}_pt")
        for qs in (0, 512):
            if qs + 512 <= (t0 // 512) * 512:
                continue
            nc.tensor.matmul(
                psum_s[:, qs:qs + 512],
                kT[64 * h:64 * h + 64, 128 * kb:128 * kb + 128],
                qT[64 * h:64 * h + 64, 1024 * q2 + qs:1024 * q2 + qs + 512],
                start=True, stop=True,
            )
        if delta >= 0:
            # causal mask via PE accumulation (no cross-engine hop)
            nc.tensor.matmul(
                psum_s[:, t0:t0 + 128], ident[:], mask_tri[:],
                start=False, stop=True, skip_group_check=True,
            )
        nc.scalar.activation(p_sb[:, t0:1024], psum_s[:, t0:1024], AF.Exp)
        return p_sb, t0

    def emit_pv(h, kb, nkb, psum_o, p_sb, t0):
        for qs in (0, 512):
            lo = max(qs, t0)
            hi = qs + 512
            if lo >= hi:
                continue
            nc.tensor.matmul(
                psum_o[:, lo:hi],
                v_sb[kb][:, 65 * h:65 * h + 65],
                p_sb[:, lo:hi],
                start=(kb == 0), stop=(kb == nkb - 1),
                skip_group_check=True,
            )

    for q2 in range(nQT2):
        nkb = 8 * q2 + 8
        psum_o = [ps_o.tile([65, 1024], FP32, name=f"{name}_o{h}")
                  for h in range(2)]
        pend = {}
        for kb in range(nkb):
            for h in range(2):
                pend[h, kb] = emit_scores(h, q2, kb)
            for h in range(2):
                if kb >= 1:
                    p_sb, t0 = pend.pop((h, kb - 1))
                    emit_pv(h, kb - 1, nkb, psum_o[h], p_sb, t0)
        for h in range(2):
            p_sb, t0 = pend.pop((h, nkb - 1))
            emit_pv(h, nkb - 1, nkb, psum_o[h], p_sb, t0)
        for h in range(2):
            # one copy evicts both attT rows and the denominator row
            with nc.allow_low_precision(reason="att fp16"):
                nc.vector.tensor_copy(
                    comb[h][:, 1024 * q2:1024 * q2 + 1024], psum_o[h][:])
            # denominator: partition 64 -> 0 via SBUF DMA, then reciprocal
            dh = dnp.tile([1, 1024], FP16, name=f"{name}_dh")
            nc.sync.dma_start(
                out=dh[:], in_=comb[h][64:65, 1024 * q2:1024 * q2 + 1024])
            with nc.allow_low_precision(reason="softmax denom fp16"):
                nc.vector.reciprocal(
                    rec[h][0:1, 1024 * q2:1024 * q2 + 1024], dh[:])


def _normalize_and_gather(nc, tc, ctx, name, n512, comb, rec, ones2,
                          contrib, gathered):
    """bcast rec per head via K=1 ones matmul; contrib = att * rec; gather."""
    psN = ctx.enter_context(tc.tile_pool(name=f"psN_{name}", bufs=2, space="PSUM"))
    ap = ctx.enter_context(tc.tile_pool(name=f"an_{name}", bufs=1))

    ncols = 512 * n512
    attn = [ap.tile([64, ncols], FP16, name=f"attn_{name}{h}") for h in range(2)]
    for h in range(2):
        for qt in range(n512):
            ps = psN.tile([128, 512], FP32, name=f"psN_{name}_t")
            nc.tensor.matmul(ps[:], ones2[0:1, :],
                             rec[h][0:1, 512 * qt:512 * qt + 512],
                             start=True, stop=True)
            with nc.allow_low_precision(reason="attnorm fp16"):
                nc.vector.tensor_mul(
                    attn[h][:, 512 * qt:512 * qt + 512],
                    comb[h][0:64, 512 * qt:512 * qt + 512], ps[0:64, :])
        nc.sync.dma_start(out=contrib[64 * h:64 * h + 64, :], in_=attn[h][:])
    nc.gpsimd.collective_compute(
        "AllGather", mybir.AluOpType.bypass, replica_groups=GROUPS,
        ins=[contrib.opt()], outs=[gathered.opt()],
    )


def build_program():
    nc = bacc.Bacc(None, target_bir_lowering=False)

    def inp(name, shape, dt=FP16):
        return nc.declare_dram_parameter(name, list(shape), dt, isOutput=False)

    # data
    xlt = inp("xlt", [4, 128, T])            # x[b,:,:512].T chunks
    xct = inp("xct", [32, 128, Tc])          # x[b].reshape(Tc,4096).T chunks
    # weights
    wq = inp("wq", [4, 128, 128])            # [qA|qB] lhsT chunks (scaled 1/8)
    bq = inp("bq", [128, 1], FP32)
    wk = inp("wk", [4, 128, 128])
    bk = inp("bk", [128, 1], FP32)
    wv = inp("wv", [4, 128, 128])            # [vA|vB] (rhs for v-direct)
    bvb = inp("bvb", [128, 128])             # [bvA|bvB] replicated to 128 rows
    wgq = inp("wgq", [4, 128, 128])
    bgq = inp("bgq", [128, 1], FP32)
    wgk = inp("wgk", [4, 128, 128])
    bgk = inp("bgk", [128, 1], FP32)
    wgv = inp("wgv", [4, 128, 128])
    bgvb = inp("bgvb", [128, 128])
    wc = inp("wc", [32, 128, 128])           # compress slice lhsT chunks
    bc = inp("bc", [128, 1], FP32)
    wplz = inp("wplz", [4, 128, 129])        # [w_lproj[:,cs] | u_l] chunks
    bplzb = inp("bplzb", [128, 129])         # [b_lproj[cs] | c0] replicated
    wpgz = inp("wpgz", [4, 128, 129])
    bpgzb = inp("bpgzb", [128, 129])
    repA = inp("repA", [128, 128])           # x4 expander (even 32-blocks)
    repB = inp("repB", [128, 128])           # x4 expander (odd 32-blocks)
    maskt = inp("maskt", [128, 128])         # strict lower-tri NEG
    out_loc = nc.declare_dram_parameter("out_loc", [T, 128], FP16, isOutput=True)
    out_glob = nc.declare_dram_parameter("out_glob", [T, 128], FP16, isOutput=True)

    with tile.TileContext(nc) as tc:
      with ExitStack() as top:
        dram = top.enter_context(tc.tile_pool(name="dram", bufs=1, space="DRAM"))
        const = top.enter_context(tc.tile_pool(name="const", bufs=1))
        persist = top.enter_context(tc.tile_pool(name="persist", bufs=1))

        # constants
        ident = const.tile([128, 128], FP16, name="ident")
        make_identity(nc, ident[:])
        mask_tri = const.tile([128, 128], FP16, name="mask_tri")
        nc.sync.dma_start(out=mask_tri[:], in_=maskt[:])
        repA_sb = const.tile([128, 128], FP16, name="repA_sb")
        nc.sync.dma_start(out=repA_sb[:], in_=repA[:])
        repB_sb = const.tile([128, 128], FP16, name="repB_sb")
        nc.sync.dma_start(out=repB_sb[:], in_=repB[:])
        ones2 = const.tile([1, 128], FP16, name="ones2")
        nc.gpsimd.memset(ones2[:], 1.0)
        consts = {"mask_tri": mask_tri, "ident": ident}
        biases = {}
        for nm, src in (("bq", bq), ("bk", bk), ("bgq", bgq), ("bgk", bgk),
                        ("bc", bc)):
            t = const.tile([128, 1], FP32, name=f"cb_{nm}")
            nc.sync.dma_start(out=t[:], in_=src[:])
            biases[nm] = t
        brows = {}
        for nm, src, w in (("bvb", bvb, 128), ("bgvb", bgvb, 128),
                           ("bplzb", bplzb, 129), ("bpgzb", bpgzb, 129)):
            t = const.tile([128, w], FP16, name=f"br_{nm}")
            nc.sync.dma_start(out=t[:], in_=src[:])
            brows[nm] = t

        # persistent attention inputs
        qT_l = persist.tile([128, T], FP16, name="qT_l")
        kT_l = persist.tile([128, T], FP16, name="kT_l")
        qT_g = persist.tile([128, Tc], FP16, name="qT_g")
        kT_g = persist.tile([128, Tc], FP16, name="kT_g")
        v_sb_l = [persist.tile([128, 130], FP16, name=f"vsb{i}")
                  for i in range(32)]
        v_sb_g = [persist.tile([128, 130], FP16, name=f"vgsb{i}")
                  for i in range(8)]
        cg_all = [persist.tile([128, Tc], FP16, name=f"cg_all{i}")
                  for i in range(4)]
        # ones columns for the PV denominator row (cols 64 and 129)
        for v_tiles in (v_sb_l, v_sb_g):
            for vt in v_tiles:
                nc.vector.memset(vt[:, 64:65], 1.0)
                nc.vector.memset(vt[:, 129:130], 1.0)

        # ------------------------------------------------------ phase B: local qkv
        pab = top.enter_context(ExitStack())
        with ExitStack() as pb:
            xlp = pb.enter_context(tc.tile_pool(name="xlt_pool", bufs=1))
            wqp = pb.enter_context(tc.tile_pool(name="wq_pool", bufs=1))
            psB = pb.enter_context(tc.tile_pool(name="psB", bufs=3, space="PSUM"))
            psV = pb.enter_context(tc.tile_pool(name="psV", bufs=2, space="PSUM"))

            xlt_sb = []
            for ch in range(4):
                xt = xlp.tile([128, T], FP16, name=f"xlt{ch}")
                nc.sync.dma_start(out=xt[:], in_=xlt[ch])
                xlt_sb.append(xt)
            wq_sb, wk_sb, wv_sb = [], [], []
            for ch in range(4):
                for (lst, src, nm) in ((wq_sb, wq, "wq"), (wk_sb, wk, "wk"),
                                       (wv_sb, wv, "wv")):
                    t = wqp.tile([128, 128], FP16, name=f"{nm}{ch}")
                    nc.sync.dma_start(out=t[:], in_=src[ch])
                    lst.append(t)

            # q and k: packed [headA|headB] out dims -> direct eviction
            for (wsb, dst, bias_ap) in ((wq_sb, qT_l, biases["bq"][:]),
                                        (wk_sb, kT_l, biases["bk"][:])):
                for qt in range(8):
                    ps = psB.tile([128, 512], FP32, name="psB_t")
                    for ch in range(4):
                        nc.tensor.matmul(
                            ps[:], wsb[ch][:],
                            xlt_sb[ch][:, 512 * qt:512 * qt + 512],
                            start=(ch == 0), stop=(ch == 3))
                    with nc.allow_low_precision(reason="qk fp16"):
                        nc.scalar.activation(dst[:, 512 * qt:512 * qt + 512],
                                             ps[:], AF.Identity, bias=bias_ap)

            # v: direct [token, vdim] layout, bias added at eviction
            bvb3 = brows["bvb"].rearrange("p (h c) -> p h c", h=2, c=64)
            for tb in range(32):
                ps = psV.tile([128, 128], FP32, name="psV_t")
                for ch in range(4):
                    nc.tensor.matmul(
                        ps[:], xlt_sb[ch][:, 128 * tb:128 * tb + 128],
                        wv_sb[ch][:], start=(ch == 0), stop=(ch == 3))
                v3 = v_sb_l[tb].rearrange("p (h c) -> p h c", h=2, c=65)
                p3 = ps.rearrange("p (h c) -> p h c", h=2, c=64)
                with nc.allow_low_precision(reason="v fp16"):
                    nc.vector.tensor_add(v3[:, :, 0:64], p3[:], bvb3[:])

        # ------------------------------------------------------ phase A: compress
        xp = pab.enter_context(tc.tile_pool(name="xct_pool", bufs=4))
        wp = pab.enter_context(tc.tile_pool(name="wc_pool", bufs=8))
        cgp = pab.enter_context(tc.tile_pool(name="cg_pool", bufs=1))
        psA = pab.enter_context(tc.tile_pool(name="psA", bufs=1, space="PSUM"))
        cgT = cgp.tile([128, Tc], FP16, name="cgT")
        ps0 = psA.tile([128, 512], FP32, name="psA_0")
        ps1 = psA.tile([128, 512], FP32, name="psA_1")
        for ch in range(32):
            wt = wp.tile([128, 128], FP16, name="wc_t")
            nc.sync.dma_start(out=wt[:], in_=wc[ch])
            xt = xp.tile([128, Tc], FP16, name="xct_t")
            nc.sync.dma_start(out=xt[:], in_=xct[ch])
            nc.tensor.matmul(ps0[:], wt[:], xt[:, 0:512],
                             start=(ch == 0), stop=(ch == 31))
            nc.tensor.matmul(ps1[:], wt[:], xt[:, 512:1024],
                             start=(ch == 0), stop=(ch == 31))
        with nc.allow_low_precision(reason="cg fp16"):
            nc.scalar.activation(cgT[:, 0:512], ps0[:],
                                 AF.Identity, bias=biases["bc"][:])
            nc.scalar.activation(cgT[:, 512:1024], ps1[:],
                                 AF.Identity, bias=biases["bc"][:])
        cg_contrib = dram.tile([128, Tc], FP16, name="cg_contrib")
        cg_gathered = dram.tile([512, Tc], FP16, name="cg_gathered")
        nc.sync.dma_start(out=cg_contrib[:], in_=cgT[:])
        nc.gpsimd.collective_compute(
            "AllGather", mybir.AluOpType.bypass, replica_groups=GROUPS,
            ins=[cg_contrib.opt()], outs=[cg_gathered.opt()],
        )
        for i in range(4):
            nc.sync.dma_start(out=cg_all[i][:],
                              in_=cg_gathered[128 * i:128 * i + 128, :])
        pab.close()

        # ------------------------------------------------------ phase C: attention
        cpool = top.enter_context(tc.tile_pool(name="c_pool", bufs=1))
        comb_l = [cpool.tile([65, T], FP16, name=f"comb_l{h}") for h in range(2)]
        comb_g = [cpool.tile([65, Tc], FP16, name=f"comb_g{h}") for h in range(2)]
        rec_l = [cpool.tile([1, T], FP16, name=f"rec_l{h}") for h in range(2)]
        rec_g = [cpool.tile([1, Tc], FP16, name=f"rec_g{h}") for h in range(2)]

        contrib_l = dram.tile([128, T], FP16, name="attnl_contrib")
        gathered_l = dram.tile([512, T], FP16, name="attnl_gathered")
        contrib_g = dram.tile([128, Tc], FP16, name="attng_contrib")
        gathered_g = dram.tile([512, Tc], FP16, name="attng_gathered")

        with ExitStack() as pc1:
            _attention(nc, tc, pc1, "la", 4, qT_l, kT_l, v_sb_l, comb_l,
                       rec_l, consts)
        with ExitStack() as pn1:
            _normalize_and_gather(nc, tc, pn1, "l", 8, comb_l, rec_l, ones2,
                                  contrib_l, gathered_l)

        # att_all DMAs issued early so they run during global attention
        app = top.enter_context(tc.tile_pool(name="attall_pool", bufs=1))
        att_all = []
        for i in range(4):
            t = app.tile([128, T], FP16, name=f"attall{i}")
            nc.sync.dma_start(out=t[:], in_=gathered_l[128 * i:128 * i + 128, :])
            att_all.append(t)

        # global qkv from gathered compressed stream
        with ExitStack() as pg:
            wgp = pg.enter_context(tc.tile_pool(name="wg_pool", bufs=1))
            psG = pg.enter_context(tc.tile_pool(name="psG", bufs=3, space="PSUM"))
            psGV = pg.enter_context(tc.tile_pool(name="psGV", bufs=2, space="PSUM"))
            wgq_sb, wgk_sb, wgv_sb = [], [], []
            for ch in range(4):
                for (lst, src, nm) in ((wgq_sb, wgq, "wgq"), (wgk_sb, wgk, "wgk"),
                                       (wgv_sb, wgv, "wgv")):
                    t = wgp.tile([128, 128], FP16, name=f"{nm}{ch}")
                    nc.sync.dma_start(out=t[:], in_=src[ch])
                    lst.append(t)
            for (wsb, dst, bias_ap) in ((wgq_sb, qT_g, biases["bgq"][:]),
                                        (wgk_sb, kT_g, biases["bgk"][:])):
                for qt in range(2):
                    ps = psG.tile([128, 512], FP32, name="psG_t")
                    for ch in range(4):
                        nc.tensor.matmul(
                            ps[:], wsb[ch][:],
                            cg_all[ch][:, 512 * qt:512 * qt + 512],
                            start=(ch == 0), stop=(ch == 3))
                    with nc.allow_low_precision(reason="gqk fp16"):
                        nc.scalar.activation(dst[:, 512 * qt:512 * qt + 512],
                                             ps[:], AF.Identity, bias=bias_ap)
            bgvb3 = brows["bgvb"].rearrange("p (h c) -> p h c", h=2, c=64)
            for tb in range(8):
                ps = psGV.tile([128, 128], FP32, name="psGV_t")
                for ch in range(4):
                    nc.tensor.matmul(
                        ps[:], cg_all[ch][:, 128 * tb:128 * tb + 128],
                        wgv_sb[ch][:], start=(ch == 0), stop=(ch == 3))
                v3 = v_sb_g[tb].rearrange("p (h c) -> p h c", h=2, c=65)
                p3 = ps.rearrange("p (h c) -> p h c", h=2, c=64)
                with nc.allow_low_precision(reason="gv fp16"):
                    nc.vector.tensor_add(v3[:, :, 0:64], p3[:], bgvb3[:])

        with ExitStack() as pc2:
            _attention(nc, tc, pc2, "ga", 1, qT_g, kT_g, v_sb_g, comb_g,
                       rec_g, consts)
        with ExitStack() as pn2:
            _normalize_and_gather(nc, tc, pn2, "g", 2, comb_g, rec_g, ones2,
                                  contrib_g, gathered_g)

        # ------------------------------------------------------ phase D: proj+gate
        with ExitStack() as pd:
            wpp = pd.enter_context(tc.tile_pool(name="wp_pool", bufs=1))
            psP = pd.enter_context(tc.tile_pool(name="psP", bufs=3, space="PSUM"))
            psE = pd.enter_context(tc.tile_pool(name="psE", bufs=2, space="PSUM"))
            gp = pd.enter_context(tc.tile_pool(name="gproj_pool", bufs=1))
            zp = pd.enter_context(tc.tile_pool(name="z_pool", bufs=1))
            outp = pd.enter_context(tc.tile_pool(name="out_pool", bufs=4))

            attg_all = []
            for i in range(4):
                t = wpp.tile([128, Tc], FP16, name=f"attgall{i}")
                nc.sync.dma_start(out=t[:],
                                  in_=gathered_g[128 * i:128 * i + 128, :])
                attg_all.append(t)
            wplz_sb, wpgz_sb = [], []
            for ch in range(4):
                t = wpp.tile([128, 129], FP16, name=f"wplz{ch}")
                nc.sync.dma_start(out=t[:], in_=wplz[ch])
                wplz_sb.append(t)
                t = wpp.tile([128, 129], FP16, name=f"wpgz{ch}")
                nc.sync.dma_start(out=t[:], in_=wpgz[ch])
                wpgz_sb.append(t)

            # local proj: [128 tok, 129] blocks -> loc_sb
            loc_sb = gp.tile([128, 32 * 129], FP16, name="loc_sb")
            loc3 = loc_sb.rearrange("p (a b) -> p a b", a=32, b=129)
            for tb in range(32):
                ps = psP.tile([128, 129], FP32, name="psP_t")
                for ch in range(4):
                    nc.tensor.matmul(ps[:],
                                     att_all[ch][:, 128 * tb:128 * tb + 128],
                                     wplz_sb[ch][:], start=(ch == 0), stop=(ch == 3))
                with nc.allow_low_precision(reason="proj fp16"):
                    nc.vector.tensor_add(loc3[:, tb, :], ps[:], brows["bplzb"][:])

            # global proj (Tc rows) -> gproj_sb, then x4 expand -> ge_sb
            gproj_sb = gp.tile([128, 8 * 129], FP16, name="gproj_sb")
            gproj3 = gproj_sb.rearrange("p (a b) -> p a b", a=8, b=129)
            for tbg in range(8):
                ps = psP.tile([128, 129], FP32, name="psP_t")
                for ch in range(4):
                    nc.tensor.matmul(ps[:],
                                     attg_all[ch][:, 128 * tbg:128 * tbg + 128],
                                     wpgz_sb[ch][:], start=(ch == 0), stop=(ch == 3))
                with nc.allow_low_precision(reason="gproj fp16"):
                    nc.vector.tensor_add(gproj3[:, tbg, :], ps[:], brows["bpgzb"][:])
            ge_sb = gp.tile([128, 32 * 129], FP16, name="ge_sb")
            ge3 = ge_sb.rearrange("p (a b) -> p a b", a=32, b=129)
            for tb in range(32):
                base = 64 * ((tb % 4) // 2)
                rep = repA_sb if tb % 2 == 0 else repB_sb
                ps = psE.tile([128, 129], FP32, name="psE_t")
                nc.tensor.matmul(ps[:], rep[base:base + 64, :],
                                 gproj3[base:base + 64, tb // 4, :],
                                 start=True, stop=True)
                with nc.allow_low_precision(reason="gexp fp16"):
                    nc.vector.tensor_copy(ge3[:, tb, :], ps[:])

            # gate: z = loc_z + ge_z; g0 = 0.5 + 0.5*tanh(z/2); g1 = 1 - g0
            zsum = zp.tile([128, 32], FP32, name="zsum")
            nc.vector.tensor_add(zsum[:], loc3[:, :, 128], ge3[:, :, 128])
            tanh_t = zp.tile([128, 32], FP32, name="tanh_t")
            nc.scalar.activation(tanh_t[:], zsum[:], AF.Tanh, scale=0.5)
            g0 = zp.tile([128, 32], FP32, name="g0")
            g1 = zp.tile([128, 32], FP32, name="g1")
            nc.vector.tensor_scalar(g0[:], tanh_t[:], 0.5, 0.5,
                                    mybir.AluOpType.mult, mybir.AluOpType.add)
            nc.vector.tensor_scalar(g1[:], tanh_t[:], -0.5, 0.5,
                                    mybir.AluOpType.mult, mybir.AluOpType.add)

            for tb in range(32):
                o = outp.tile([128, 128], FP16, name="outl")
                with nc.allow_low_precision(reason="out fp16"):
                    nc.vector.tensor_scalar_mul(o[:], loc3[:, tb, 0:128],
                                                g0[:, tb:tb + 1])
                nc.sync.dma_start(out=out_loc[128 * tb:128 * tb + 128, :], in_=o[:])
                o2 = outp.tile([128, 128], FP16, name="outg")
                with nc.allow_low_precision(reason="out fp16"):
                    nc.vector.tensor_scalar_mul(o2[:], ge3[:, tb, 0:128],
                                                g1[:, tb:tb + 1])
                nc.sync.dma_start(out=out_glob[128 * tb:128 * tb + 128, :], in_=o2[:])

    nc.finalize()
    return nc


# ---------------------------------------------------------------------------
# Host side
# ---------------------------------------------------------------------------

_NC_CACHE = []


def _get_program():
    if not _NC_CACHE:
        _NC_CACHE.append(build_program())
    return _NC_CACHE[0]


def _prep_inputs(x, w_lqkv, b_lqkv, w_gqkv, b_gqkv, w_comp, b_comp,
                 w_lproj, b_lproj, w_gproj, b_gproj, w_gate, b_gate):
    f32, f16 = np.float32, np.float16
    wd = (w_gate[:, 0] - w_gate[:, 1]).astype(f32)
    u_l = (w_lproj @ wd[:LD]).astype(f32)
    u_g = (w_gproj @ wd[LD:]).astype(f32)
    c0 = float(b_lproj @ wd[:LD] + b_gproj @ wd[LD:] + b_gate[0] - b_gate[1])

    mask_tri = np.where(np.arange(128)[None, :] >= np.arange(128)[:, None],
                        0.0, NEG).astype(f16)
    e0 = np.zeros((64, 128), f32)
    e0[np.arange(128) // 4, np.arange(128)] = 1.0
    e1 = np.zeros((64, 128), f32)
    e1[32 + np.arange(128) // 4, np.arange(128)] = 1.0
    repA_ = np.concatenate([e0, e0], axis=0).astype(f16)
    repB_ = np.concatenate([e1, e1], axis=0).astype(f16)

    def packed_cols(w, b, off, ha, hb, scale=1.0):
        wp = np.concatenate([w[:, off + D * ha:off + D * ha + D],
                             w[:, off + D * hb:off + D * hb + D]], axis=1) * scale
        bp = np.concatenate([b[off + D * ha:off + D * ha + D],
                             b[off + D * hb:off + D * hb + D]]) * scale
        return wp.astype(f16).reshape(4, 128, 128), bp.astype(f32).reshape(128, 1)

    in_maps = []
    for core in range(NCORES):
        b_idx, g = core // 4, core % 4
        ha, hb = 2 * g, 2 * g + 1
        cs = slice(128 * g, 128 * g + 128)

        xlt_ = np.ascontiguousarray(x[b_idx, :, :LD].T).astype(f16).reshape(4, 128, T)
        xct_ = np.ascontiguousarray(
            x[b_idx].reshape(Tc, R * E).T).astype(f16).reshape(32, 128, Tc)

        wq_, bq_ = packed_cols(w_lqkv, b_lqkv, 0, ha, hb, 1.0 / 8.0)
        wk_, bk_ = packed_cols(w_lqkv, b_lqkv, LD, ha, hb)
        wv_, bv_ = packed_cols(w_lqkv, b_lqkv, 2 * LD, ha, hb)
        wgq_, bgq_ = packed_cols(w_gqkv, b_gqkv, 0, ha, hb, 1.0 / 8.0)
        wgk_, bgk_ = packed_cols(w_gqkv, b_gqkv, LD, ha, hb)
        wgv_, bgv_ = packed_cols(w_gqkv, b_gqkv, 2 * LD, ha, hb)

        wplz_ = np.concatenate(
            [w_lproj[:, cs], u_l[:, None]], axis=1).astype(f16).reshape(4, 128, 129)
        wpgz_ = np.concatenate(
            [w_gproj[:, cs], u_g[:, None]], axis=1).astype(f16).reshape(4, 128, 129)

        in_maps.append({
            "xlt": xlt_, "xct": xct_,
            "wq": wq_, "bq": bq_, "wk": wk_, "bk": bk_,
            "wv": wv_,
            "bvb": np.tile(bv_.reshape(1, 128), (128, 1)).astype(f16),
            "wgq": wgq_, "bgq": bgq_, "wgk": wgk_, "bgk": bgk_,
            "wgv": wgv_,
            "bgvb": np.tile(bgv_.reshape(1, 128), (128, 1)).astype(f16),
            "wc": np.ascontiguousarray(
                w_comp[:, LD + 128 * g:LD + 128 * g + 128]).astype(f16)
                .reshape(32, 128, 128),
            "bc": b_comp[LD + 128 * g:LD + 128 * g + 128].astype(f32)
                .reshape(128, 1),
            "wplz": wplz_,
            "bplzb": np.tile(np.concatenate([b_lproj[cs], [c0]])
                             .reshape(1, 129), (128, 1)).astype(f16),
            "wpgz": wpgz_,
            "bpgzb": np.tile(np.concatenate([b_gproj[cs], [0.0]])
                             .reshape(1, 129), (128, 1)).astype(f16),
            "repA": repA_, "repB": repB_, "maskt": mask_tri,
        })
    return in_maps


def _run(in_maps, trace=False):
    nc = _get_program()
    return run_bass_kernel_spmd(nc, in_maps, list(range(NCORES)), trace=trace)


def assemble(results):
    out = np.empty((B, T, E), np.float32)
    for core in range(NCORES):
        b_idx, g = core // 4, core % 4
        out[b_idx, :, 128 * g:128 * g + 128] = \
            results[core]["out_loc"].astype(np.float32)
        out[b_idx, :, LD + 128 * g:LD + 128 * g + 128] = \
            results[core]["out_glob"].astype(np.float32)
    return out


def kernel(**inputs):
    in_maps = _prep_inputs(**inputs)
    res = _run(in_maps)
    return assemble(res.results)


def kernel_traced(**inputs):
    """test.py helper: returns (output, BassKernelResults with timing)."""
    in_maps = _prep_inputs(**inputs)
    res = _run(in_maps, trace=True)
    return assemble(res.results), res


# revision 26
# speedup vs baseline: 1.3711x; 1.0792x over previous
"""DualResolutionAttention Trainium2 kernel (8 NeuronCores, Bass/Tile).

Sharding: core c -> (batch b = c//4, group g = c%4).
Each core computes local heads {2g, 2g+1} and global heads {2g, 2g+1} over the
full sequence, plus the output channel slice [128g, 128g+128) of each branch.
Three AllGathers within each 4-core batch group: (1) compressed stream cgT,
(2) normalized local attention (fp16), (3) normalized global attention (fp16).

v2 design (vs baseline): fp16 compute everywhere (FWL weight loads, 2x less
DMA/SBUF), V computed directly in [token, vdim] layout (no PE transposes),
q/k evicted straight from PSUM to qT/kT (packed per-head weights), gate logits
folded into the projection matmuls as a 129th output column, masks via 64-row
identity matmuls (no PE tiling-mode switch inside attention).
"""
import os
import sys

sys.path.insert(0, "/opt/trn_rl_repo")
os.environ.setdefault("JAX_PLATFORMS", "axon,cpu")

from contextlib import ExitStack

import numpy as np

import concourse.bass as bass
import concourse.mybir as mybir
import concourse.tile as tile
from concourse import bacc
from concourse.bass_utils import run_bass_kernel_spmd
from concourse.masks import make_identity

FP32 = mybir.dt.float32
FP16 = mybir.dt.float16
AF = mybir.ActivationFunctionType

# Problem constants
B, T, E = 2, 4096, 1024
LD = 512            # local/global stream dim
D = 64              # head dim
HH = 8              # heads per branch
R = 4               # compression ratio
Tc = T // R         # 1024
NCORES = 8
GROUPS = [[0, 1, 2, 3], [4, 5, 6, 7]]

NEG = -30000.0      # fp16-safe mask value


# ---------------------------------------------------------------------------
# Program builder
# ---------------------------------------------------------------------------

def _attention(nc, tc, ctx, name, nQT2, qT, kTh, v_sb, comb, rec, consts):
    """Attention body: S^T layout scores, [v|ones] PV with denominator row.

    All matmuls run in 128-row tiling mode (kTh[h] is the per-head key tile
    with the other head's partition half zeroed), so the PE never pays a
    tiling-mode-switch drain.  Both heads' scores land in one 4-bank PSUM
    tile and a single Exp instruction covers them.  comb[h] is a
    [65, n512*512] fp16 tile: rows 0:64 = unnormalized attT, row 64 = softmax
    denominator.  rec[h] [1, ncols] gets 1/denominator (computed in a
    [128, 8] layout via DMA reshape - reciprocal is 8 cyc/elem on the DVE).
    """
    ps_s = ctx.enter_context(tc.tile_pool(name=f"{name}_ps_s", bufs=1, space="PSUM"))
    ps_o = ctx.enter_context(tc.tile_pool(name=f"{name}_ps_o", bufs=1, space="PSUM"))
    p_pool = ctx.enter_context(tc.tile_pool(name=f"{name}_p", bufs=3))
    dnp = ctx.enter_context(tc.tile_pool(name=f"{name}_dn", bufs=2))

    mask_tri = consts["mask_tri"]
    ident = consts["ident"]

    def emit_scores(q2, kb):
        delta = kb - 8 * q2
        t0 = max(0, 128 * delta)
        ps2 = ps_s.tile([128, 2048], FP32, name=f"{name}_s2")
        p_sb = p_pool.tile([128, 2048], FP16, name=f"{name}_pt")
        for h in range(2):
            for qs in (0, 512):
                if qs + 512 <= (t0 // 512) * 512:
                    continue
                nc.tensor.matmul(
                    ps2[:, 1024 * h + qs:1024 * h + qs + 512],
                    kTh[h][:, 128 * kb:128 * kb + 128],
                    qT[:, 1024 * q2 + qs:1024 * q2 + qs + 512],
                    start=True, stop=True,
                )
        if delta >= 0:
            for h in range(2):
                nc.tensor.matmul(
                    ps2[:, 1024 * h + t0:1024 * h + t0 + 128],
                    ident[:], mask_tri[:],
                    start=False, stop=True, skip_group_check=True,
                )
        if t0 == 0:
            nc.scalar.activation(p_sb[:], ps2[:], AF.Exp)
        else:
            p3 = p_sb.rearrange("p (h c) -> p h c", h=2, c=1024)
            s3 = ps2.rearrange("p (h c) -> p h c", h=2, c=1024)
            nc.scalar.activation(p3[:, :, t0:1024], s3[:, :, t0:1024], AF.Exp)
        return p_sb, t0

    def emit_pv(h, kb, nkb, psum_o, p_sb, t0):
        for qs in (0, 512):
            lo = max(qs, t0)
            hi = qs + 512
            if lo >= hi:
                continue
            nc.tensor.matmul(
                psum_o[:, lo:hi],
                v_sb[kb][:, 65 * h:65 * h + 65],
                p_sb[:, 1024 * h + lo:1024 * h + hi],
                start=(kb == 0), stop=(kb == nkb - 1),
                skip_group_check=True,
            )

    for q2 in range(nQT2):
        nkb = 8 * q2 + 8
        psum_o = [ps_o.tile([65, 1024], FP32, name=f"{name}_o{h}")
                  for h in range(2)]
        pend = {}
        for kb in range(nkb):
            pend[kb] = emit_scores(q2, kb)
            if kb >= 1:
                p_sb, t0 = pend.pop(kb - 1)
                for h in range(2):
                    emit_pv(h, kb - 1, nkb, psum_o[h], p_sb, t0)
        p_sb, t0 = pend.pop(nkb - 1)
        for h in range(2):
            emit_pv(h, nkb - 1, nkb, psum_o[h], p_sb, t0)
        for h in range(2):
            # one copy evicts both attT rows and the denominator row
            with nc.allow_low_precision(reason="att fp16"):
                nc.vector.tensor_copy(
                    comb[h][:, 1024 * q2:1024 * q2 + 1024], psum_o[h][:])
            # reciprocal in [128, 8] layout (DMA reshape there and back)
            dh = dnp.tile([128, 8], FP16, name=f"{name}_dh")
            nc.sync.dma_start(
                out=dh[:], in_=comb[h][64:65, 1024 * q2:1024 * q2 + 1024])
            rc = dnp.tile([128, 8], FP16, name=f"{name}_rc")
            with nc.allow_low_precision(reason="softmax denom fp16"):
                nc.vector.reciprocal(rc[:], dh[:])
            nc.sync.dma_start(
                out=rec[h][0:1, 1024 * q2:1024 * q2 + 1024], in_=rc[:])


def _normalize_and_gather(nc, tc, ctx, name, n512, comb, rec, ones2,
                          contrib, gathered):
    """bcast rec per head via K=1 ones matmul; contrib = att * rec; gather."""
    psN = ctx.enter_context(tc.tile_pool(name=f"psN_{name}", bufs=2, space="PSUM"))
    ap = ctx.enter_context(tc.tile_pool(name=f"an_{name}", bufs=1))

    ncols = 512 * n512
    attn = [ap.tile([64, ncols], FP16, name=f"attn_{name}{h}") for h in range(2)]
    for h in range(2):
        for qt in range(n512):
            ps = psN.tile([128, 512], FP32, name=f"psN_{name}_t")
            nc.tensor.matmul(ps[:], ones2[0:1, :],
                             rec[h][0:1, 512 * qt:512 * qt + 512],
                             start=True, stop=True)
            with nc.allow_low_precision(reason="attnorm fp16"):
                nc.vector.tensor_mul(
                    attn[h][:, 512 * qt:512 * qt + 512],
                    comb[h][0:64, 512 * qt:512 * qt + 512], ps[0:64, :])
        nc.sync.dma_start(out=contrib[64 * h:64 * h + 64, :], in_=attn[h][:])
    nc.gpsimd.collective_compute(
        "AllGather", mybir.AluOpType.bypass, replica_groups=GROUPS,
        ins=[contrib.opt()], outs=[gathered.opt()],
    )


def build_program():
    nc = bacc.Bacc(None, target_bir_lowering=False)

    def inp(name, shape, dt=FP16):
        return nc.declare_dram_parameter(name, list(shape), dt, isOutput=False)

    # data
    xlt = inp("xlt", [4, 128, T])            # x[b,:,:512].T chunks
    xct = inp("xct", [32, 128, Tc])          # x[b].reshape(Tc,4096).T chunks
    # weights
    wq = inp("wq", [4, 128, 128])            # [qA|qB] lhsT chunks (scaled 1/8)
    bq = inp("bq", [128, 1], FP32)
    wk = inp("wk", [4, 128, 128])
    bk = inp("bk", [128, 1], FP32)
    wv = inp("wv", [4, 128, 128])            # [vA|vB] (rhs for v-direct)
    bvb = inp("bvb", [128, 128])             # [bvA|bvB] replicated to 128 rows
    wgq = inp("wgq", [4, 128, 128])
    bgq = inp("bgq", [128, 1], FP32)
    wgk = inp("wgk", [4, 128, 128])
    bgk = inp("bgk", [128, 1], FP32)
    wgv = inp("wgv", [4, 128, 128])
    bgvb = inp("bgvb", [128, 128])
    wc = inp("wc", [32, 128, 128])           # compress slice lhsT chunks
    bc = inp("bc", [128, 1], FP32)
    wplz = inp("wplz", [4, 128, 129])        # [w_lproj[:,cs] | u_l] chunks
    bplzb = inp("bplzb", [128, 129])         # [b_lproj[cs] | c0] replicated
    wpgz = inp("wpgz", [4, 128, 129])
    bpgzb = inp("bpgzb", [128, 129])
    repA = inp("repA", [128, 128])           # x4 expander (even 32-blocks)
    repB = inp("repB", [128, 128])           # x4 expander (odd 32-blocks)
    maskt = inp("maskt", [128, 128])         # strict lower-tri NEG
    out_loc = nc.declare_dram_parameter("out_loc", [T, 128], FP16, isOutput=True)
    out_glob = nc.declare_dram_parameter("out_glob", [T, 128], FP16, isOutput=True)

    with tile.TileContext(nc) as tc:
      with ExitStack() as top:
        dram = top.enter_context(tc.tile_pool(name="dram", bufs=1, space="DRAM"))
        const = top.enter_context(tc.tile_pool(name="const", bufs=1))
        persist = top.enter_context(tc.tile_pool(name="persist", bufs=1))

        # constants
        ident = const.tile([128, 128], FP16, name="ident")
        make_identity(nc, ident[:])
        mask_tri = const.tile([128, 128], FP16, name="mask_tri")
        nc.sync.dma_start(out=mask_tri[:], in_=maskt[:])
        repA_sb = const.tile([128, 128], FP16, name="repA_sb")
        nc.sync.dma_start(out=repA_sb[:], in_=repA[:])
        repB_sb = const.tile([128, 128], FP16, name="repB_sb")
        nc.sync.dma_start(out=repB_sb[:], in_=repB[:])
        ones2 = const.tile([1, 128], FP16, name="ones2")
        nc.gpsimd.memset(ones2[:], 1.0)
        consts = {"mask_tri": mask_tri, "ident": ident}
        biases = {}
        for nm, src in (("bq", bq), ("bk", bk), ("bgq", bgq), ("bgk", bgk),
                        ("bc", bc)):
            t = const.tile([128, 1], FP32, name=f"cb_{nm}")
            nc.sync.dma_start(out=t[:], in_=src[:])
            biases[nm] = t
        brows = {}
        for nm, src, w in (("bvb", bvb, 128), ("bgvb", bgvb, 128),
                           ("bplzb", bplzb, 129), ("bpgzb", bpgzb, 129)):
            t = const.tile([128, w], FP16, name=f"br_{nm}")
            nc.sync.dma_start(out=t[:], in_=src[:])
            brows[nm] = t

        # persistent attention inputs.  kT is stored per head with the other
        # head's partition half zeroed so score matmuls run at K=128 (no PE
        # tiling-mode switches, FWL-eligible weight loads).
        qT_l = persist.tile([128, T], FP16, name="qT_l")
        kT_lh = [persist.tile([128, T], FP16, name=f"kT_l{h}") for h in range(2)]
        qT_g = persist.tile([128, Tc], FP16, name="qT_g")
        kT_gh = [persist.tile([128, Tc], FP16, name=f"kT_g{h}") for h in range(2)]
        nc.gpsimd.memset(kT_lh[0][64:128, :], 0.0)
        nc.gpsimd.memset(kT_lh[1][0:64, :], 0.0)
        nc.gpsimd.memset(kT_gh[0][64:128, :], 0.0)
        nc.gpsimd.memset(kT_gh[1][0:64, :], 0.0)
        v_sb_l = [persist.tile([128, 130], FP16, name=f"vsb{i}")
                  for i in range(32)]
        v_sb_g = [persist.tile([128, 130], FP16, name=f"vgsb{i}")
                  for i in range(8)]
        cg_all = [persist.tile([128, Tc], FP16, name=f"cg_all{i}")
                  for i in range(4)]
        # ones columns for the PV denominator row (cols 64 and 129)
        for v_tiles in (v_sb_l, v_sb_g):
            for vt in v_tiles:
                nc.vector.memset(vt[:, 64:65], 1.0)
                nc.vector.memset(vt[:, 129:130], 1.0)

        # ------------------------------------------------------ phase B: local qkv
        pab = top.enter_context(ExitStack())
        with ExitStack() as pb:
            xlp = pb.enter_context(tc.tile_pool(name="xlt_pool", bufs=1))
            wqp = pb.enter_context(tc.tile_pool(name="wq_pool", bufs=1))
            psB = pb.enter_context(tc.tile_pool(name="psB", bufs=3, space="PSUM"))
            psV = pb.enter_context(tc.tile_pool(name="psV", bufs=2, space="PSUM"))

            xlt_sb = []
            for ch in range(4):
                xt = xlp.tile([128, T], FP16, name=f"xlt{ch}")
                nc.sync.dma_start(out=xt[:], in_=xlt[ch])
                xlt_sb.append(xt)
            wq_sb, wk_sb, wv_sb = [], [], []
            for ch in range(4):
                for (lst, src, nm) in ((wq_sb, wq, "wq"), (wk_sb, wk, "wk"),
                                       (wv_sb, wv, "wv")):
                    t = wqp.tile([128, 128], FP16, name=f"{nm}{ch}")
                    nc.sync.dma_start(out=t[:], in_=src[ch])
                    lst.append(t)

            # q and k: packed [headA|headB] out dims -> direct eviction
            kT_st = xlp.tile([128, T], FP16, name="kT_st")
            for (wsb, dst, bias_ap) in ((wq_sb, qT_l, biases["bq"][:]),
                                        (wk_sb, kT_st, biases["bk"][:])):
                for qt in range(8):
                    ps = psB.tile([128, 512], FP32, name="psB_t")
                    for ch in range(4):
                        nc.tensor.matmul(
                            ps[:], wsb[ch][:],
                            xlt_sb[ch][:, 512 * qt:512 * qt + 512],
                            start=(ch == 0), stop=(ch == 3))
                    with nc.allow_low_precision(reason="qk fp16"):
                        nc.scalar.activation(dst[:, 512 * qt:512 * qt + 512],
                                             ps[:], AF.Identity, bias=bias_ap)
            # split k into per-head zero-padded tiles (partition-preserving)
            nc.sync.dma_start(out=kT_lh[0][0:64, :], in_=kT_st[0:64, :])
            nc.sync.dma_start(out=kT_lh[1][64:128, :], in_=kT_st[64:128, :])

            # v: direct [token, vdim] layout, bias added at eviction
            bvb3 = brows["bvb"].rearrange("p (h c) -> p h c", h=2, c=64)
            for tb in range(32):
                ps = psV.tile([128, 128], FP32, name="psV_t")
                for ch in range(4):
                    nc.tensor.matmul(
                        ps[:], xlt_sb[ch][:, 128 * tb:128 * tb + 128],
                        wv_sb[ch][:], start=(ch == 0), stop=(ch == 3))
                v3 = v_sb_l[tb].rearrange("p (h c) -> p h c", h=2, c=65)
                p3 = ps.rearrange("p (h c) -> p h c", h=2, c=64)
                with nc.allow_low_precision(reason="v fp16"):
                    nc.vector.tensor_add(v3[:, :, 0:64], p3[:], bvb3[:])

        # ------------------------------------------------------ phase A: compress
        xp = pab.enter_context(tc.tile_pool(name="xct_pool", bufs=4))
        wp = pab.enter_context(tc.tile_pool(name="wc_pool", bufs=8))
        cgp = pab.enter_context(tc.tile_pool(name="cg_pool", bufs=1))
        psA = pab.enter_context(tc.tile_pool(name="psA", bufs=1, space="PSUM"))
        cgT = cgp.tile([128, Tc], FP16, name="cgT")
        ps0 = psA.tile([128, 512], FP32, name="psA_0")
        ps1 = psA.tile([128, 512], FP32, name="psA_1")
        for ch in range(32):
            wt = wp.tile([128, 128], FP16, name="wc_t")
            nc.sync.dma_start(out=wt[:], in_=wc[ch])
            xt = xp.tile([128, Tc], FP16, name="xct_t")
            nc.sync.dma_start(out=xt[:], in_=xct[ch])
            nc.tensor.matmul(ps0[:], wt[:], xt[:, 0:512],
                             start=(ch == 0), stop=(ch == 31))
            nc.tensor.matmul(ps1[:], wt[:], xt[:, 512:1024],
                             start=(ch == 0), stop=(ch == 31))
        with nc.allow_low_precision(reason="cg fp16"):
            nc.scalar.activation(cgT[:, 0:512], ps0[:],
                                 AF.Identity, bias=biases["bc"][:])
            nc.scalar.activation(cgT[:, 512:1024], ps1[:],
                                 AF.Identity, bias=biases["bc"][:])
        cg_contrib = dram.tile([128, Tc], FP16, name="cg_contrib")
        cg_gathered = dram.tile([512, Tc], FP16, name="cg_gathered")
        nc.sync.dma_start(out=cg_contrib[:], in_=cgT[:])
        nc.gpsimd.collective_compute(
            "AllGather", mybir.AluOpType.bypass, replica_groups=GROUPS,
            ins=[cg_contrib.opt()], outs=[cg_gathered.opt()],
        )
        for i in range(4):
            nc.sync.dma_start(out=cg_all[i][:],
                              in_=cg_gathered[128 * i:128 * i + 128, :])
        pab.close()

        # ------------------------------------------------------ phase C: attention
        cpool = top.enter_context(tc.tile_pool(name="c_pool", bufs=1))
        comb_l = [cpool.tile([65, T], FP16, name=f"comb_l{h}") for h in range(2)]
        comb_g = [cpool.tile([65, Tc], FP16, name=f"comb_g{h}") for h in range(2)]
        rec_l = [cpool.tile([1, T], FP16, name=f"rec_l{h}") for h in range(2)]
        rec_g = [cpool.tile([1, Tc], FP16, name=f"rec_g{h}") for h in range(2)]

        contrib_l = dram.tile([128, T], FP16, name="attnl_contrib")
        gathered_l = dram.tile([512, T], FP16, name="attnl_gathered")
        contrib_g = dram.tile([128, Tc], FP16, name="attng_contrib")
        gathered_g = dram.tile([512, Tc], FP16, name="attng_gathered")

        with ExitStack() as pc1:
            _attention(nc, tc, pc1, "la", 4, qT_l, kT_lh, v_sb_l, comb_l,
                       rec_l, consts)

        # att_all DMAs issued early so they run during global attention
        app = top.enter_context(tc.tile_pool(name="attall_pool", bufs=1))

        # global qkv emitted before the local normalize so the PE stream
        # never stalls on the reciprocal/bcast chain
        with ExitStack() as pg:
            wgp = pg.enter_context(tc.tile_pool(name="wg_pool", bufs=1))
            psG = pg.enter_context(tc.tile_pool(name="psG", bufs=3, space="PSUM"))
            psGV = pg.enter_context(tc.tile_pool(name="psGV", bufs=2, space="PSUM"))
            wgq_sb, wgk_sb, wgv_sb = [], [], []
            for ch in range(4):
                for (lst, src, nm) in ((wgq_sb, wgq, "wgq"), (wgk_sb, wgk, "wgk"),
                                       (wgv_sb, wgv, "wgv")):
                    t = wgp.tile([128, 128], FP16, name=f"{nm}{ch}")
                    nc.sync.dma_start(out=t[:], in_=src[ch])
                    lst.append(t)
            kTg_st = wgp.tile([128, Tc], FP16, name="kTg_st")
            for (wsb, dst, bias_ap) in ((wgq_sb, qT_g, biases["bgq"][:]),
                                        (wgk_sb, kTg_st, biases["bgk"][:])):
                for qt in range(2):
                    ps = psG.tile([128, 512], FP32, name="psG_t")
                    for ch in range(4):
                        nc.tensor.matmul(
                            ps[:], wsb[ch][:],
                            cg_all[ch][:, 512 * qt:512 * qt + 512],
                            start=(ch == 0), stop=(ch == 3))
                    with nc.allow_low_precision(reason="gqk fp16"):
                        nc.scalar.activation(dst[:, 512 * qt:512 * qt + 512],
                                             ps[:], AF.Identity, bias=bias_ap)
            nc.sync.dma_start(out=kT_gh[0][0:64, :], in_=kTg_st[0:64, :])
            nc.sync.dma_start(out=kT_gh[1][64:128, :], in_=kTg_st[64:128, :])
            bgvb3 = brows["bgvb"].rearrange("p (h c) -> p h c", h=2, c=64)
            for tb in range(8):
                ps = psGV.tile([128, 128], FP32, name="psGV_t")
                for ch in range(4):
                    nc.tensor.matmul(
                        ps[:], cg_all[ch][:, 128 * tb:128 * tb + 128],
                        wgv_sb[ch][:], start=(ch == 0), stop=(ch == 3))
                v3 = v_sb_g[tb].rearrange("p (h c) -> p h c", h=2, c=65)
                p3 = ps.rearrange("p (h c) -> p h c", h=2, c=64)
                with nc.allow_low_precision(reason="gv fp16"):
                    nc.vector.tensor_add(v3[:, :, 0:64], p3[:], bgvb3[:])

        # local normalize + gather (PE bcast matmuls run after global qkv)
        with ExitStack() as pn1:
            _normalize_and_gather(nc, tc, pn1, "l", 8, comb_l, rec_l, ones2,
                                  contrib_l, gathered_l)
        att_all = []
        for i in range(4):
            t = app.tile([128, T], FP16, name=f"attall{i}")
            nc.sync.dma_start(out=t[:], in_=gathered_l[128 * i:128 * i + 128, :])
            att_all.append(t)

        with ExitStack() as pc2:
            _attention(nc, tc, pc2, "ga", 1, qT_g, kT_gh, v_sb_g, comb_g,
                       rec_g, consts)

        # ------------------------------------------------------ phase D: proj+gate
        with ExitStack() as pd:
            wpp = pd.enter_context(tc.tile_pool(name="wp_pool", bufs=1))
            psP = pd.enter_context(tc.tile_pool(name="psP", bufs=3, space="PSUM"))
            psE = pd.enter_context(tc.tile_pool(name="psE", bufs=2, space="PSUM"))
            gp = pd.enter_context(tc.tile_pool(name="gproj_pool", bufs=1))
            zp = pd.enter_context(tc.tile_pool(name="z_pool", bufs=1))
            outp = pd.enter_context(tc.tile_pool(name="out_pool", bufs=4))

            wplz_sb, wpgz_sb = [], []
            for ch in range(4):
                t = wpp.tile([128, 129], FP16, name=f"wplz{ch}")
                nc.sync.dma_start(out=t[:], in_=wplz[ch])
                wplz_sb.append(t)
                t = wpp.tile([128, 129], FP16, name=f"wpgz{ch}")
                nc.sync.dma_start(out=t[:], in_=wpgz[ch])
                wpgz_sb.append(t)

            # local proj: [128 tok, 129] blocks -> loc_sb
            loc_sb = gp.tile([128, 32 * 129], FP16, name="loc_sb")
            loc3 = loc_sb.rearrange("p (a b) -> p a b", a=32, b=129)
            for tb in range(32):
                ps = psP.tile([128, 129], FP32, name="psP_t")
                for ch in range(4):
                    nc.tensor.matmul(ps[:],
                                     att_all[ch][:, 128 * tb:128 * tb + 128],
                                     wplz_sb[ch][:], start=(ch == 0), stop=(ch == 3))
                with nc.allow_low_precision(reason="proj fp16"):
                    nc.vector.tensor_add(loc3[:, tb, :], ps[:], brows["bplzb"][:])

            # global normalize + gather overlaps the local proj matmuls
            with ExitStack() as pn2:
                _normalize_and_gather(nc, tc, pn2, "g", 2, comb_g, rec_g,
                                      ones2, contrib_g, gathered_g)
            attg_all = []
            for i in range(4):
                t = wpp.tile([128, Tc], FP16, name=f"attgall{i}")
                nc.sync.dma_start(out=t[:],
                                  in_=gathered_g[128 * i:128 * i + 128, :])
                attg_all.append(t)

            # global proj (Tc rows) -> gproj_sb, then x4 expand -> ge_sb
            gproj_sb = gp.tile([128, 8 * 129], FP16, name="gproj_sb")
            gproj3 = gproj_sb.rearrange("p (a b) -> p a b", a=8, b=129)
            for tbg in range(8):
                ps = psP.tile([128, 129], FP32, name="psP_t")
                for ch in range(4):
                    nc.tensor.matmul(ps[:],
                                     attg_all[ch][:, 128 * tbg:128 * tbg + 128],
                                     wpgz_sb[ch][:], start=(ch == 0), stop=(ch == 3))
                with nc.allow_low_precision(reason="gproj fp16"):
                    nc.vector.tensor_add(gproj3[:, tbg, :], ps[:], brows["bpgzb"][:])
            ge_sb = gp.tile([128, 32 * 129], FP16, name="ge_sb")
            ge3 = ge_sb.rearrange("p (a b) -> p a b", a=32, b=129)
            for tb in range(32):
                base = 64 * ((tb % 4) // 2)
                rep = repA_sb if tb % 2 == 0 else repB_sb
                ps = psE.tile([128, 129], FP32, name="psE_t")
                nc.tensor.matmul(ps[:], rep[base:base + 64, :],
                                 gproj3[base:base + 64, tb // 4, :],
                                 start=True, stop=True)
                with nc.allow_low_precision(reason="gexp fp16"):
                    nc.vector.tensor_copy(ge3[:, tb, :], ps[:])

            # gate: z = loc_z + ge_z; g0 = 0.5 + 0.5*tanh(z/2); g1 = 1 - g0
            zsum = zp.tile([128, 32], FP32, name="zsum")
            nc.vector.tensor_add(zsum[:], loc3[:, :, 128], ge3[:, :, 128])
            tanh_t = zp.tile([128, 32], FP32, name="tanh_t")
            nc.scalar.activation(tanh_t[:], zsum[:], AF.Tanh, scale=0.5)
            g0 = zp.tile([128, 32], FP32, name="g0")
            g1 = zp.tile([128, 32], FP32, name="g1")
            nc.vector.tensor_scalar(g0[:], tanh_t[:], 0.5, 0.5,
                                    mybir.AluOpType.mult, mybir.AluOpType.add)
            nc.vector.tensor_scalar(g1[:], tanh_t[:], -0.5, 0.5,
                                    mybir.AluOpType.mult, mybir.AluOpType.add)

            for tb in range(32):
                o = outp.tile([128, 128], FP16, name="outl")
                with nc.allow_low_precision(reason="out fp16"):
                    nc.vector.tensor_scalar_mul(o[:], loc3[:, tb, 0:128],
                                                g0[:, tb:tb + 1])
                nc.sync.dma_start(out=out_loc[128 * tb:128 * tb + 128, :], in_=o[:])
                o2 = outp.tile([128, 128], FP16, name="outg")
                with nc.allow_low_precision(reason="out fp16"):
                    nc.vector.tensor_scalar_mul(o2[:], ge3[:, tb, 0:128],
                                                g1[:, tb:tb + 1])
                nc.sync.dma_start(out=out_glob[128 * tb:128 * tb + 128, :], in_=o2[:])

    nc.finalize()
    return nc


# ---------------------------------------------------------------------------
# Host side
# ---------------------------------------------------------------------------

_NC_CACHE = []


def _get_program():
    if not _NC_CACHE:
        _NC_CACHE.append(build_program())
    return _NC_CACHE[0]


def _prep_inputs(x, w_lqkv, b_lqkv, w_gqkv, b_gqkv, w_comp, b_comp,
                 w_lproj, b_lproj, w_gproj, b_gproj, w_gate, b_gate):
    f32, f16 = np.float32, np.float16
    wd = (w_gate[:, 0] - w_gate[:, 1]).astype(f32)
    u_l = (w_lproj @ wd[:LD]).astype(f32)
    u_g = (w_gproj @ wd[LD:]).astype(f32)
    c0 = float(b_lproj @ wd[:LD] + b_gproj @ wd[LD:] + b_gate[0] - b_gate[1])

    mask_tri = np.where(np.arange(128)[None, :] >= np.arange(128)[:, None],
                        0.0, NEG).astype(f16)
    e0 = np.zeros((64, 128), f32)
    e0[np.arange(128) // 4, np.arange(128)] = 1.0
    e1 = np.zeros((64, 128), f32)
    e1[32 + np.arange(128) // 4, np.arange(128)] = 1.0
    repA_ = np.concatenate([e0, e0], axis=0).astype(f16)
    repB_ = np.concatenate([e1, e1], axis=0).astype(f16)

    def packed_cols(w, b, off, ha, hb, scale=1.0):
        wp = np.concatenate([w[:, off + D * ha:off + D * ha + D],
                             w[:, off + D * hb:off + D * hb + D]], axis=1) * scale
        bp = np.concatenate([b[off + D * ha:off + D * ha + D],
                             b[off + D * hb:off + D * hb + D]]) * scale
        return wp.astype(f16).reshape(4, 128, 128), bp.astype(f32).reshape(128, 1)

    in_maps = []
    for core in range(NCORES):
        b_idx, g = core // 4, core % 4
        ha, hb = 2 * g, 2 * g + 1
        cs = slice(128 * g, 128 * g + 128)

        xlt_ = np.ascontiguousarray(x[b_idx, :, :LD].T).astype(f16).reshape(4, 128, T)
        xct_ = np.ascontiguousarray(
            x[b_idx].reshape(Tc, R * E).T).astype(f16).reshape(32, 128, Tc)

        wq_, bq_ = packed_cols(w_lqkv, b_lqkv, 0, ha, hb, 1.0 / 8.0)
        wk_, bk_ = packed_cols(w_lqkv, b_lqkv, LD, ha, hb)
        wv_, bv_ = packed_cols(w_lqkv, b_lqkv, 2 * LD, ha, hb)
        wgq_, bgq_ = packed_cols(w_gqkv, b_gqkv, 0, ha, hb, 1.0 / 8.0)
        wgk_, bgk_ = packed_cols(w_gqkv, b_gqkv, LD, ha, hb)
        wgv_, bgv_ = packed_cols(w_gqkv, b_gqkv, 2 * LD, ha, hb)

        wplz_ = np.concatenate(
            [w_lproj[:, cs], u_l[:, None]], axis=1).astype(f16).reshape(4, 128, 129)
        wpgz_ = np.concatenate(
            [w_gproj[:, cs], u_g[:, None]], axis=1).astype(f16).reshape(4, 128, 129)

        in_maps.append({
            "xlt": xlt_, "xct": xct_,
            "wq": wq_, "bq": bq_, "wk": wk_, "bk": bk_,
            "wv": wv_,
            "bvb": np.tile(bv_.reshape(1, 128), (128, 1)).astype(f16),
            "wgq": wgq_, "bgq": bgq_, "wgk": wgk_, "bgk": bgk_,
            "wgv": wgv_,
            "bgvb": np.tile(bgv_.reshape(1, 128), (128, 1)).astype(f16),
            "wc": np.ascontiguousarray(
                w_comp[:, LD + 128 * g:LD + 128 * g + 128]).astype(f16)
                .reshape(32, 128, 128),
            "bc": b_comp[LD + 128 * g:LD + 128 * g + 128].astype(f32)
                .reshape(128, 1),
            "wplz": wplz_,
            "bplzb": np.tile(np.concatenate([b_lproj[cs], [c0]])
                             .reshape(1, 129), (128, 1)).astype(f16),
            "wpgz": wpgz_,
            "bpgzb": np.tile(np.concatenate([b_gproj[cs], [0.0]])
                             .reshape(1, 129), (128, 1)).astype(f16),
            "repA": repA_, "repB": repB_, "maskt": mask_tri,
        })
    return in_maps


def _run(in_maps, trace=False):
    nc = _get_program()
    return run_bass_kernel_spmd(nc, in_maps, list(range(NCORES)), trace=trace)


def assemble(results):
    out = np.empty((B, T, E), np.float32)
    for core in range(NCORES):
        b_idx, g = core // 4, core % 4
        out[b_idx, :, 128 * g:128 * g + 128] = \
            results[core]["out_loc"].astype(np.float32)
        out[b_idx, :, LD + 128 * g:LD + 128 * g + 128] = \
            results[core]["out_glob"].astype(np.float32)
    return out


def kernel(**inputs):
    in_maps = _prep_inputs(**inputs)
    res = _run(in_maps)
    return assemble(res.results)


def kernel_traced(**inputs):
    """test.py helper: returns (output, BassKernelResults with timing)."""
    in_maps = _prep_inputs(**inputs)
    res = _run(in_maps, trace=True)
    return assemble(res.results), res


# revision 43
# speedup vs baseline: 1.6623x; 1.2124x over previous
"""DualResolutionAttention Trainium2 kernel (8 NeuronCores, Bass/Tile).

Sharding: core c -> (batch b = c//4, group g = c%4).
Each core computes local heads {2g, 2g+1} and global heads {2g, 2g+1} over the
full sequence, plus the output channel slice [128g, 128g+128) of each branch.
Three AllGathers within each 4-core batch group: (1) compressed stream cgT,
(2) normalized local attention (fp16), (3) normalized global attention (fp16).

v2 design (vs baseline): fp16 compute everywhere (FWL weight loads, 2x less
DMA/SBUF), V computed directly in [token, vdim] layout (no PE transposes),
q/k evicted straight from PSUM to qT/kT (packed per-head weights), gate logits
folded into the projection matmuls as a 129th output column, masks via 64-row
identity matmuls (no PE tiling-mode switch inside attention).
"""
import os
import sys

sys.path.insert(0, "/opt/trn_rl_repo")
os.environ.setdefault("JAX_PLATFORMS", "axon,cpu")

from contextlib import ExitStack

import numpy as np

import concourse.bass as bass
import concourse.mybir as mybir
import concourse.tile as tile
from concourse import bacc
from concourse.bass_utils import run_bass_kernel_spmd
from concourse.masks import make_identity

FP32 = mybir.dt.float32
FP16 = mybir.dt.float16
AF = mybir.ActivationFunctionType

# Problem constants
B, T, E = 2, 4096, 1024
LD = 512            # local/global stream dim
D = 64              # head dim
HH = 8              # heads per branch
R = 4               # compression ratio
Tc = T // R         # 1024
NCORES = 8
GROUPS = [[0, 1, 2, 3], [4, 5, 6, 7]]

NEG = -30000.0      # fp16-safe mask value


# ---------------------------------------------------------------------------
# Program builder
# ---------------------------------------------------------------------------

def _attention(nc, tc, ctx, name, nQT2, qT, kTh, v_sb, comb, rec, consts,
               ones2, contribs, gathereds, anp):
    """Attention body: S^T layout scores, [v|ones] PV with denominator row.

    All matmuls run in 128-row tiling mode (kTh[h] is the per-head key tile
    with the other head's partition half zeroed), so the PE never pays a
    tiling-mode-switch drain.  The kb loop runs per 512-query half with a
    double-buffered [128, 1024] score PSUM tile (both heads side by side,
    one Exp per iteration), so the scores->exp WAR never stalls the PE.

    comb[h] is a [65, nQT2*1024] fp16 tile: rows 0:64 = unnormalized attT,
    row 64 = softmax denominator.  rec[h] [1, ncols] gets 1/denominator
    (computed in a [128, 8] layout via DMA reshape - reciprocal is
    8 cyc/elem on the DVE).  Each q2 chunk is normalized and AllGathered
    separately (contribs[q2] -> gathereds[q2]) so the collectives overlap
    later compute; the normalize matmuls for chunk q2 are deferred and
    emitted a few kb into chunk q2+1 (the PE never waits on the reciprocal
    chain).  Returns the last chunk's un-emitted normalize closure - the
    caller must invoke it after emitting some independent PE work.
    """
    ps_s = ctx.enter_context(tc.tile_pool(name=f"{name}_ps_s", bufs=2, space="PSUM"))
    ps_o = ctx.enter_context(tc.tile_pool(name=f"{name}_ps_o", bufs=1, space="PSUM"))
    p_pool = ctx.enter_context(tc.tile_pool(name=f"{name}_p", bufs=3))
    dnp = ctx.enter_context(tc.tile_pool(name=f"{name}_dn", bufs=2))

    mask_tri = consts["mask_tri"]
    ident = consts["ident"]

    def emit_scores(q2, qs, kb):
        t0 = max(0, 128 * kb - 1024 * q2 - qs)   # mask start within the half
        ps2 = ps_s.tile([128, 1024], FP32, name=f"{name}_s2", tag=f"{name}_s2")
        p_sb = p_pool.tile([128, 1024], FP16, name=f"{name}_pt")
        for h in range(2):
            nc.tensor.matmul(
                ps2[:, 512 * h:512 * h + 512],
                kTh[h][:, 128 * kb:128 * kb + 128],
                qT[:, 1024 * q2 + qs:1024 * q2 + qs + 512],
                start=True, stop=True,
            )
        if t0 > 0 or 128 * kb >= 1024 * q2 + qs:
            for h in range(2):
                nc.tensor.matmul(
                    ps2[:, 512 * h + t0:512 * h + t0 + 128],
                    ident[:], mask_tri[:],
                    start=False, stop=True, skip_group_check=True,
                )
        if t0 == 0:
            nc.scalar.activation(p_sb[:], ps2[:], AF.Exp)
        else:
            p3 = p_sb.rearrange("p (h c) -> p h c", h=2, c=512)
            s3 = ps2.rearrange("p (h c) -> p h c", h=2, c=512)
            nc.scalar.activation(p3[:, :, t0:512], s3[:, :, t0:512], AF.Exp)
        return p_sb, t0

    def emit_pv(h, kb, nkb_h, qs, psum_o, p_sb, t0):
        nc.tensor.matmul(
            psum_o[:, qs + t0:qs + 512],
            v_sb[kb][:, 65 * h:65 * h + 65],
            p_sb[:, 512 * h + t0:512 * h + 512],
            start=(kb == 0), stop=(kb == nkb_h - 1),
            skip_group_check=True,
        )

    def make_norm(q2):
        def flush(pool=None, tag=None):
            pool = pool if pool is not None else ps_s
            tag = tag if tag is not None else f"{name}_s2"
            contrib, gathered = contribs[q2], gathereds[q2]
            for h in range(2):
                attn = anp.tile([64, 1024], FP16, name=f"{name}_attn")
                for c2 in range(2):
                    ps = pool.tile([128, 512], FP32, name=f"{name}_bc",
                                   tag=tag)
                    nc.tensor.matmul(
                        ps[:], ones2[0:1, :],
                        rec[h][0:1, 1024 * q2 + 512 * c2:
                               1024 * q2 + 512 * c2 + 512],
                        start=True, stop=True)
                    with nc.allow_low_precision(reason="attnorm fp16"):
                        nc.vector.tensor_mul(
                            attn[:, 512 * c2:512 * c2 + 512],
                            comb[h][0:64, 1024 * q2 + 512 * c2:
                                    1024 * q2 + 512 * c2 + 512],
                            ps[0:64, :])
                nc.sync.dma_start(out=contrib[64 * h:64 * h + 64, :],
                                  in_=attn[:])
            nc.gpsimd.collective_compute(
                "AllGather", mybir.AluOpType.bypass, replica_groups=GROUPS,
                ins=[contrib.opt()], outs=[gathered.opt()],
            )
        return flush

    pending = None
    for q2 in range(nQT2):
        psum_o = [ps_o.tile([65, 1024], FP32, name=f"{name}_o{h}")
                  for h in range(2)]
        for half in range(2):
            qs = 512 * half
            nkb_h = 8 * q2 + 4 * (half + 1)
            pend = None
            for kb in range(nkb_h):
                cur = emit_scores(q2, qs, kb)
                if pending is not None and kb == 2:
                    pending()
                    pending = None
                if pend is not None:
                    p_sb, t0 = pend
                    for h in range(2):
                        emit_pv(h, kb - 1, nkb_h, qs, psum_o[h], p_sb, t0)
                pend = cur
            p_sb, t0 = pend
            for h in range(2):
                emit_pv(h, nkb_h - 1, nkb_h, qs, psum_o[h], p_sb, t0)
        for h in range(2):
            # one copy evicts both attT rows and the denominator row
            with nc.allow_low_precision(reason="att fp16"):
                nc.vector.tensor_copy(
                    comb[h][:, 1024 * q2:1024 * q2 + 1024], psum_o[h][:])
            # reciprocal in [128, 8] layout (DMA reshape there and back)
            dh = dnp.tile([128, 8], FP16, name=f"{name}_dh")
            nc.sync.dma_start(
                out=dh[:], in_=comb[h][64:65, 1024 * q2:1024 * q2 + 1024])
            rc = dnp.tile([128, 8], FP16, name=f"{name}_rc")
            with nc.allow_low_precision(reason="softmax denom fp16"):
                nc.vector.reciprocal(rc[:], dh[:])
            nc.sync.dma_start(
                out=rec[h][0:1, 1024 * q2:1024 * q2 + 1024], in_=rc[:])
        pending = make_norm(q2)
    return pending


def build_program():
    nc = bacc.Bacc(None, target_bir_lowering=False)

    def inp(name, shape, dt=FP16):
        return nc.declare_dram_parameter(name, list(shape), dt, isOutput=False)

    # data
    xlt = inp("xlt", [4, 128, T])            # x[b,:,:512].T chunks
    xct = inp("xct", [32, 128, Tc])          # x[b].reshape(Tc,4096).T chunks
    # weights
    wq = inp("wq", [4, 128, 128])            # [qA|qB] lhsT chunks (scaled 1/8)
    bq = inp("bq", [128, 1], FP32)
    wk = inp("wk", [4, 128, 128])
    bk = inp("bk", [128, 1], FP32)
    wv = inp("wv", [4, 128, 128])            # [vA|vB] (rhs for v-direct)
    bvb = inp("bvb", [128, 128])             # [bvA|bvB] replicated to 128 rows
    wgq = inp("wgq", [4, 128, 128])
    bgq = inp("bgq", [128, 1], FP32)
    wgk = inp("wgk", [4, 128, 128])
    bgk = inp("bgk", [128, 1], FP32)
    wgv = inp("wgv", [4, 128, 128])
    bgvb = inp("bgvb", [128, 128])
    wc = inp("wc", [32, 128, 128])           # compress slice lhsT chunks
    bc = inp("bc", [128, 1], FP32)
    wplz = inp("wplz", [4, 128, 129])        # [w_lproj[:,cs] | u_l] chunks
    bplzb = inp("bplzb", [128, 129])         # [b_lproj[cs] | c0] replicated
    wpgz = inp("wpgz", [4, 128, 129])
    bpgzb = inp("bpgzb", [128, 129])
    repA = inp("repA", [128, 128])           # x4 expander (even 32-blocks)
    repB = inp("repB", [128, 128])           # x4 expander (odd 32-blocks)
    maskt = inp("maskt", [128, 128])         # strict lower-tri NEG
    out_loc = nc.declare_dram_parameter("out_loc", [T, 128], FP16, isOutput=True)
    out_glob = nc.declare_dram_parameter("out_glob", [T, 128], FP16, isOutput=True)

    with tile.TileContext(nc) as tc:
      with ExitStack() as top:
        dram = top.enter_context(tc.tile_pool(name="dram", bufs=1, space="DRAM"))
        const = top.enter_context(tc.tile_pool(name="const", bufs=1))
        persist = top.enter_context(tc.tile_pool(name="persist", bufs=1))

        # constants
        ident = const.tile([128, 128], FP16, name="ident")
        make_identity(nc, ident[:])
        mask_tri = const.tile([128, 128], FP16, name="mask_tri")
        nc.sync.dma_start(out=mask_tri[:], in_=maskt[:])
        repA_sb = const.tile([128, 128], FP16, name="repA_sb")
        nc.sync.dma_start(out=repA_sb[:], in_=repA[:])
        repB_sb = const.tile([128, 128], FP16, name="repB_sb")
        nc.sync.dma_start(out=repB_sb[:], in_=repB[:])
        ones2 = const.tile([1, 128], FP16, name="ones2")
        nc.gpsimd.memset(ones2[:], 1.0)
        consts = {"mask_tri": mask_tri, "ident": ident}
        biases = {}
        for nm, src in (("bq", bq), ("bk", bk), ("bgq", bgq), ("bgk", bgk),
                        ("bc", bc)):
            t = const.tile([128, 1], FP32, name=f"cb_{nm}")
            nc.sync.dma_start(out=t[:], in_=src[:])
            biases[nm] = t
        brows = {}
        for nm, src, w in (("bvb", bvb, 128), ("bgvb", bgvb, 128),
                           ("bplzb", bplzb, 129), ("bpgzb", bpgzb, 129)):
            t = const.tile([128, w], FP16, name=f"br_{nm}")
            nc.sync.dma_start(out=t[:], in_=src[:])
            brows[nm] = t

        # persistent attention inputs.  kT is stored per head with the other
        # head's partition half zeroed so score matmuls run at K=128 (no PE
        # tiling-mode switches, FWL-eligible weight loads).
        qT_l = persist.tile([128, T], FP16, name="qT_l")
        kT_lh = [persist.tile([128, T], FP16, name=f"kT_l{h}") for h in range(2)]
        qT_g = persist.tile([128, Tc], FP16, name="qT_g")
        kT_gh = [persist.tile([128, Tc], FP16, name=f"kT_g{h}") for h in range(2)]
        nc.gpsimd.memset(kT_lh[0][64:128, :], 0.0)
        nc.gpsimd.memset(kT_lh[1][0:64, :], 0.0)
        nc.gpsimd.memset(kT_gh[0][64:128, :], 0.0)
        nc.gpsimd.memset(kT_gh[1][0:64, :], 0.0)
        v_sb_l = [persist.tile([128, 130], FP16, name=f"vsb{i}")
                  for i in range(32)]
        v_sb_g = [persist.tile([128, 130], FP16, name=f"vgsb{i}")
                  for i in range(8)]
        cg_all = [persist.tile([128, Tc], FP16, name=f"cg_all{i}")
                  for i in range(4)]
        # ones columns for the PV denominator row (cols 64 and 129)
        for v_tiles in (v_sb_l, v_sb_g):
            for vt in v_tiles:
                nc.vector.memset(vt[:, 64:65], 1.0)
                nc.vector.memset(vt[:, 129:130], 1.0)

        # ------------------------------------------------------ phase B: local qkv
        pab = top.enter_context(ExitStack())
        with ExitStack() as pb:
            xlp = pb.enter_context(tc.tile_pool(name="xlt_pool", bufs=1))
            wqp = pb.enter_context(tc.tile_pool(name="wq_pool", bufs=1))
            psB = pb.enter_context(tc.tile_pool(name="psB", bufs=3, space="PSUM"))
            psV = pb.enter_context(tc.tile_pool(name="psV", bufs=2, space="PSUM"))

            xlt_sb = []
            for ch in range(4):
                xt = xlp.tile([128, T], FP16, name=f"xlt{ch}")
                nc.sync.dma_start(out=xt[:], in_=xlt[ch])
                xlt_sb.append(xt)
            wq_sb, wk_sb, wv_sb = [], [], []
            for ch in range(4):
                for (lst, src, nm) in ((wq_sb, wq, "wq"), (wk_sb, wk, "wk"),
                                       (wv_sb, wv, "wv")):
                    t = wqp.tile([128, 128], FP16, name=f"{nm}{ch}")
                    nc.sync.dma_start(out=t[:], in_=src[ch])
                    lst.append(t)

            # q and k: packed [headA|headB] out dims -> direct eviction
            kT_st = xlp.tile([128, T], FP16, name="kT_st")
            for (wsb, dst, bias_ap) in ((wq_sb, qT_l, biases["bq"][:]),
                                        (wk_sb, kT_st, biases["bk"][:])):
                for qt in range(8):
                    ps = psB.tile([128, 512], FP32, name="psB_t")
                    for ch in range(4):
                        nc.tensor.matmul(
                            ps[:], wsb[ch][:],
                            xlt_sb[ch][:, 512 * qt:512 * qt + 512],
                            start=(ch == 0), stop=(ch == 3))
                    with nc.allow_low_precision(reason="qk fp16"):
                        nc.scalar.activation(dst[:, 512 * qt:512 * qt + 512],
                                             ps[:], AF.Identity, bias=bias_ap)
            # split k into per-head zero-padded tiles (partition-preserving)
            nc.sync.dma_start(out=kT_lh[0][0:64, :], in_=kT_st[0:64, :])
            nc.sync.dma_start(out=kT_lh[1][64:128, :], in_=kT_st[64:128, :])

            # v: direct [token, vdim] layout, bias added at eviction
            bvb3 = brows["bvb"].rearrange("p (h c) -> p h c", h=2, c=64)
            for tb in range(32):
                ps = psV.tile([128, 128], FP32, name="psV_t")
                for ch in range(4):
                    nc.tensor.matmul(
                        ps[:], xlt_sb[ch][:, 128 * tb:128 * tb + 128],
                        wv_sb[ch][:], start=(ch == 0), stop=(ch == 3))
                v3 = v_sb_l[tb].rearrange("p (h c) -> p h c", h=2, c=65)
                p3 = ps.rearrange("p (h c) -> p h c", h=2, c=64)
                with nc.allow_low_precision(reason="v fp16"):
                    nc.vector.tensor_add(v3[:, :, 0:64], p3[:], bvb3[:])

        # ------------------------------------------------------ phase A: compress
        xp = pab.enter_context(tc.tile_pool(name="xct_pool", bufs=8))
        wp = pab.enter_context(tc.tile_pool(name="wc_pool", bufs=8))
        cgp = pab.enter_context(tc.tile_pool(name="cg_pool", bufs=1))
        psA = pab.enter_context(tc.tile_pool(name="psA", bufs=1, space="PSUM"))
        cgT = cgp.tile([128, Tc], FP16, name="cgT")
        ps0 = psA.tile([128, 512], FP32, name="psA_0")
        ps1 = psA.tile([128, 512], FP32, name="psA_1")
        for ch in range(32):
            wt = wp.tile([128, 128], FP16, name="wc_t")
            nc.sync.dma_start(out=wt[:], in_=wc[ch])
            xt = xp.tile([128, Tc], FP16, name="xct_t")
            nc.sync.dma_start(out=xt[:], in_=xct[ch])
            nc.tensor.matmul(ps0[:], wt[:], xt[:, 0:512],
                             start=(ch == 0), stop=(ch == 31))
            nc.tensor.matmul(ps1[:], wt[:], xt[:, 512:1024],
                             start=(ch == 0), stop=(ch == 31))
        with nc.allow_low_precision(reason="cg fp16"):
            nc.scalar.activation(cgT[:, 0:512], ps0[:],
                                 AF.Identity, bias=biases["bc"][:])
            nc.scalar.activation(cgT[:, 512:1024], ps1[:],
                                 AF.Identity, bias=biases["bc"][:])
        cg_contrib = dram.tile([128, Tc], FP16, name="cg_contrib")
        cg_gathered = dram.tile([512, Tc], FP16, name="cg_gathered")
        nc.sync.dma_start(out=cg_contrib[:], in_=cgT[:])
        nc.gpsimd.collective_compute(
            "AllGather", mybir.AluOpType.bypass, replica_groups=GROUPS,
            ins=[cg_contrib.opt()], outs=[cg_gathered.opt()],
        )
        for i in range(4):
            nc.sync.dma_start(out=cg_all[i][:],
                              in_=cg_gathered[128 * i:128 * i + 128, :])
        pab.close()

        # ------------------------------------------------------ phase C: attention
        cpool = top.enter_context(tc.tile_pool(name="c_pool", bufs=1))
        comb_l = [cpool.tile([65, T], FP16, name=f"comb_l{h}") for h in range(2)]
        comb_g = [cpool.tile([65, Tc], FP16, name=f"comb_g{h}") for h in range(2)]
        rec_l = [cpool.tile([1, T], FP16, name=f"rec_l{h}") for h in range(2)]
        rec_g = [cpool.tile([1, Tc], FP16, name=f"rec_g{h}") for h in range(2)]

        contribs_l = [dram.tile([128, Tc], FP16, name=f"attnl_c{i}")
                      for i in range(4)]
        gathereds_l = [dram.tile([512, Tc], FP16, name=f"attnl_g{i}")
                       for i in range(4)]
        contrib_g = dram.tile([128, Tc], FP16, name="attng_contrib")
        gathered_g = dram.tile([512, Tc], FP16, name="attng_gathered")

        app = top.enter_context(tc.tile_pool(name="attall_pool", bufs=1))
        att_all = [app.tile([128, T], FP16, name=f"attall{i}") for i in range(4)]
        anp_top = top.enter_context(tc.tile_pool(name="anp_top", bufs=2))

        pc1 = top.enter_context(ExitStack())
        pend_l = _attention(nc, tc, pc1, "la", 4, qT_l, kT_lh, v_sb_l, comb_l,
                            rec_l, consts, ones2, contribs_l, gathereds_l,
                            anp_top)
        pc1.close()
        # att_all chunk DMAs for the already-gathered q2 chunks
        for q2 in range(3):
            for i in range(4):
                nc.sync.dma_start(
                    out=att_all[i][:, 1024 * q2:1024 * q2 + 1024],
                    in_=gathereds_l[q2][128 * i:128 * i + 128, :])

        # global qkv emitted before the last local normalize chunk so the PE
        # stream never stalls on the reciprocal/bcast chain
        with ExitStack() as pg:
            wgp = pg.enter_context(tc.tile_pool(name="wg_pool", bufs=1))
            psG = pg.enter_context(tc.tile_pool(name="psG", bufs=3, space="PSUM"))
            psGV = pg.enter_context(tc.tile_pool(name="psGV", bufs=2, space="PSUM"))
            wgq_sb, wgk_sb, wgv_sb = [], [], []
            for ch in range(4):
                for (lst, src, nm) in ((wgq_sb, wgq, "wgq"), (wgk_sb, wgk, "wgk"),
                                       (wgv_sb, wgv, "wgv")):
                    t = wgp.tile([128, 128], FP16, name=f"{nm}{ch}")
                    nc.sync.dma_start(out=t[:], in_=src[ch])
                    lst.append(t)
            kTg_st = wgp.tile([128, Tc], FP16, name="kTg_st")
            for (wsb, dst, bias_ap) in ((wgq_sb, qT_g, biases["bgq"][:]),
                                        (wgk_sb, kTg_st, biases["bgk"][:])):
                for qt in range(2):
                    ps = psG.tile([128, 512], FP32, name="psG_t")
                    for ch in range(4):
                        nc.tensor.matmul(
                            ps[:], wsb[ch][:],
                            cg_all[ch][:, 512 * qt:512 * qt + 512],
                            start=(ch == 0), stop=(ch == 3))
                    with nc.allow_low_precision(reason="gqk fp16"):
                        nc.scalar.activation(dst[:, 512 * qt:512 * qt + 512],
                                             ps[:], AF.Identity, bias=bias_ap)
            nc.sync.dma_start(out=kT_gh[0][0:64, :], in_=kTg_st[0:64, :])
            nc.sync.dma_start(out=kT_gh[1][64:128, :], in_=kTg_st[64:128, :])
            # flush the last local normalize chunk now that the PE has
            # independent global-qkv work queued ahead of it
            pend_l(psG, "psG_t")
            for i in range(4):
                nc.sync.dma_start(
                    out=att_all[i][:, 3072:4096],
                    in_=gathereds_l[3][128 * i:128 * i + 128, :])
            bgvb3 = brows["bgvb"].rearrange("p (h c) -> p h c", h=2, c=64)
            for tb in range(8):
                ps = psGV.tile([128, 128], FP32, name="psGV_t")
                for ch in range(4):
                    nc.tensor.matmul(
                        ps[:], cg_all[ch][:, 128 * tb:128 * tb + 128],
                        wgv_sb[ch][:], start=(ch == 0), stop=(ch == 3))
                v3 = v_sb_g[tb].rearrange("p (h c) -> p h c", h=2, c=65)
                p3 = ps.rearrange("p (h c) -> p h c", h=2, c=64)
                with nc.allow_low_precision(reason="gv fp16"):
                    nc.vector.tensor_add(v3[:, :, 0:64], p3[:], bgvb3[:])

        pc2 = top.enter_context(ExitStack())
        pend_g = _attention(nc, tc, pc2, "ga", 1, qT_g, kT_gh, v_sb_g, comb_g,
                            rec_g, consts, ones2, [contrib_g], [gathered_g],
                            anp_top)
        pc2.close()

        # ------------------------------------------------------ phase D: proj+gate
        with ExitStack() as pd:
            wpp = pd.enter_context(tc.tile_pool(name="wp_pool", bufs=1))
            psP = pd.enter_context(tc.tile_pool(name="psP", bufs=3, space="PSUM"))
            psE = pd.enter_context(tc.tile_pool(name="psE", bufs=2, space="PSUM"))
            gp = pd.enter_context(tc.tile_pool(name="gproj_pool", bufs=1))
            zp = pd.enter_context(tc.tile_pool(name="z_pool", bufs=1))
            outp = pd.enter_context(tc.tile_pool(name="out_pool", bufs=4))

            wplz_sb, wpgz_sb = [], []
            for ch in range(4):
                t = wpp.tile([128, 129], FP16, name=f"wplz{ch}")
                nc.sync.dma_start(out=t[:], in_=wplz[ch])
                wplz_sb.append(t)
                t = wpp.tile([128, 129], FP16, name=f"wpgz{ch}")
                nc.sync.dma_start(out=t[:], in_=wpgz[ch])
                wpgz_sb.append(t)

            # local proj: [128 tok, 129] blocks -> loc_sb
            loc_sb = gp.tile([128, 32 * 129], FP16, name="loc_sb")
            loc3 = loc_sb.rearrange("p (a b) -> p a b", a=32, b=129)
            for tb in range(32):
                ps = psP.tile([128, 129], FP32, name="psP_t")
                for ch in range(4):
                    nc.tensor.matmul(ps[:],
                                     att_all[ch][:, 128 * tb:128 * tb + 128],
                                     wplz_sb[ch][:], start=(ch == 0), stop=(ch == 3))
                with nc.allow_low_precision(reason="proj fp16"):
                    nc.vector.tensor_add(loc3[:, tb, :], ps[:], brows["bplzb"][:])

            # global normalize + gather overlaps the local proj matmuls
            pend_g(psP, "psP_t")
            attg_all = []
            for i in range(4):
                t = wpp.tile([128, Tc], FP16, name=f"attgall{i}")
                nc.sync.dma_start(out=t[:],
                                  in_=gathered_g[128 * i:128 * i + 128, :])
                attg_all.append(t)

            # global proj (Tc rows) -> gproj_sb, then x4 expand -> ge_sb
            gproj_sb = gp.tile([128, 8 * 129], FP16, name="gproj_sb")
            gproj3 = gproj_sb.rearrange("p (a b) -> p a b", a=8, b=129)
            for tbg in range(8):
                ps = psP.tile([128, 129], FP32, name="psP_t")
                for ch in range(4):
                    nc.tensor.matmul(ps[:],
                                     attg_all[ch][:, 128 * tbg:128 * tbg + 128],
                                     wpgz_sb[ch][:], start=(ch == 0), stop=(ch == 3))
                with nc.allow_low_precision(reason="gproj fp16"):
                    nc.vector.tensor_add(gproj3[:, tbg, :], ps[:], brows["bpgzb"][:])
            ge_sb = gp.tile([128, 32 * 129], FP16, name="ge_sb")
            ge3 = ge_sb.rearrange("p (a b) -> p a b", a=32, b=129)
            for tb in range(32):
                base = 64 * ((tb % 4) // 2)
                rep = repA_sb if tb % 2 == 0 else repB_sb
                ps = psE.tile([128, 129], FP32, name="psE_t")
                nc.tensor.matmul(ps[:], rep[base:base + 64, :],
                                 gproj3[base:base + 64, tb // 4, :],
                                 start=True, stop=True)
                with nc.allow_low_precision(reason="gexp fp16"):
                    nc.vector.tensor_copy(ge3[:, tb, :], ps[:])

            # gate in groups of 8 blocks: z = loc_z + ge_z;
            # g0 = 0.5 + 0.5*tanh(z/2); g1 = 1 - g0; batched out DMAs
            for grp in range(4):
                b0 = 8 * grp
                zsum = zp.tile([128, 8], FP32, name="zsum")
                nc.vector.tensor_add(zsum[:], loc3[:, b0:b0 + 8, 128],
                                     ge3[:, b0:b0 + 8, 128])
                tanh_t = zp.tile([128, 8], FP32, name="tanh_t")
                nc.scalar.activation(tanh_t[:], zsum[:], AF.Tanh, scale=0.5)
                g0 = zp.tile([128, 8], FP32, name="g0")
                g1 = zp.tile([128, 8], FP32, name="g1")
                nc.vector.tensor_scalar(g0[:], tanh_t[:], 0.5, 0.5,
                                        mybir.AluOpType.mult,
                                        mybir.AluOpType.add)
                nc.vector.tensor_scalar(g1[:], tanh_t[:], -0.5, 0.5,
                                        mybir.AluOpType.mult,
                                        mybir.AluOpType.add)
                ol = outp.tile([128, 8 * 128], FP16, name="outl")
                ol3 = ol.rearrange("p (b c) -> p b c", b=8, c=128)
                og = outp.tile([128, 8 * 128], FP16, name="outg")
                og3 = og.rearrange("p (b c) -> p b c", b=8, c=128)
                for j in range(8):
                    tb = b0 + j
                    with nc.allow_low_precision(reason="out fp16"):
                        nc.vector.tensor_scalar_mul(ol3[:, j, :],
                                                    loc3[:, tb, 0:128],
                                                    g0[:, j:j + 1])
                        nc.vector.tensor_scalar_mul(og3[:, j, :],
                                                    ge3[:, tb, 0:128],
                                                    g1[:, j:j + 1])
                out_l_v = out_loc[1024 * grp:1024 * grp + 1024, :] \
                    .rearrange("(b p) c -> p b c", b=8, p=128)
                nc.sync.dma_start(out=out_l_v, in_=ol3[:])
                out_g_v = out_glob[1024 * grp:1024 * grp + 1024, :] \
                    .rearrange("(b p) c -> p b c", b=8, p=128)
                nc.sync.dma_start(out=out_g_v, in_=og3[:])

    nc.finalize()
    return nc


# ---------------------------------------------------------------------------
# Host side
# ---------------------------------------------------------------------------

_NC_CACHE = []


def _get_program():
    if not _NC_CACHE:
        _NC_CACHE.append(build_program())
    return _NC_CACHE[0]


def _prep_inputs(x, w_lqkv, b_lqkv, w_gqkv, b_gqkv, w_comp, b_comp,
                 w_lproj, b_lproj, w_gproj, b_gproj, w_gate, b_gate):
    f32, f16 = np.float32, np.float16
    wd = (w_gate[:, 0] - w_gate[:, 1]).astype(f32)
    u_l = (w_lproj @ wd[:LD]).astype(f32)
    u_g = (w_gproj @ wd[LD:]).astype(f32)
    c0 = float(b_lproj @ wd[:LD] + b_gproj @ wd[LD:] + b_gate[0] - b_gate[1])

    mask_tri = np.where(np.arange(128)[None, :] >= np.arange(128)[:, None],
                        0.0, NEG).astype(f16)
    e0 = np.zeros((64, 128), f32)
    e0[np.arange(128) // 4, np.arange(128)] = 1.0
    e1 = np.zeros((64, 128), f32)
    e1[32 + np.arange(128) // 4, np.arange(128)] = 1.0
    repA_ = np.concatenate([e0, e0], axis=0).astype(f16)
    repB_ = np.concatenate([e1, e1], axis=0).astype(f16)

    def packed_cols(w, b, off, ha, hb, scale=1.0):
        wp = np.concatenate([w[:, off + D * ha:off + D * ha + D],
                             w[:, off + D * hb:off + D * hb + D]], axis=1) * scale
        bp = np.concatenate([b[off + D * ha:off + D * ha + D],
                             b[off + D * hb:off + D * hb + D]]) * scale
        return wp.astype(f16).reshape(4, 128, 128), bp.astype(f32).reshape(128, 1)

    in_maps = []
    for core in range(NCORES):
        b_idx, g = core // 4, core % 4
        ha, hb = 2 * g, 2 * g + 1
        cs = slice(128 * g, 128 * g + 128)

        xlt_ = np.ascontiguousarray(x[b_idx, :, :LD].T).astype(f16).reshape(4, 128, T)
        xct_ = np.ascontiguousarray(
            x[b_idx].reshape(Tc, R * E).T).astype(f16).reshape(32, 128, Tc)

        wq_, bq_ = packed_cols(w_lqkv, b_lqkv, 0, ha, hb, 1.0 / 8.0)
        wk_, bk_ = packed_cols(w_lqkv, b_lqkv, LD, ha, hb)
        wv_, bv_ = packed_cols(w_lqkv, b_lqkv, 2 * LD, ha, hb)
        wgq_, bgq_ = packed_cols(w_gqkv, b_gqkv, 0, ha, hb, 1.0 / 8.0)
        wgk_, bgk_ = packed_cols(w_gqkv, b_gqkv, LD, ha, hb)
        wgv_, bgv_ = packed_cols(w_gqkv, b_gqkv, 2 * LD, ha, hb)

        wplz_ = np.concatenate(
            [w_lproj[:, cs], u_l[:, None]], axis=1).astype(f16).reshape(4, 128, 129)
        wpgz_ = np.concatenate(
            [w_gproj[:, cs], u_g[:, None]], axis=1).astype(f16).reshape(4, 128, 129)

        in_maps.append({
            "xlt": xlt_, "xct": xct_,
            "wq": wq_, "bq": bq_, "wk": wk_, "bk": bk_,
            "wv": wv_,
            "bvb": np.tile(bv_.reshape(1, 128), (128, 1)).astype(f16),
            "wgq": wgq_, "bgq": bgq_, "wgk": wgk_, "bgk": bgk_,
            "wgv": wgv_,
            "bgvb": np.tile(bgv_.reshape(1, 128), (128, 1)).astype(f16),
            "wc": np.ascontiguousarray(
                w_comp[:, LD + 128 * g:LD + 128 * g + 128]).astype(f16)
                .reshape(32, 128, 128),
            "bc": b_comp[LD + 128 * g:LD + 128 * g + 128].astype(f32)
                .reshape(128, 1),
            "wplz": wplz_,
            "bplzb": np.tile(np.concatenate([b_lproj[cs], [c0]])
                             .reshape(1, 129), (128, 1)).astype(f16),
            "wpgz": wpgz_,
            "bpgzb": np.tile(np.concatenate([b_gproj[cs], [0.0]])
                             .reshape(1, 129), (128, 1)).astype(f16),
            "repA": repA_, "repB": repB_, "maskt": mask_tri,
        })
    return in_maps


def _run(in_maps, trace=False):
    nc = _get_program()
    return run_bass_kernel_spmd(nc, in_maps, list(range(NCORES)), trace=trace)


def assemble(results):
    out = np.empty((B, T, E), np.float32)
    for core in range(NCORES):
        b_idx, g = core // 4, core % 4
        out[b_idx, :, 128 * g:128 * g + 128] = \
            results[core]["out_loc"].astype(np.float32)
        out[b_idx, :, LD + 128 * g:LD + 128 * g + 128] = \
            results[core]["out_glob"].astype(np.float32)
    return out


def kernel(**inputs):
    in_maps = _prep_inputs(**inputs)
    res = _run(in_maps)
    return assemble(res.results)


def kernel_traced(**inputs):
    """test.py helper: returns (output, BassKernelResults with timing)."""
    in_maps = _prep_inputs(**inputs)
    res = _run(in_maps, trace=True)
    return assemble(res.results), res


# revision 49
# speedup vs baseline: 1.6648x; 1.0015x over previous
"""DualResolutionAttention Trainium2 kernel (8 NeuronCores, Bass/Tile).

Sharding: core c -> (batch b = c//4, group g = c%4).
Each core computes local heads {2g, 2g+1} and global heads {2g, 2g+1} over the
full sequence, plus the output channel slice [128g, 128g+128) of each branch.
Three AllGathers within each 4-core batch group: (1) compressed stream cgT,
(2) normalized local attention (fp16), (3) normalized global attention (fp16).

v2 design (vs baseline): fp16 compute everywhere (FWL weight loads, 2x less
DMA/SBUF), V computed directly in [token, vdim] layout (no PE transposes),
q/k evicted straight from PSUM to qT/kT (packed per-head weights), gate logits
folded into the projection matmuls as a 129th output column, masks via 64-row
identity matmuls (no PE tiling-mode switch inside attention).
"""
import os
import sys

sys.path.insert(0, "/opt/trn_rl_repo")
os.environ.setdefault("JAX_PLATFORMS", "axon,cpu")

from contextlib import ExitStack

import numpy as np

import concourse.bass as bass
import concourse.mybir as mybir
import concourse.tile as tile
from concourse import bacc
from concourse.bass_utils import run_bass_kernel_spmd
from concourse.masks import make_identity

FP32 = mybir.dt.float32
FP16 = mybir.dt.float16
AF = mybir.ActivationFunctionType

# Problem constants
B, T, E = 2, 4096, 1024
LD = 512            # local/global stream dim
D = 64              # head dim
HH = 8              # heads per branch
R = 4               # compression ratio
Tc = T // R         # 1024
NCORES = 8
GROUPS = [[0, 1, 2, 3], [4, 5, 6, 7]]

NEG = -30000.0      # fp16-safe mask value


# ---------------------------------------------------------------------------
# Program builder
# ---------------------------------------------------------------------------

def _attention(nc, tc, ctx, name, nQT2, qT, kTh, v_sb, comb, rec, consts,
               ones2, contribs, gathereds, anp):
    """Attention body: S^T layout scores, [v|ones] PV with denominator row.

    All matmuls run in 128-row tiling mode (kTh[h] is the per-head key tile
    with the other head's partition half zeroed), so the PE never pays a
    tiling-mode-switch drain.  The kb loop runs per 512-query half with a
    double-buffered [128, 1024] score PSUM tile (both heads side by side,
    one Exp per iteration), so the scores->exp WAR never stalls the PE.

    comb[h] is a [65, nQT2*1024] fp16 tile: rows 0:64 = unnormalized attT,
    row 64 = softmax denominator.  rec[h] [1, ncols] gets 1/denominator
    (computed in a [128, 8] layout via DMA reshape - reciprocal is
    8 cyc/elem on the DVE).  Each q2 chunk is normalized and AllGathered
    separately (contribs[q2] -> gathereds[q2]) so the collectives overlap
    later compute; the normalize matmuls for chunk q2 are deferred and
    emitted a few kb into chunk q2+1 (the PE never waits on the reciprocal
    chain).  Returns the last chunk's un-emitted normalize closure - the
    caller must invoke it after emitting some independent PE work.
    """
    ps_s = ctx.enter_context(tc.tile_pool(name=f"{name}_ps_s", bufs=2, space="PSUM"))
    ps_o = ctx.enter_context(tc.tile_pool(name=f"{name}_ps_o", bufs=1, space="PSUM"))
    p_pool = ctx.enter_context(tc.tile_pool(name=f"{name}_p", bufs=3))
    dnp = ctx.enter_context(tc.tile_pool(name=f"{name}_dn", bufs=2))

    mask_tri = consts["mask_tri"]
    ident = consts["ident"]

    def emit_scores(q2, qs, kb):
        t0 = max(0, 128 * kb - 1024 * q2 - qs)   # mask start within the half
        has_mask = 128 * kb >= 1024 * q2 + qs
        ps2 = ps_s.tile([128, 1024], FP32, name=f"{name}_s2", tag=f"{name}_s2")
        p_sb = p_pool.tile([128, 1024], FP16, name=f"{name}_pt")
        for h in range(2):
            nc.tensor.matmul(
                ps2[:, 512 * h:512 * h + 512],
                kTh[h][:, 128 * kb:128 * kb + 128],
                qT[:, 1024 * q2 + qs:1024 * q2 + qs + 512],
                start=True, stop=True,
            )
        if has_mask:
            for h in range(2):
                nc.tensor.matmul(
                    ps2[:, 512 * h + t0:512 * h + t0 + 128],
                    ident[:], mask_tri[:],
                    start=False, stop=True, skip_group_check=True,
                )
        if not has_mask and kb % 3 == 2:
            # Schraudolph fast exp on the (otherwise idle) DVE: write the
            # fp16 bit pattern of e^x as an int16 affine transform.
            # bits = x * 2^10/ln2 + (15*2^10 - 61); max rel err ~4%.
            with nc.allow_low_precision(reason="schraudolph exp"):
                nc.vector.tensor_scalar(
                    p_sb.bitcast(mybir.dt.int16)[:], ps2[:],
                    1477.3197, 15299.0,
                    mybir.AluOpType.mult, mybir.AluOpType.add)
        elif t0 == 0:
            nc.scalar.activation(p_sb[:], ps2[:], AF.Exp)
        else:
            p3 = p_sb.rearrange("p (h c) -> p h c", h=2, c=512)
            s3 = ps2.rearrange("p (h c) -> p h c", h=2, c=512)
            nc.scalar.activation(p3[:, :, t0:512], s3[:, :, t0:512], AF.Exp)
        return p_sb, t0

    def emit_pv(h, kb, nkb_h, qs, psum_o, p_sb, t0):
        nc.tensor.matmul(
            psum_o[:, qs + t0:qs + 512],
            v_sb[kb][:, 65 * h:65 * h + 65],
            p_sb[:, 512 * h + t0:512 * h + 512],
            start=(kb == 0), stop=(kb == nkb_h - 1),
            skip_group_check=True,
        )

    def make_norm(q2):
        def flush(pool=None, tag=None):
            pool = pool if pool is not None else ps_s
            tag = tag if tag is not None else f"{name}_s2"
            contrib, gathered = contribs[q2], gathereds[q2]
            for h in range(2):
                attn = anp.tile([64, 1024], FP16, name=f"{name}_attn")
                for c2 in range(2):
                    ps = pool.tile([128, 512], FP32, name=f"{name}_bc",
                                   tag=tag)
                    nc.tensor.matmul(
                        ps[:], ones2[0:1, :],
                        rec[h][0:1, 1024 * q2 + 512 * c2:
                               1024 * q2 + 512 * c2 + 512],
                        start=True, stop=True)
                    with nc.allow_low_precision(reason="attnorm fp16"):
                        nc.vector.tensor_mul(
                            attn[:, 512 * c2:512 * c2 + 512],
                            comb[h][0:64, 1024 * q2 + 512 * c2:
                                    1024 * q2 + 512 * c2 + 512],
                            ps[0:64, :])
                nc.sync.dma_start(out=contrib[64 * h:64 * h + 64, :],
                                  in_=attn[:])
            nc.gpsimd.collective_compute(
                "AllGather", mybir.AluOpType.bypass, replica_groups=GROUPS,
                ins=[contrib.opt()], outs=[gathered.opt()],
            )
        return flush

    pending = None
    for q2 in range(nQT2):
        psum_o = [ps_o.tile([65, 1024], FP32, name=f"{name}_o{h}")
                  for h in range(2)]
        for half in range(2):
            qs = 512 * half
            nkb_h = 8 * q2 + 4 * (half + 1)
            pend = None
            for kb in range(nkb_h):
                cur = emit_scores(q2, qs, kb)
                if pending is not None and kb == 4:
                    pending()
                    pending = None
                if pend is not None:
                    p_sb, t0 = pend
                    for h in range(2):
                        emit_pv(h, kb - 1, nkb_h, qs, psum_o[h], p_sb, t0)
                pend = cur
            p_sb, t0 = pend
            for h in range(2):
                emit_pv(h, nkb_h - 1, nkb_h, qs, psum_o[h], p_sb, t0)
        for h in range(2):
            # one copy evicts both attT rows and the denominator row
            with nc.allow_low_precision(reason="att fp16"):
                nc.vector.tensor_copy(
                    comb[h][:, 1024 * q2:1024 * q2 + 1024], psum_o[h][:])
            # reciprocal in [128, 8] layout (DMA reshape there and back)
            dh = dnp.tile([128, 8], FP16, name=f"{name}_dh")
            nc.sync.dma_start(
                out=dh[:], in_=comb[h][64:65, 1024 * q2:1024 * q2 + 1024])
            rc = dnp.tile([128, 8], FP16, name=f"{name}_rc")
            with nc.allow_low_precision(reason="softmax denom fp16"):
                nc.vector.reciprocal(rc[:], dh[:])
            nc.sync.dma_start(
                out=rec[h][0:1, 1024 * q2:1024 * q2 + 1024], in_=rc[:])
        pending = make_norm(q2)
    return pending


def build_program():
    nc = bacc.Bacc(None, target_bir_lowering=False)

    def inp(name, shape, dt=FP16):
        return nc.declare_dram_parameter(name, list(shape), dt, isOutput=False)

    # data
    xlt = inp("xlt", [4, 128, T])            # x[b,:,:512].T chunks
    xct = inp("xct", [32, 128, Tc])          # x[b].reshape(Tc,4096).T chunks
    # weights
    wq = inp("wq", [4, 128, 128])            # [qA|qB] lhsT chunks (scaled 1/8)
    bq = inp("bq", [128, 1], FP32)
    wk = inp("wk", [4, 128, 128])
    bk = inp("bk", [128, 1], FP32)
    wv = inp("wv", [4, 128, 128])            # [vA|vB] (rhs for v-direct)
    bvb = inp("bvb", [128, 128])             # [bvA|bvB] replicated to 128 rows
    wgq = inp("wgq", [4, 128, 128])
    bgq = inp("bgq", [128, 1], FP32)
    wgk = inp("wgk", [4, 128, 128])
    bgk = inp("bgk", [128, 1], FP32)
    wgv = inp("wgv", [4, 128, 128])
    bgvb = inp("bgvb", [128, 128])
    wc = inp("wc", [32, 128, 128])           # compress slice lhsT chunks
    bc = inp("bc", [128, 1], FP32)
    wplz = inp("wplz", [4, 128, 129])        # [w_lproj[:,cs] | u_l] chunks
    bplzb = inp("bplzb", [128, 129])         # [b_lproj[cs] | c0] replicated
    wpgz = inp("wpgz", [4, 128, 129])
    bpgzb = inp("bpgzb", [128, 129])
    repA = inp("repA", [128, 128])           # x4 expander (even 32-blocks)
    repB = inp("repB", [128, 128])           # x4 expander (odd 32-blocks)
    maskt = inp("maskt", [128, 128])         # strict lower-tri NEG
    out_loc = nc.declare_dram_parameter("out_loc", [T, 128], FP16, isOutput=True)
    out_glob = nc.declare_dram_parameter("out_glob", [T, 128], FP16, isOutput=True)

    with tile.TileContext(nc) as tc:
      with ExitStack() as top:
        dram = top.enter_context(tc.tile_pool(name="dram", bufs=1, space="DRAM"))
        const = top.enter_context(tc.tile_pool(name="const", bufs=1))
        persist = top.enter_context(tc.tile_pool(name="persist", bufs=1))

        # constants (DMA issues for these are deferred into phase B so the
        # first qkv matmul's inputs go out on the queue first)
        ident = const.tile([128, 128], FP16, name="ident")
        make_identity(nc, ident[:])
        mask_tri = const.tile([128, 128], FP16, name="mask_tri")
        repA_sb = const.tile([128, 128], FP16, name="repA_sb")
        repB_sb = const.tile([128, 128], FP16, name="repB_sb")
        ones2 = const.tile([1, 128], FP16, name="ones2")
        nc.gpsimd.memset(ones2[:], 1.0)
        consts = {"mask_tri": mask_tri, "ident": ident}
        biases = {nm: const.tile([128, 1], FP32, name=f"cb_{nm}")
                  for nm in ("bq", "bk", "bgq", "bgk", "bc")}
        brows = {nm: const.tile([128, w], FP16, name=f"br_{nm}")
                 for nm, w in (("bvb", 128), ("bgvb", 128),
                               ("bplzb", 129), ("bpgzb", 129))}

        def load_consts():
            nc.sync.dma_start(out=biases["bq"][:], in_=bq[:])
            nc.sync.dma_start(out=biases["bk"][:], in_=bk[:])
            nc.sync.dma_start(out=mask_tri[:], in_=maskt[:])
            nc.sync.dma_start(out=brows["bvb"][:], in_=bvb[:])
            for t, src in ((biases["bgq"], bgq), (biases["bgk"], bgk),
                           (biases["bc"], bc), (brows["bgvb"], bgvb),
                           (brows["bplzb"], bplzb), (brows["bpgzb"], bpgzb)):
                nc.sync.dma_start(out=t[:], in_=src[:])
            nc.sync.dma_start(out=repA_sb[:], in_=repA[:])
            nc.sync.dma_start(out=repB_sb[:], in_=repB[:])

        # persistent attention inputs.  kT is stored per head with the other
        # head's partition half zeroed so score matmuls run at K=128 (no PE
        # tiling-mode switches, FWL-eligible weight loads).
        qT_l = persist.tile([128, T], FP16, name="qT_l")
        kT_lh = [persist.tile([128, T], FP16, name=f"kT_l{h}") for h in range(2)]
        qT_g = persist.tile([128, Tc], FP16, name="qT_g")
        kT_gh = [persist.tile([128, Tc], FP16, name=f"kT_g{h}") for h in range(2)]
        nc.gpsimd.memset(kT_lh[0][64:128, :], 0.0)
        nc.gpsimd.memset(kT_lh[1][0:64, :], 0.0)
        nc.gpsimd.memset(kT_gh[0][64:128, :], 0.0)
        nc.gpsimd.memset(kT_gh[1][0:64, :], 0.0)
        v_sb_l = [persist.tile([128, 130], FP16, name=f"vsb{i}")
                  for i in range(32)]
        v_sb_g = [persist.tile([128, 130], FP16, name=f"vgsb{i}")
                  for i in range(8)]
        cg_all = [persist.tile([128, Tc], FP16, name=f"cg_all{i}")
                  for i in range(4)]
        # ones columns for the PV denominator row (cols 64 and 129)
        for v_tiles in (v_sb_l, v_sb_g):
            for vt in v_tiles:
                nc.vector.memset(vt[:, 64:65], 1.0)
                nc.vector.memset(vt[:, 129:130], 1.0)

        # ------------------------------------------------------ phase B: local qkv
        pab = top.enter_context(ExitStack())
        with ExitStack() as pb:
            xlp = pb.enter_context(tc.tile_pool(name="xlt_pool", bufs=1))
            wqp = pb.enter_context(tc.tile_pool(name="wq_pool", bufs=1))
            psB = pb.enter_context(tc.tile_pool(name="psB", bufs=3, space="PSUM"))
            psV = pb.enter_context(tc.tile_pool(name="psV", bufs=2, space="PSUM"))

            # weights first (tiny transfers), then the big x stream: the
            # first qkv matmul only waits on wq + xlt[0]
            wq_sb, wk_sb, wv_sb = [], [], []
            for ch in range(4):
                for (lst, src, nm) in ((wq_sb, wq, "wq"), (wk_sb, wk, "wk"),
                                       (wv_sb, wv, "wv")):
                    t = wqp.tile([128, 128], FP16, name=f"{nm}{ch}")
                    nc.sync.dma_start(out=t[:], in_=src[ch])
                    lst.append(t)
            xlt_sb = []
            for ch in range(4):
                xt = xlp.tile([128, T], FP16, name=f"xlt{ch}")
                nc.sync.dma_start(out=xt[:], in_=xlt[ch])
                xlt_sb.append(xt)
            load_consts()

            # q and k: packed [headA|headB] out dims -> direct eviction
            kT_st = xlp.tile([128, T], FP16, name="kT_st")
            for (wsb, dst, bias_ap) in ((wq_sb, qT_l, biases["bq"][:]),
                                        (wk_sb, kT_st, biases["bk"][:])):
                for qt in range(8):
                    ps = psB.tile([128, 512], FP32, name="psB_t")
                    for ch in range(4):
                        nc.tensor.matmul(
                            ps[:], wsb[ch][:],
                            xlt_sb[ch][:, 512 * qt:512 * qt + 512],
                            start=(ch == 0), stop=(ch == 3))
                    with nc.allow_low_precision(reason="qk fp16"):
                        nc.scalar.activation(dst[:, 512 * qt:512 * qt + 512],
                                             ps[:], AF.Identity, bias=bias_ap)
            # split k into per-head zero-padded tiles (partition-preserving)
            nc.sync.dma_start(out=kT_lh[0][0:64, :], in_=kT_st[0:64, :])
            nc.sync.dma_start(out=kT_lh[1][64:128, :], in_=kT_st[64:128, :])

            # v: direct [token, vdim] layout, bias added at eviction
            bvb3 = brows["bvb"].rearrange("p (h c) -> p h c", h=2, c=64)
            for tb in range(32):
                ps = psV.tile([128, 128], FP32, name="psV_t")
                for ch in range(4):
                    nc.tensor.matmul(
                        ps[:], xlt_sb[ch][:, 128 * tb:128 * tb + 128],
                        wv_sb[ch][:], start=(ch == 0), stop=(ch == 3))
                v3 = v_sb_l[tb].rearrange("p (h c) -> p h c", h=2, c=65)
                p3 = ps.rearrange("p (h c) -> p h c", h=2, c=64)
                with nc.allow_low_precision(reason="v fp16"):
                    nc.vector.tensor_add(v3[:, :, 0:64], p3[:], bvb3[:])

        # ------------------------------------------------------ phase A: compress
        xp = pab.enter_context(tc.tile_pool(name="xct_pool", bufs=8))
        wp = pab.enter_context(tc.tile_pool(name="wc_pool", bufs=8))
        cgp = pab.enter_context(tc.tile_pool(name="cg_pool", bufs=1))
        psA = pab.enter_context(tc.tile_pool(name="psA", bufs=1, space="PSUM"))
        cgT = cgp.tile([128, Tc], FP16, name="cgT")
        ps0 = psA.tile([128, 512], FP32, name="psA_0")
        ps1 = psA.tile([128, 512], FP32, name="psA_1")
        for ch in range(32):
            wt = wp.tile([128, 128], FP16, name="wc_t")
            nc.sync.dma_start(out=wt[:], in_=wc[ch])
            xt = xp.tile([128, Tc], FP16, name="xct_t")
            nc.sync.dma_start(out=xt[:], in_=xct[ch])
            nc.tensor.matmul(ps0[:], wt[:], xt[:, 0:512],
                             start=(ch == 0), stop=(ch == 31))
            nc.tensor.matmul(ps1[:], wt[:], xt[:, 512:1024],
                             start=(ch == 0), stop=(ch == 31))
        with nc.allow_low_precision(reason="cg fp16"):
            nc.scalar.activation(cgT[:, 0:512], ps0[:],
                                 AF.Identity, bias=biases["bc"][:])
            nc.scalar.activation(cgT[:, 512:1024], ps1[:],
                                 AF.Identity, bias=biases["bc"][:])
        cg_contrib = dram.tile([128, Tc], FP16, name="cg_contrib")
        cg_gathered = dram.tile([512, Tc], FP16, name="cg_gathered")
        nc.sync.dma_start(out=cg_contrib[:], in_=cgT[:])
        nc.gpsimd.collective_compute(
            "AllGather", mybir.AluOpType.bypass, replica_groups=GROUPS,
            ins=[cg_contrib.opt()], outs=[cg_gathered.opt()],
        )
        for i in range(4):
            nc.sync.dma_start(out=cg_all[i][:],
                              in_=cg_gathered[128 * i:128 * i + 128, :])
        pab.close()

        # ------------------------------------------------------ phase C: attention
        cpool = top.enter_context(tc.tile_pool(name="c_pool", bufs=1))
        comb_l = [cpool.tile([65, T], FP16, name=f"comb_l{h}") for h in range(2)]
        comb_g = [cpool.tile([65, Tc], FP16, name=f"comb_g{h}") for h in range(2)]
        rec_l = [cpool.tile([1, T], FP16, name=f"rec_l{h}") for h in range(2)]
        rec_g = [cpool.tile([1, Tc], FP16, name=f"rec_g{h}") for h in range(2)]

        contribs_l = [dram.tile([128, Tc], FP16, name=f"attnl_c{i}")
                      for i in range(4)]
        gathereds_l = [dram.tile([512, Tc], FP16, name=f"attnl_g{i}")
                       for i in range(4)]
        contrib_g = dram.tile([128, Tc], FP16, name="attng_contrib")
        gathered_g = dram.tile([512, Tc], FP16, name="attng_gathered")

        app = top.enter_context(tc.tile_pool(name="attall_pool", bufs=1))
        att_all = [app.tile([128, T], FP16, name=f"attall{i}") for i in range(4)]
        anp_top = top.enter_context(tc.tile_pool(name="anp_top", bufs=2))

        pc1 = top.enter_context(ExitStack())
        pend_l = _attention(nc, tc, pc1, "la", 4, qT_l, kT_lh, v_sb_l, comb_l,
                            rec_l, consts, ones2, contribs_l, gathereds_l,
                            anp_top)
        pc1.close()
        # att_all chunk DMAs for the already-gathered q2 chunks
        for q2 in range(3):
            for i in range(4):
                nc.gpsimd.dma_start(
                    out=att_all[i][:, 1024 * q2:1024 * q2 + 1024],
                    in_=gathereds_l[q2][128 * i:128 * i + 128, :])

        # global qkv emitted before the last local normalize chunk so the PE
        # stream never stalls on the reciprocal/bcast chain
        with ExitStack() as pg:
            wgp = pg.enter_context(tc.tile_pool(name="wg_pool", bufs=1))
            psG = pg.enter_context(tc.tile_pool(name="psG", bufs=3, space="PSUM"))
            psGV = pg.enter_context(tc.tile_pool(name="psGV", bufs=2, space="PSUM"))
            wgq_sb, wgk_sb, wgv_sb = [], [], []
            for ch in range(4):
                for (lst, src, nm) in ((wgq_sb, wgq, "wgq"), (wgk_sb, wgk, "wgk"),
                                       (wgv_sb, wgv, "wgv")):
                    t = wgp.tile([128, 128], FP16, name=f"{nm}{ch}")
                    nc.sync.dma_start(out=t[:], in_=src[ch])
                    lst.append(t)
            kTg_st = wgp.tile([128, Tc], FP16, name="kTg_st")
            for (wsb, dst, bias_ap) in ((wgq_sb, qT_g, biases["bgq"][:]),
                                        (wgk_sb, kTg_st, biases["bgk"][:])):
                for qt in range(2):
                    ps = psG.tile([128, 512], FP32, name="psG_t")
                    for ch in range(4):
                        nc.tensor.matmul(
                            ps[:], wsb[ch][:],
                            cg_all[ch][:, 512 * qt:512 * qt + 512],
                            start=(ch == 0), stop=(ch == 3))
                    with nc.allow_low_precision(reason="gqk fp16"):
                        nc.scalar.activation(dst[:, 512 * qt:512 * qt + 512],
                                             ps[:], AF.Identity, bias=bias_ap)
            nc.sync.dma_start(out=kT_gh[0][0:64, :], in_=kTg_st[0:64, :])
            nc.sync.dma_start(out=kT_gh[1][64:128, :], in_=kTg_st[64:128, :])
            # flush the last local normalize chunk now that the PE has
            # independent global-qkv work queued ahead of it
            pend_l(psG, "psG_t")
            for i in range(4):
                nc.gpsimd.dma_start(
                    out=att_all[i][:, 3072:4096],
                    in_=gathereds_l[3][128 * i:128 * i + 128, :])
            bgvb3 = brows["bgvb"].rearrange("p (h c) -> p h c", h=2, c=64)
            for tb in range(8):
                ps = psGV.tile([128, 128], FP32, name="psGV_t")
                for ch in range(4):
                    nc.tensor.matmul(
                        ps[:], cg_all[ch][:, 128 * tb:128 * tb + 128],
                        wgv_sb[ch][:], start=(ch == 0), stop=(ch == 3))
                v3 = v_sb_g[tb].rearrange("p (h c) -> p h c", h=2, c=65)
                p3 = ps.rearrange("p (h c) -> p h c", h=2, c=64)
                with nc.allow_low_precision(reason="gv fp16"):
                    nc.vector.tensor_add(v3[:, :, 0:64], p3[:], bgvb3[:])

        pc2 = top.enter_context(ExitStack())
        pend_g = _attention(nc, tc, pc2, "ga", 1, qT_g, kT_gh, v_sb_g, comb_g,
                            rec_g, consts, ones2, [contrib_g], [gathered_g],
                            anp_top)
        pc2.close()

        # ------------------------------------------------------ phase D: proj+gate
        with ExitStack() as pd:
            wpp = pd.enter_context(tc.tile_pool(name="wp_pool", bufs=1))
            psP = pd.enter_context(tc.tile_pool(name="psP", bufs=3, space="PSUM"))
            psE = pd.enter_context(tc.tile_pool(name="psE", bufs=2, space="PSUM"))
            gp = pd.enter_context(tc.tile_pool(name="gproj_pool", bufs=1))
            zp = pd.enter_context(tc.tile_pool(name="z_pool", bufs=1))
            outp = pd.enter_context(tc.tile_pool(name="out_pool", bufs=4))

            wplz_sb, wpgz_sb = [], []
            for ch in range(4):
                t = wpp.tile([128, 129], FP16, name=f"wplz{ch}")
                nc.sync.dma_start(out=t[:], in_=wplz[ch])
                wplz_sb.append(t)
                t = wpp.tile([128, 129], FP16, name=f"wpgz{ch}")
                nc.sync.dma_start(out=t[:], in_=wpgz[ch])
                wpgz_sb.append(t)

            # local proj: [128 tok, 129] blocks -> loc_sb
            loc_sb = gp.tile([128, 32 * 129], FP16, name="loc_sb")
            loc3 = loc_sb.rearrange("p (a b) -> p a b", a=32, b=129)
            for tb in range(32):
                ps = psP.tile([128, 129], FP32, name="psP_t")
                for ch in range(4):
                    nc.tensor.matmul(ps[:],
                                     att_all[ch][:, 128 * tb:128 * tb + 128],
                                     wplz_sb[ch][:], start=(ch == 0), stop=(ch == 3))
                with nc.allow_low_precision(reason="proj fp16"):
                    nc.vector.tensor_add(loc3[:, tb, :], ps[:], brows["bplzb"][:])

            # global normalize + gather overlaps the local proj matmuls
            pend_g(psP, "psP_t")
            attg_all = []
            for i in range(4):
                t = wpp.tile([128, Tc], FP16, name=f"attgall{i}")
                nc.gpsimd.dma_start(out=t[:],
                                    in_=gathered_g[128 * i:128 * i + 128, :])
                attg_all.append(t)

            # global proj (Tc rows) -> gproj_sb, then x4 expand -> ge_sb
            gproj_sb = gp.tile([128, 8 * 129], FP16, name="gproj_sb")
            gproj3 = gproj_sb.rearrange("p (a b) -> p a b", a=8, b=129)
            for tbg in range(8):
                ps = psP.tile([128, 129], FP32, name="psP_t")
                for ch in range(4):
                    nc.tensor.matmul(ps[:],
                                     attg_all[ch][:, 128 * tbg:128 * tbg + 128],
                                     wpgz_sb[ch][:], start=(ch == 0), stop=(ch == 3))
                with nc.allow_low_precision(reason="gproj fp16"):
                    nc.vector.tensor_add(gproj3[:, tbg, :], ps[:], brows["bpgzb"][:])
            ge_sb = gp.tile([128, 32 * 129], FP16, name="ge_sb")
            ge3 = ge_sb.rearrange("p (a b) -> p a b", a=32, b=129)
            for tb in range(32):
                base = 64 * ((tb % 4) // 2)
                rep = repA_sb if tb % 2 == 0 else repB_sb
                ps = psE.tile([128, 129], FP32, name="psE_t")
                nc.tensor.matmul(ps[:], rep[base:base + 64, :],
                                 gproj3[base:base + 64, tb // 4, :],
                                 start=True, stop=True)
                with nc.allow_low_precision(reason="gexp fp16"):
                    nc.vector.tensor_copy(ge3[:, tb, :], ps[:])

            # gate in groups of 8 blocks: z = loc_z + ge_z;
            # g0 = 0.5 + 0.5*tanh(z/2); g1 = 1 - g0; batched out DMAs
            for grp in range(4):
                b0 = 8 * grp
                zsum = zp.tile([128, 8], FP32, name="zsum")
                nc.vector.tensor_add(zsum[:], loc3[:, b0:b0 + 8, 128],
                                     ge3[:, b0:b0 + 8, 128])
                tanh_t = zp.tile([128, 8], FP32, name="tanh_t")
                nc.scalar.activation(tanh_t[:], zsum[:], AF.Tanh, scale=0.5)
                g0 = zp.tile([128, 8], FP32, name="g0")
                g1 = zp.tile([128, 8], FP32, name="g1")
                nc.vector.tensor_scalar(g0[:], tanh_t[:], 0.5, 0.5,
                                        mybir.AluOpType.mult,
                                        mybir.AluOpType.add)
                nc.vector.tensor_scalar(g1[:], tanh_t[:], -0.5, 0.5,
                                        mybir.AluOpType.mult,
                                        mybir.AluOpType.add)
                ol = outp.tile([128, 8 * 128], FP16, name="outl")
                ol3 = ol.rearrange("p (b c) -> p b c", b=8, c=128)
                og = outp.tile([128, 8 * 128], FP16, name="outg")
                og3 = og.rearrange("p (b c) -> p b c", b=8, c=128)
                for j in range(8):
                    tb = b0 + j
                    with nc.allow_low_precision(reason="out fp16"):
                        nc.vector.tensor_scalar_mul(ol3[:, j, :],
                                                    loc3[:, tb, 0:128],
                                                    g0[:, j:j + 1])
                        nc.vector.tensor_scalar_mul(og3[:, j, :],
                                                    ge3[:, tb, 0:128],
                                                    g1[:, j:j + 1])
                out_l_v = out_loc[1024 * grp:1024 * grp + 1024, :] \
                    .rearrange("(b p) c -> p b c", b=8, p=128)
                nc.sync.dma_start(out=out_l_v, in_=ol3[:])
                out_g_v = out_glob[1024 * grp:1024 * grp + 1024, :] \
                    .rearrange("(b p) c -> p b c", b=8, p=128)
                nc.sync.dma_start(out=out_g_v, in_=og3[:])

    nc.finalize()
    return nc


# ---------------------------------------------------------------------------
# Host side
# ---------------------------------------------------------------------------

_NC_CACHE = []


def _get_program():
    if not _NC_CACHE:
        _NC_CACHE.append(build_program())
    return _NC_CACHE[0]


def _prep_inputs(x, w_lqkv, b_lqkv, w_gqkv, b_gqkv, w_comp, b_comp,
                 w_lproj, b_lproj, w_gproj, b_gproj, w_gate, b_gate):
    f32, f16 = np.float32, np.float16
    wd = (w_gate[:, 0] - w_gate[:, 1]).astype(f32)
    u_l = (w_lproj @ wd[:LD]).astype(f32)
    u_g = (w_gproj @ wd[LD:]).astype(f32)
    c0 = float(b_lproj @ wd[:LD] + b_gproj @ wd[LD:] + b_gate[0] - b_gate[1])

    mask_tri = np.where(np.arange(128)[None, :] >= np.arange(128)[:, None],
                        0.0, NEG).astype(f16)
    e0 = np.zeros((64, 128), f32)
    e0[np.arange(128) // 4, np.arange(128)] = 1.0
    e1 = np.zeros((64, 128), f32)
    e1[32 + np.arange(128) // 4, np.arange(128)] = 1.0
    repA_ = np.concatenate([e0, e0], axis=0).astype(f16)
    repB_ = np.concatenate([e1, e1], axis=0).astype(f16)

    def packed_cols(w, b, off, ha, hb, scale=1.0):
        wp = np.concatenate([w[:, off + D * ha:off + D * ha + D],
                             w[:, off + D * hb:off + D * hb + D]], axis=1) * scale
        bp = np.concatenate([b[off + D * ha:off + D * ha + D],
                             b[off + D * hb:off + D * hb + D]]) * scale
        return wp.astype(f16).reshape(4, 128, 128), bp.astype(f32).reshape(128, 1)

    in_maps = []
    for core in range(NCORES):
        b_idx, g = core // 4, core % 4
        ha, hb = 2 * g, 2 * g + 1
        cs = slice(128 * g, 128 * g + 128)

        xlt_ = np.ascontiguousarray(x[b_idx, :, :LD].T).astype(f16).reshape(4, 128, T)
        xct_ = np.ascontiguousarray(
            x[b_idx].reshape(Tc, R * E).T).astype(f16).reshape(32, 128, Tc)

        wq_, bq_ = packed_cols(w_lqkv, b_lqkv, 0, ha, hb, 1.0 / 8.0)
        wk_, bk_ = packed_cols(w_lqkv, b_lqkv, LD, ha, hb)
        wv_, bv_ = packed_cols(w_lqkv, b_lqkv, 2 * LD, ha, hb)
        wgq_, bgq_ = packed_cols(w_gqkv, b_gqkv, 0, ha, hb, 1.0 / 8.0)
        wgk_, bgk_ = packed_cols(w_gqkv, b_gqkv, LD, ha, hb)
        wgv_, bgv_ = packed_cols(w_gqkv, b_gqkv, 2 * LD, ha, hb)

        wplz_ = np.concatenate(
            [w_lproj[:, cs], u_l[:, None]], axis=1).astype(f16).reshape(4, 128, 129)
        wpgz_ = np.concatenate(
            [w_gproj[:, cs], u_g[:, None]], axis=1).astype(f16).reshape(4, 128, 129)

        in_maps.append({
            "xlt": xlt_, "xct": xct_,
            "wq": wq_, "bq": bq_, "wk": wk_, "bk": bk_,
            "wv": wv_,
            "bvb": np.tile(bv_.reshape(1, 128), (128, 1)).astype(f16),
            "wgq": wgq_, "bgq": bgq_, "wgk": wgk_, "bgk": bgk_,
            "wgv": wgv_,
            "bgvb": np.tile(bgv_.reshape(1, 128), (128, 1)).astype(f16),
            "wc": np.ascontiguousarray(
                w_comp[:, LD + 128 * g:LD + 128 * g + 128]).astype(f16)
                .reshape(32, 128, 128),
            "bc": b_comp[LD + 128 * g:LD + 128 * g + 128].astype(f32)
                .reshape(128, 1),
            "wplz": wplz_,
            "bplzb": np.tile(np.concatenate([b_lproj[cs], [c0]])
                             .reshape(1, 129), (128, 1)).astype(f16),
            "wpgz": wpgz_,
            "bpgzb": np.tile(np.concatenate([b_gproj[cs], [0.0]])
                             .reshape(1, 129), (128, 1)).astype(f16),
            "repA": repA_, "repB": repB_, "maskt": mask_tri,
        })
    return in_maps


def _run(in_maps, trace=False):
    nc = _get_program()
    return run_bass_kernel_spmd(nc, in_maps, list(range(NCORES)), trace=trace)


def assemble(results):
    out = np.empty((B, T, E), np.float32)
    for core in range(NCORES):
        b_idx, g = core // 4, core % 4
        out[b_idx, :, 128 * g:128 * g + 128] = \
            results[core]["out_loc"].astype(np.float32)
        out[b_idx, :, LD + 128 * g:LD + 128 * g + 128] = \
            results[core]["out_glob"].astype(np.float32)
    return out


def kernel(**inputs):
    in_maps = _prep_inputs(**inputs)
    res = _run(in_maps)
    return assemble(res.results)


def kernel_traced(**inputs):
    """test.py helper: returns (output, BassKernelResults with timing)."""
    in_maps = _prep_inputs(**inputs)
    res = _run(in_maps, trace=True)
    return assemble(res.results), res


# revision 52
# speedup vs baseline: 1.8262x; 1.0970x over previous
"""DualResolutionAttention Trainium2 kernel (8 NeuronCores, Bass/Tile).

Sharding: core c -> (batch b = c//4, group g = c%4).
Each core computes local heads {2g, 2g+1} and global heads {2g, 2g+1} over the
full sequence, plus the output channel slice [128g, 128g+128) of each branch.
Three AllGathers within each 4-core batch group: (1) compressed stream cgT,
(2) normalized local attention (fp16), (3) normalized global attention (fp16).

v2 design (vs baseline): fp16 compute everywhere (FWL weight loads, 2x less
DMA/SBUF), V computed directly in [token, vdim] layout (no PE transposes),
q/k evicted straight from PSUM to qT/kT (packed per-head weights), gate logits
folded into the projection matmuls as a 129th output column, masks via 64-row
identity matmuls (no PE tiling-mode switch inside attention).
"""
import os
import sys

sys.path.insert(0, "/opt/trn_rl_repo")
os.environ.setdefault("JAX_PLATFORMS", "axon,cpu")

from contextlib import ExitStack

import numpy as np

import concourse.bass as bass
import concourse.mybir as mybir
import concourse.tile as tile
from concourse import bacc
from concourse.bass_utils import run_bass_kernel_spmd
from concourse.masks import make_identity

FP32 = mybir.dt.float32
FP16 = mybir.dt.float16
AF = mybir.ActivationFunctionType

# Problem constants
B, T, E = 2, 4096, 1024
LD = 512            # local/global stream dim
D = 64              # head dim
HH = 8              # heads per branch
R = 4               # compression ratio
Tc = T // R         # 1024
NCORES = 8
GROUPS = [[0, 1, 2, 3], [4, 5, 6, 7]]

NEG = -30000.0      # fp16-safe mask value


# ---------------------------------------------------------------------------
# Program builder
# ---------------------------------------------------------------------------

def _attention(nc, tc, ctx, name, nQT2, qT, kTh, v_sb, comb, rec, consts,
               ones2, contribs, gathereds, anp):
    """Attention body: S^T layout scores, [v|ones] PV with denominator row.

    All matmuls run in 128-row tiling mode (kTh[h] is the per-head key tile
    with the other head's partition half zeroed), so the PE never pays a
    tiling-mode-switch drain.  The kb loop runs per 512-query half with a
    double-buffered [128, 1024] score PSUM tile (both heads side by side,
    one Exp per iteration), so the scores->exp WAR never stalls the PE.

    comb[h] is a [65, nQT2*1024] fp16 tile: rows 0:64 = unnormalized attT,
    row 64 = softmax denominator.  rec[h] [1, ncols] gets 1/denominator
    (computed in a [128, 8] layout via DMA reshape - reciprocal is
    8 cyc/elem on the DVE).  Each q2 chunk is normalized and AllGathered
    separately (contribs[q2] -> gathereds[q2]) so the collectives overlap
    later compute; the normalize matmuls for chunk q2 are deferred and
    emitted a few kb into chunk q2+1 (the PE never waits on the reciprocal
    chain).  Returns the last chunk's un-emitted normalize closure - the
    caller must invoke it after emitting some independent PE work.
    """
    ps_s = ctx.enter_context(tc.tile_pool(name=f"{name}_ps_s", bufs=2, space="PSUM"))
    ps_o = ctx.enter_context(tc.tile_pool(name=f"{name}_ps_o", bufs=1, space="PSUM"))
    p_pool = ctx.enter_context(tc.tile_pool(name=f"{name}_p", bufs=3))
    dnp = ctx.enter_context(tc.tile_pool(name=f"{name}_dn", bufs=2))

    mask_tri = consts["mask_tri"]
    ident = consts["ident"]

    def emit_scores(q2, qs, kb):
        t0 = max(0, 128 * kb - 1024 * q2 - qs)   # mask start within the half
        has_mask = 128 * kb >= 1024 * q2 + qs
        ps2 = ps_s.tile([128, 1024], FP32, name=f"{name}_s2", tag=f"{name}_s2")
        p_sb = p_pool.tile([128, 1024], FP16, name=f"{name}_pt")
        for h in range(2):
            nc.tensor.matmul(
                ps2[:, 512 * h:512 * h + 512],
                kTh[h][:, 128 * kb:128 * kb + 128],
                qT[:, 1024 * q2 + qs:1024 * q2 + qs + 512],
                start=True, stop=True,
            )
        if has_mask:
            for h in range(2):
                nc.tensor.matmul(
                    ps2[:, 512 * h + t0:512 * h + t0 + 128],
                    ident[:], mask_tri[:],
                    start=False, stop=True, skip_group_check=True,
                )
        if not has_mask and kb % 3 == 2:
            # Schraudolph fast exp on the (otherwise idle) DVE: write the
            # fp16 bit pattern of e^x as an int16 affine transform.
            # bits = x * 2^10/ln2 + (15*2^10 - 61); max rel err ~4%.
            with nc.allow_low_precision(reason="schraudolph exp"):
                nc.vector.tensor_scalar(
                    p_sb.bitcast(mybir.dt.int16)[:], ps2[:],
                    1477.3197, 15299.0,
                    mybir.AluOpType.mult, mybir.AluOpType.add)
        elif t0 == 0:
            nc.scalar.activation(p_sb[:], ps2[:], AF.Exp)
        else:
            p3 = p_sb.rearrange("p (h c) -> p h c", h=2, c=512)
            s3 = ps2.rearrange("p (h c) -> p h c", h=2, c=512)
            nc.scalar.activation(p3[:, :, t0:512], s3[:, :, t0:512], AF.Exp)
        return p_sb, t0

    def emit_pv(h, kb, nkb_h, qs, psum_o, p_sb, t0):
        nc.tensor.matmul(
            psum_o[:, qs + t0:qs + 512],
            v_sb[kb][:, 65 * h:65 * h + 65],
            p_sb[:, 512 * h + t0:512 * h + 512],
            start=(kb == 0), stop=(kb == nkb_h - 1),
            skip_group_check=True,
        )

    def make_norm(q2):
        def flush(pool=None, tag=None):
            pool = pool if pool is not None else ps_s
            tag = tag if tag is not None else f"{name}_s2"
            contrib, gathered = contribs[q2], gathereds[q2]
            for h in range(2):
                attn = anp.tile([64, 1024], FP16, name=f"{name}_attn")
                for c2 in range(2):
                    ps = pool.tile([128, 512], FP32, name=f"{name}_bc",
                                   tag=tag)
                    nc.tensor.matmul(
                        ps[:], ones2[0:1, :],
                        rec[h][0:1, 1024 * q2 + 512 * c2:
                               1024 * q2 + 512 * c2 + 512],
                        start=True, stop=True)
                    with nc.allow_low_precision(reason="attnorm fp16"):
                        nc.vector.tensor_mul(
                            attn[:, 512 * c2:512 * c2 + 512],
                            comb[h][0:64, 1024 * q2 + 512 * c2:
                                    1024 * q2 + 512 * c2 + 512],
                            ps[0:64, :])
                nc.sync.dma_start(out=contrib[64 * h:64 * h + 64, :],
                                  in_=attn[:])
            nc.gpsimd.collective_compute(
                "AllGather", mybir.AluOpType.bypass, replica_groups=GROUPS,
                ins=[contrib.opt()], outs=[gathered.opt()],
            )
        return flush

    pending = None
    for q2 in range(nQT2):
        psum_o = [ps_o.tile([65, 1024], FP32, name=f"{name}_o{h}")
                  for h in range(2)]
        for half in range(2):
            qs = 512 * half
            nkb_h = 8 * q2 + 4 * (half + 1)
            pend = None
            for kb in range(nkb_h):
                cur = emit_scores(q2, qs, kb)
                if pending is not None and kb == 4:
                    pending()
                    pending = None
                if pend is not None:
                    p_sb, t0 = pend
                    for h in range(2):
                        emit_pv(h, kb - 1, nkb_h, qs, psum_o[h], p_sb, t0)
                pend = cur
            p_sb, t0 = pend
            for h in range(2):
                emit_pv(h, nkb_h - 1, nkb_h, qs, psum_o[h], p_sb, t0)
        for h in range(2):
            # evict attT rows + denominator row; the two heads go to
            # different engines so the evictions run in parallel
            with nc.allow_low_precision(reason="att fp16"):
                if h == 0:
                    nc.vector.tensor_copy(
                        comb[h][:, 1024 * q2:1024 * q2 + 1024], psum_o[h][:])
                else:
                    nc.scalar.activation(
                        comb[h][:, 1024 * q2:1024 * q2 + 1024], psum_o[h][:],
                        AF.Copy)
            # reciprocal in [128, 8] layout (DMA reshape there and back);
            # the whole chain stays on the DVE queue - no sync-FIFO blocking
            dh = dnp.tile([128, 8], FP16, name=f"{name}_dh")
            nc.scalar.dma_start(
                out=dh[:], in_=comb[h][64:65, 1024 * q2:1024 * q2 + 1024])
            rc = dnp.tile([128, 8], FP16, name=f"{name}_rc")
            with nc.allow_low_precision(reason="softmax denom fp16"):
                nc.vector.reciprocal(rc[:], dh[:])
            nc.scalar.dma_start(
                out=rec[h][0:1, 1024 * q2:1024 * q2 + 1024], in_=rc[:])
        pending = make_norm(q2)
    return pending


def build_program():
    nc = bacc.Bacc(None, target_bir_lowering=False)

    def inp(name, shape, dt=FP16):
        return nc.declare_dram_parameter(name, list(shape), dt, isOutput=False)

    # data
    xlt = inp("xlt", [4, 128, T])            # x[b,:,:512].T chunks
    xct = inp("xct", [32, 128, Tc])          # x[b].reshape(Tc,4096).T chunks
    # weights
    wq = inp("wq", [4, 128, 128])            # [qA|qB] lhsT chunks (scaled 1/8)
    bq = inp("bq", [128, 1], FP32)
    wk = inp("wk", [4, 128, 128])
    bk = inp("bk", [128, 1], FP32)
    wv = inp("wv", [4, 128, 128])            # [vA|vB] (rhs for v-direct)
    bvb = inp("bvb", [128, 128])             # [bvA|bvB] replicated to 128 rows
    wgq = inp("wgq", [4, 128, 128])
    bgq = inp("bgq", [128, 1], FP32)
    wgk = inp("wgk", [4, 128, 128])
    bgk = inp("bgk", [128, 1], FP32)
    wgv = inp("wgv", [4, 128, 128])
    bgvb = inp("bgvb", [128, 128])
    wc = inp("wc", [32, 128, 128])           # compress slice lhsT chunks
    bc = inp("bc", [128, 1], FP32)
    wplz = inp("wplz", [4, 128, 129])        # [w_lproj[:,cs] | u_l] chunks
    bplzb = inp("bplzb", [128, 129])         # [b_lproj[cs] | c0] replicated
    wpgz = inp("wpgz", [4, 128, 129])
    bpgzb = inp("bpgzb", [128, 129])
    repA = inp("repA", [128, 128])           # x4 expander (even 32-blocks)
    repB = inp("repB", [128, 128])           # x4 expander (odd 32-blocks)
    maskt = inp("maskt", [128, 128])         # strict lower-tri NEG
    out_loc = nc.declare_dram_parameter("out_loc", [T, 128], FP16, isOutput=True)
    out_glob = nc.declare_dram_parameter("out_glob", [T, 128], FP16, isOutput=True)

    with tile.TileContext(nc) as tc:
      with ExitStack() as top:
        dram = top.enter_context(tc.tile_pool(name="dram", bufs=1, space="DRAM"))
        const = top.enter_context(tc.tile_pool(name="const", bufs=1))
        persist = top.enter_context(tc.tile_pool(name="persist", bufs=1))

        # constants (DMA issues for these are deferred into phase B so the
        # first qkv matmul's inputs go out on the queue first)
        ident = const.tile([128, 128], FP16, name="ident")
        make_identity(nc, ident[:])
        mask_tri = const.tile([128, 128], FP16, name="mask_tri")
        repA_sb = const.tile([128, 128], FP16, name="repA_sb")
        repB_sb = const.tile([128, 128], FP16, name="repB_sb")
        ones2 = const.tile([1, 128], FP16, name="ones2")
        nc.gpsimd.memset(ones2[:], 1.0)
        consts = {"mask_tri": mask_tri, "ident": ident}
        biases = {nm: const.tile([128, 1], FP32, name=f"cb_{nm}")
                  for nm in ("bq", "bk", "bgq", "bgk", "bc")}
        brows = {nm: const.tile([128, w], FP16, name=f"br_{nm}")
                 for nm, w in (("bvb", 128), ("bgvb", 128),
                               ("bplzb", 129), ("bpgzb", 129))}

        def load_consts():
            nc.sync.dma_start(out=biases["bq"][:], in_=bq[:])
            nc.sync.dma_start(out=biases["bk"][:], in_=bk[:])
            nc.sync.dma_start(out=mask_tri[:], in_=maskt[:])
            nc.sync.dma_start(out=brows["bvb"][:], in_=bvb[:])
            for t, src in ((biases["bgq"], bgq), (biases["bgk"], bgk),
                           (biases["bc"], bc), (brows["bgvb"], bgvb),
                           (brows["bplzb"], bplzb), (brows["bpgzb"], bpgzb)):
                nc.sync.dma_start(out=t[:], in_=src[:])
            nc.sync.dma_start(out=repA_sb[:], in_=repA[:])
            nc.sync.dma_start(out=repB_sb[:], in_=repB[:])

        # persistent attention inputs.  kT is stored per head with the other
        # head's partition half zeroed so score matmuls run at K=128 (no PE
        # tiling-mode switches, FWL-eligible weight loads).
        qT_l = persist.tile([128, T], FP16, name="qT_l")
        kT_lh = [persist.tile([128, T], FP16, name=f"kT_l{h}") for h in range(2)]
        qT_g = persist.tile([128, Tc], FP16, name="qT_g")
        kT_gh = [persist.tile([128, Tc], FP16, name=f"kT_g{h}") for h in range(2)]
        nc.gpsimd.memset(kT_lh[0][64:128, :], 0.0)
        nc.gpsimd.memset(kT_lh[1][0:64, :], 0.0)
        nc.gpsimd.memset(kT_gh[0][64:128, :], 0.0)
        nc.gpsimd.memset(kT_gh[1][0:64, :], 0.0)
        v_sb_l = [persist.tile([128, 130], FP16, name=f"vsb{i}")
                  for i in range(32)]
        v_sb_g = [persist.tile([128, 130], FP16, name=f"vgsb{i}")
                  for i in range(8)]
        cg_all = [persist.tile([128, Tc], FP16, name=f"cg_all{i}")
                  for i in range(4)]
        # ones columns for the PV denominator row (cols 64 and 129)
        for v_tiles in (v_sb_l, v_sb_g):
            for vt in v_tiles:
                nc.vector.memset(vt[:, 64:65], 1.0)
                nc.vector.memset(vt[:, 129:130], 1.0)

        # ------------------------------------------------------ phase B: local qkv
        pab = top.enter_context(ExitStack())
        with ExitStack() as pb:
            xlp = pb.enter_context(tc.tile_pool(name="xlt_pool", bufs=1))
            wqp = pb.enter_context(tc.tile_pool(name="wq_pool", bufs=1))
            psB = pb.enter_context(tc.tile_pool(name="psB", bufs=3, space="PSUM"))
            psV = pb.enter_context(tc.tile_pool(name="psV", bufs=2, space="PSUM"))

            # weights first (tiny transfers), then the big x stream: the
            # first qkv matmul only waits on wq + xlt[0]
            wq_sb, wk_sb, wv_sb = [], [], []
            for ch in range(4):
                for (lst, src, nm) in ((wq_sb, wq, "wq"), (wk_sb, wk, "wk"),
                                       (wv_sb, wv, "wv")):
                    t = wqp.tile([128, 128], FP16, name=f"{nm}{ch}")
                    nc.sync.dma_start(out=t[:], in_=src[ch])
                    lst.append(t)
            xlt_sb = []
            for ch in range(4):
                xt = xlp.tile([128, T], FP16, name=f"xlt{ch}")
                nc.sync.dma_start(out=xt[:], in_=xlt[ch])
                xlt_sb.append(xt)
            load_consts()

            # q and k: packed [headA|headB] out dims -> direct eviction
            kT_st = xlp.tile([128, T], FP16, name="kT_st")
            for (wsb, dst, bias_ap) in ((wq_sb, qT_l, biases["bq"][:]),
                                        (wk_sb, kT_st, biases["bk"][:])):
                for qt in range(8):
                    ps = psB.tile([128, 512], FP32, name="psB_t")
                    for ch in range(4):
                        nc.tensor.matmul(
                            ps[:], wsb[ch][:],
                            xlt_sb[ch][:, 512 * qt:512 * qt + 512],
                            start=(ch == 0), stop=(ch == 3))
                    with nc.allow_low_precision(reason="qk fp16"):
                        nc.scalar.activation(dst[:, 512 * qt:512 * qt + 512],
                                             ps[:], AF.Identity, bias=bias_ap)
            # split k into per-head zero-padded tiles (partition-preserving)
            nc.sync.dma_start(out=kT_lh[0][0:64, :], in_=kT_st[0:64, :])
            nc.sync.dma_start(out=kT_lh[1][64:128, :], in_=kT_st[64:128, :])

            # v: direct [token, vdim] layout, bias added at eviction
            bvb3 = brows["bvb"].rearrange("p (h c) -> p h c", h=2, c=64)
            for tb in range(32):
                ps = psV.tile([128, 128], FP32, name="psV_t")
                for ch in range(4):
                    nc.tensor.matmul(
                        ps[:], xlt_sb[ch][:, 128 * tb:128 * tb + 128],
                        wv_sb[ch][:], start=(ch == 0), stop=(ch == 3))
                v3 = v_sb_l[tb].rearrange("p (h c) -> p h c", h=2, c=65)
                p3 = ps.rearrange("p (h c) -> p h c", h=2, c=64)
                with nc.allow_low_precision(reason="v fp16"):
                    nc.vector.tensor_add(v3[:, :, 0:64], p3[:], bvb3[:])

        # ------------------------------------------------------ phase A: compress
        xp = pab.enter_context(tc.tile_pool(name="xct_pool", bufs=8))
        wp = pab.enter_context(tc.tile_pool(name="wc_pool", bufs=8))
        cgp = pab.enter_context(tc.tile_pool(name="cg_pool", bufs=1))
        psA = pab.enter_context(tc.tile_pool(name="psA", bufs=1, space="PSUM"))
        cgT = cgp.tile([128, Tc], FP16, name="cgT")
        ps0 = psA.tile([128, 512], FP32, name="psA_0")
        ps1 = psA.tile([128, 512], FP32, name="psA_1")
        for ch in range(32):
            wt = wp.tile([128, 128], FP16, name="wc_t")
            nc.sync.dma_start(out=wt[:], in_=wc[ch])
            xt = xp.tile([128, Tc], FP16, name="xct_t")
            nc.sync.dma_start(out=xt[:], in_=xct[ch])
            nc.tensor.matmul(ps0[:], wt[:], xt[:, 0:512],
                             start=(ch == 0), stop=(ch == 31))
            nc.tensor.matmul(ps1[:], wt[:], xt[:, 512:1024],
                             start=(ch == 0), stop=(ch == 31))
        with nc.allow_low_precision(reason="cg fp16"):
            nc.scalar.activation(cgT[:, 0:512], ps0[:],
                                 AF.Identity, bias=biases["bc"][:])
            nc.scalar.activation(cgT[:, 512:1024], ps1[:],
                                 AF.Identity, bias=biases["bc"][:])
        cg_contrib = dram.tile([128, Tc], FP16, name="cg_contrib")
        cg_gathered = dram.tile([512, Tc], FP16, name="cg_gathered")
        nc.sync.dma_start(out=cg_contrib[:], in_=cgT[:])
        nc.gpsimd.collective_compute(
            "AllGather", mybir.AluOpType.bypass, replica_groups=GROUPS,
            ins=[cg_contrib.opt()], outs=[cg_gathered.opt()],
        )
        for i in range(4):
            nc.sync.dma_start(out=cg_all[i][:],
                              in_=cg_gathered[128 * i:128 * i + 128, :])
        pab.close()

        # ------------------------------------------------------ phase C: attention
        cpool = top.enter_context(tc.tile_pool(name="c_pool", bufs=1))
        comb_l = [cpool.tile([65, T], FP16, name=f"comb_l{h}") for h in range(2)]
        comb_g = [cpool.tile([65, Tc], FP16, name=f"comb_g{h}") for h in range(2)]
        rec_l = [cpool.tile([1, T], FP16, name=f"rec_l{h}") for h in range(2)]
        rec_g = [cpool.tile([1, Tc], FP16, name=f"rec_g{h}") for h in range(2)]

        contribs_l = [dram.tile([128, Tc], FP16, name=f"attnl_c{i}")
                      for i in range(4)]
        gathereds_l = [dram.tile([512, Tc], FP16, name=f"attnl_g{i}")
                       for i in range(4)]
        contrib_g = dram.tile([128, Tc], FP16, name="attng_contrib")
        gathered_g = dram.tile([512, Tc], FP16, name="attng_gathered")

        app = top.enter_context(tc.tile_pool(name="attall_pool", bufs=1))
        att_all = [app.tile([128, T], FP16, name=f"attall{i}") for i in range(4)]
        anp_top = top.enter_context(tc.tile_pool(name="anp_top", bufs=2))

        pc1 = top.enter_context(ExitStack())
        pend_l = _attention(nc, tc, pc1, "la", 4, qT_l, kT_lh, v_sb_l, comb_l,
                            rec_l, consts, ones2, contribs_l, gathereds_l,
                            anp_top)
        pc1.close()
        # att_all chunk DMAs for the already-gathered q2 chunks
        for q2 in range(3):
            for i in range(4):
                nc.gpsimd.dma_start(
                    out=att_all[i][:, 1024 * q2:1024 * q2 + 1024],
                    in_=gathereds_l[q2][128 * i:128 * i + 128, :])

        # global qkv emitted before the last local normalize chunk so the PE
        # stream never stalls on the reciprocal/bcast chain
        with ExitStack() as pg:
            wgp = pg.enter_context(tc.tile_pool(name="wg_pool", bufs=1))
            psG = pg.enter_context(tc.tile_pool(name="psG", bufs=3, space="PSUM"))
            psGV = pg.enter_context(tc.tile_pool(name="psGV", bufs=2, space="PSUM"))
            wgq_sb, wgk_sb, wgv_sb = [], [], []
            for ch in range(4):
                for (lst, src, nm) in ((wgq_sb, wgq, "wgq"), (wgk_sb, wgk, "wgk"),
                                       (wgv_sb, wgv, "wgv")):
                    t = wgp.tile([128, 128], FP16, name=f"{nm}{ch}")
                    nc.sync.dma_start(out=t[:], in_=src[ch])
                    lst.append(t)
            kTg_st = wgp.tile([128, Tc], FP16, name="kTg_st")
            for (wsb, dst, bias_ap) in ((wgq_sb, qT_g, biases["bgq"][:]),
                                        (wgk_sb, kTg_st, biases["bgk"][:])):
                for qt in range(2):
                    ps = psG.tile([128, 512], FP32, name="psG_t")
                    for ch in range(4):
                        nc.tensor.matmul(
                            ps[:], wsb[ch][:],
                            cg_all[ch][:, 512 * qt:512 * qt + 512],
                            start=(ch == 0), stop=(ch == 3))
                    with nc.allow_low_precision(reason="gqk fp16"):
                        nc.scalar.activation(dst[:, 512 * qt:512 * qt + 512],
                                             ps[:], AF.Identity, bias=bias_ap)
            nc.sync.dma_start(out=kT_gh[0][0:64, :], in_=kTg_st[0:64, :])
            nc.sync.dma_start(out=kT_gh[1][64:128, :], in_=kTg_st[64:128, :])
            # flush the last local normalize chunk now that the PE has
            # independent global-qkv work queued ahead of it
            pend_l(psG, "psG_t")
            for i in range(4):
                nc.gpsimd.dma_start(
                    out=att_all[i][:, 3072:4096],
                    in_=gathereds_l[3][128 * i:128 * i + 128, :])
            bgvb3 = brows["bgvb"].rearrange("p (h c) -> p h c", h=2, c=64)
            for tb in range(8):
                ps = psGV.tile([128, 128], FP32, name="psGV_t")
                for ch in range(4):
                    nc.tensor.matmul(
                        ps[:], cg_all[ch][:, 128 * tb:128 * tb + 128],
                        wgv_sb[ch][:], start=(ch == 0), stop=(ch == 3))
                v3 = v_sb_g[tb].rearrange("p (h c) -> p h c", h=2, c=65)
                p3 = ps.rearrange("p (h c) -> p h c", h=2, c=64)
                with nc.allow_low_precision(reason="gv fp16"):
                    nc.vector.tensor_add(v3[:, :, 0:64], p3[:], bgvb3[:])

        pc2 = top.enter_context(ExitStack())
        pend_g = _attention(nc, tc, pc2, "ga", 1, qT_g, kT_gh, v_sb_g, comb_g,
                            rec_g, consts, ones2, [contrib_g], [gathered_g],
                            anp_top)
        pc2.close()

        # ------------------------------------------------------ phase D: proj+gate
        with ExitStack() as pd:
            wpp = pd.enter_context(tc.tile_pool(name="wp_pool", bufs=1))
            psP = pd.enter_context(tc.tile_pool(name="psP", bufs=3, space="PSUM"))
            psE = pd.enter_context(tc.tile_pool(name="psE", bufs=2, space="PSUM"))
            gp = pd.enter_context(tc.tile_pool(name="gproj_pool", bufs=1))
            zp = pd.enter_context(tc.tile_pool(name="z_pool", bufs=1))
            outp = pd.enter_context(tc.tile_pool(name="out_pool", bufs=4))

            wplz_sb, wpgz_sb = [], []
            for ch in range(4):
                t = wpp.tile([128, 129], FP16, name=f"wplz{ch}")
                nc.sync.dma_start(out=t[:], in_=wplz[ch])
                wplz_sb.append(t)
                t = wpp.tile([128, 129], FP16, name=f"wpgz{ch}")
                nc.sync.dma_start(out=t[:], in_=wpgz[ch])
                wpgz_sb.append(t)

            # global normalize first so the attn_g gather starts ASAP and
            # overlaps the local proj matmuls
            pend_g(psP, "psP_t")

            # local proj: [128 tok, 129] blocks -> loc_sb
            loc_sb = gp.tile([128, 32 * 129], FP16, name="loc_sb")
            loc3 = loc_sb.rearrange("p (a b) -> p a b", a=32, b=129)
            for tb in range(32):
                ps = psP.tile([128, 129], FP32, name="psP_t")
                for ch in range(4):
                    nc.tensor.matmul(ps[:],
                                     att_all[ch][:, 128 * tb:128 * tb + 128],
                                     wplz_sb[ch][:], start=(ch == 0), stop=(ch == 3))
                with nc.allow_low_precision(reason="proj fp16"):
                    nc.vector.tensor_add(loc3[:, tb, :], ps[:], brows["bplzb"][:])
            attg_all = []
            for i in range(4):
                t = wpp.tile([128, Tc], FP16, name=f"attgall{i}")
                nc.gpsimd.dma_start(out=t[:],
                                    in_=gathered_g[128 * i:128 * i + 128, :])
                attg_all.append(t)

            # global proj (Tc rows) -> gproj_sb, then x4 expand -> ge_sb
            gproj_sb = gp.tile([128, 8 * 129], FP16, name="gproj_sb")
            gproj3 = gproj_sb.rearrange("p (a b) -> p a b", a=8, b=129)
            for tbg in range(8):
                ps = psP.tile([128, 129], FP32, name="psP_t")
                for ch in range(4):
                    nc.tensor.matmul(ps[:],
                                     attg_all[ch][:, 128 * tbg:128 * tbg + 128],
                                     wpgz_sb[ch][:], start=(ch == 0), stop=(ch == 3))
                with nc.allow_low_precision(reason="gproj fp16"):
                    nc.vector.tensor_add(gproj3[:, tbg, :], ps[:], brows["bpgzb"][:])
            ge_sb = gp.tile([128, 32 * 129], FP16, name="ge_sb")
            ge3 = ge_sb.rearrange("p (a b) -> p a b", a=32, b=129)
            for tb in range(32):
                base = 64 * ((tb % 4) // 2)
                rep = repA_sb if tb % 2 == 0 else repB_sb
                ps = psE.tile([128, 129], FP32, name="psE_t")
                nc.tensor.matmul(ps[:], rep[base:base + 64, :],
                                 gproj3[base:base + 64, tb // 4, :],
                                 start=True, stop=True)
                with nc.allow_low_precision(reason="gexp fp16"):
                    nc.vector.tensor_copy(ge3[:, tb, :], ps[:])

            # gate in groups of 8 blocks: z = loc_z + ge_z;
            # g0 = 0.5 + 0.5*tanh(z/2); g1 = 1 - g0; batched out DMAs
            for grp in range(4):
                b0 = 8 * grp
                zsum = zp.tile([128, 8], FP32, name="zsum")
                nc.vector.tensor_add(zsum[:], loc3[:, b0:b0 + 8, 128],
                                     ge3[:, b0:b0 + 8, 128])
                tanh_t = zp.tile([128, 8], FP32, name="tanh_t")
                nc.scalar.activation(tanh_t[:], zsum[:], AF.Tanh, scale=0.5)
                g0 = zp.tile([128, 8], FP32, name="g0")
                g1 = zp.tile([128, 8], FP32, name="g1")
                nc.vector.tensor_scalar(g0[:], tanh_t[:], 0.5, 0.5,
                                        mybir.AluOpType.mult,
                                        mybir.AluOpType.add)
                nc.vector.tensor_scalar(g1[:], tanh_t[:], -0.5, 0.5,
                                        mybir.AluOpType.mult,
                                        mybir.AluOpType.add)
                ol = outp.tile([128, 8 * 128], FP16, name="outl")
                ol3 = ol.rearrange("p (b c) -> p b c", b=8, c=128)
                og = outp.tile([128, 8 * 128], FP16, name="outg")
                og3 = og.rearrange("p (b c) -> p b c", b=8, c=128)
                for j in range(8):
                    tb = b0 + j
                    with nc.allow_low_precision(reason="out fp16"):
                        nc.vector.tensor_scalar_mul(ol3[:, j, :],
                                                    loc3[:, tb, 0:128],
                                                    g0[:, j:j + 1])
                        nc.vector.tensor_scalar_mul(og3[:, j, :],
                                                    ge3[:, tb, 0:128],
                                                    g1[:, j:j + 1])
                out_l_v = out_loc[1024 * grp:1024 * grp + 1024, :] \
                    .rearrange("(b p) c -> p b c", b=8, p=128)
                nc.sync.dma_start(out=out_l_v, in_=ol3[:])
                out_g_v = out_glob[1024 * grp:1024 * grp + 1024, :] \
                    .rearrange("(b p) c -> p b c", b=8, p=128)
                nc.sync.dma_start(out=out_g_v, in_=og3[:])

    nc.finalize()
    return nc


# ---------------------------------------------------------------------------
# Host side
# ---------------------------------------------------------------------------

_NC_CACHE = []


def _get_program():
    if not _NC_CACHE:
        _NC_CACHE.append(build_program())
    return _NC_CACHE[0]


def _prep_inputs(x, w_lqkv, b_lqkv, w_gqkv, b_gqkv, w_comp, b_comp,
                 w_lproj, b_lproj, w_gproj, b_gproj, w_gate, b_gate):
    f32, f16 = np.float32, np.float16
    wd = (w_gate[:, 0] - w_gate[:, 1]).astype(f32)
    u_l = (w_lproj @ wd[:LD]).astype(f32)
    u_g = (w_gproj @ wd[LD:]).astype(f32)
    c0 = float(b_lproj @ wd[:LD] + b_gproj @ wd[LD:] + b_gate[0] - b_gate[1])

    mask_tri = np.where(np.arange(128)[None, :] >= np.arange(128)[:, None],
                        0.0, NEG).astype(f16)
    e0 = np.zeros((64, 128), f32)
    e0[np.arange(128) // 4, np.arange(128)] = 1.0
    e1 = np.zeros((64, 128), f32)
    e1[32 + np.arange(128) // 4, np.arange(128)] = 1.0
    repA_ = np.concatenate([e0, e0], axis=0).astype(f16)
    repB_ = np.concatenate([e1, e1], axis=0).astype(f16)

    def packed_cols(w, b, off, ha, hb, scale=1.0):
        wp = np.concatenate([w[:, off + D * ha:off + D * ha + D],
                             w[:, off + D * hb:off + D * hb + D]], axis=1) * scale
        bp = np.concatenate([b[off + D * ha:off + D * ha + D],
                             b[off + D * hb:off + D * hb + D]]) * scale
        return wp.astype(f16).reshape(4, 128, 128), bp.astype(f32).reshape(128, 1)

    in_maps = []
    for core in range(NCORES):
        b_idx, g = core // 4, core % 4
        ha, hb = 2 * g, 2 * g + 1
        cs = slice(128 * g, 128 * g + 128)

        xlt_ = np.ascontiguousarray(x[b_idx, :, :LD].T).astype(f16).reshape(4, 128, T)
        xct_ = np.ascontiguousarray(
            x[b_idx].reshape(Tc, R * E).T).astype(f16).reshape(32, 128, Tc)

        wq_, bq_ = packed_cols(w_lqkv, b_lqkv, 0, ha, hb, 1.0 / 8.0)
        wk_, bk_ = packed_cols(w_lqkv, b_lqkv, LD, ha, hb)
        wv_, bv_ = packed_cols(w_lqkv, b_lqkv, 2 * LD, ha, hb)
        wgq_, bgq_ = packed_cols(w_gqkv, b_gqkv, 0, ha, hb, 1.0 / 8.0)
        wgk_, bgk_ = packed_cols(w_gqkv, b_gqkv, LD, ha, hb)
        wgv_, bgv_ = packed_cols(w_gqkv, b_gqkv, 2 * LD, ha, hb)

        wplz_ = np.concatenate(
            [w_lproj[:, cs], u_l[:, None]], axis=1).astype(f16).reshape(4, 128, 129)
        wpgz_ = np.concatenate(
            [w_gproj[:, cs], u_g[:, None]], axis=1).astype(f16).reshape(4, 128, 129)

        in_maps.append({
            "xlt": xlt_, "xct": xct_,
            "wq": wq_, "bq": bq_, "wk": wk_, "bk": bk_,
            "wv": wv_,
            "bvb": np.tile(bv_.reshape(1, 128), (128, 1)).astype(f16),
            "wgq": wgq_, "bgq": bgq_, "wgk": wgk_, "bgk": bgk_,
            "wgv": wgv_,
            "bgvb": np.tile(bgv_.reshape(1, 128), (128, 1)).astype(f16),
            "wc": np.ascontiguousarray(
                w_comp[:, LD + 128 * g:LD + 128 * g + 128]).astype(f16)
                .reshape(32, 128, 128),
            "bc": b_comp[LD + 128 * g:LD + 128 * g + 128].astype(f32)
                .reshape(128, 1),
            "wplz": wplz_,
            "bplzb": np.tile(np.concatenate([b_lproj[cs], [c0]])
                             .reshape(1, 129), (128, 1)).astype(f16),
            "wpgz": wpgz_,
            "bpgzb": np.tile(np.concatenate([b_gproj[cs], [0.0]])
                             .reshape(1, 129), (128, 1)).astype(f16),
            "repA": repA_, "repB": repB_, "maskt": mask_tri,
        })
    return in_maps


def _run(in_maps, trace=False):
    nc = _get_program()
    return run_bass_kernel_spmd(nc, in_maps, list(range(NCORES)), trace=trace)


def assemble(results):
    out = np.empty((B, T, E), np.float32)
    for core in range(NCORES):
        b_idx, g = core // 4, core % 4
        out[b_idx, :, 128 * g:128 * g + 128] = \
            results[core]["out_loc"].astype(np.float32)
        out[b_idx, :, LD + 128 * g:LD + 128 * g + 128] = \
            results[core]["out_glob"].astype(np.float32)
    return out


def kernel(**inputs):
    in_maps = _prep_inputs(**inputs)
    res = _run(in_maps)
    return assemble(res.results)


def kernel_traced(**inputs):
    """test.py helper: returns (output, BassKernelResults with timing)."""
    in_maps = _prep_inputs(**inputs)
    res = _run(in_maps, trace=True)
    return assemble(res.results), res
